# revision 1
# baseline (speedup 1.0000x reference)
"""Trainium2 Bass kernel for nn_Attention_12515534700827.

Multi-head causal attention with RoPE: B=2, S=2048, D=1024, H=16, HD=64.
Sharding: 8 cores = 2 (batch) x 4 (head groups of 4 heads). Each core
computes its 4 heads' attention + its slice of the wo projection; the host
sums the 4 partial outputs per batch (the "all-reduce after wo").

v2 (bf16): all matmul operands bf16 (fp32 PSUM accumulate), inputs packed
host-side into few DMA-able tensors, deeply software-pipelined:
  - head: x/w tiles consumed it-major across 4 V psums + 4 Q/K psums while
    the input DMA streams, so the PE is never fully idle during the load
  - QK rope pipelined one chunk behind its projection (the PE swap matmul
    never waits on the ScalarE psum copy)
  - attention: per (chunk, pair), PE stream is s0,s1,pv0,s2,pv1,... with
    scores psum double-buffered; 2 heads packed per [128,1024] score tile,
    one exp per k-block via a 3D access pattern; causal diag mask added on
    the PE as a psum-accumulated [ident @ mdiagT] matmul
  - softmax denominators from a fused ones-column in the V' stationary;
    normalization = PE outer-product broadcast of the denominator row +
    reciprocal_approx_fast + multiplies, DEFERRED into the next pair's
    attention loop so nothing waits on it
  - wo(chunk c-1) tiles spread one-per-iteration through the second half
    of chunk c's p1 attention loop (PE slack absorbs them, no Act bubble);
    psum slots shared with the PV accumulators via pool tags
  - output written bf16, [S, D] per core; host sums partials in fp32

Per-core dataflow (pair = 2 heads; 2 pairs per core):
  - Q^T,K^T computed in [head_dim, seq] layout (contraction over D on
    partitions); RoPE = A*C + swap(A)*S with swap via a PE permutation
    matmul, combine on VectorE in bf16 (4x DVE mode).
  - scores transposed [k, q]; causal k-blocks skipped; probs = exp on
    ScalarE with fused 1/sqrt(hd) scale.
  - PV: probsT [k,q] moving, V' [k, V|ones] stationary; denominators in
    psum row 64.
  - wo: attnT pair tiles stationary, psum accumulated over pairs.
"""

import sys

if "/opt/trn_rl_repo" not in sys.path:
    sys.path.insert(0, "/opt/trn_rl_repo")

import numpy as np

import concourse.mybir as mybir
import concourse.tile as tile
from concourse import bacc
from concourse.bass_utils import run_bass_kernel_spmd

F32 = mybir.dt.float32
BF16 = mybir.dt.bfloat16
AF = mybir.ActivationFunctionType
DIV = mybir.AluOpType.divide

B, S, D, H, HD = 2, 2048, 1024, 16, 64
NCORES = 8
GROUPS = 4            # head groups (cores per batch)
HPG = H // GROUPS     # heads per core = 4
NPAIR = HPG // 2      # head pairs per core = 2
NEG_INF = -1e9
SM_SCALE = 1.0 / float(np.sqrt(HD))  # 0.125

NIT = D // 128        # 8 contraction tiles
NSB = S // 128        # 16 seq blocks
QCH = 512             # attention q-chunk
NCHUNK = S // QCH     # 4
KPC = QCH // 128      # k-blocks per chunk = 4

_PROG_CACHE = {}


def _build_program(mask_kind: str):
    """mask_kind: 'causal' (skip + diag mask) or 'zeros' (full, no mask)."""
    causal = mask_kind == "causal"
    nc = bacc.Bacc("TRN2", target_bir_lowering=False, debug=False,
                   num_devices=NCORES)

    # inputs are packed host-side to minimize DMA instruction count
    # (each dma_start costs ~0.6us of HWDGE descriptor time)
    xT_d = nc.dram_tensor("xT", [D, S], BF16, kind="ExternalInput").ap()
    # per 128-row block: [wq | wk | wv] column slices
    wqkv_d = nc.dram_tensor("wqkvT", [D, 3 * HPG * HD], BF16,
                            kind="ExternalInput").ap()
    woT_d = nc.dram_tensor("woT", [HPG * HD, D], BF16, kind="ExternalInput").ap()
    cs_d = nc.dram_tensor("cs128", [128, 2 * S], BF16, kind="ExternalInput").ap()
    # [pmat | ident | mdiagT]
    msk_d = nc.dram_tensor("msk", [128, 384], BF16, kind="ExternalInput").ap()
    out_d = nc.dram_tensor("out", [S, D], BF16, kind="ExternalOutput").ap()

    with tile.TileContext(nc) as tc:
        from contextlib import ExitStack

        with ExitStack() as root:
            pers = root.enter_context(tc.tile_pool(name="pers", bufs=1))

            # ---- persistent SBUF tiles ----
            qt = [pers.tile([128, S], BF16, tag=f"qt{p}", name=f"qt{p}")
                  for p in range(NPAIR)]
            kt = [pers.tile([128, S], BF16, tag=f"kt{p}", name=f"kt{p}")
                  for p in range(NPAIR)]
            # V' per (pair, s-block): [128,130] = V_A|ones|V_B|ones
            vp = [[pers.tile([128, 130], BF16, tag=f"vp{p}_{sb}",
                             name=f"vp{p}_{sb}")
                   for sb in range(NSB)] for p in range(NPAIR)]
            at = [pers.tile([128, S], BF16, tag=f"at{p}", name=f"at{p}")
                  for p in range(NPAIR)]
            wo2 = pers.tile([128, NPAIR * D], BF16, tag="wo2", name="wo2")
            wo_t = [wo2[:, p * D:(p + 1) * D] for p in range(NPAIR)]
            msk_t = pers.tile([128, 384], BF16, tag="msk", name="msk")
            pm_t = msk_t[:, 0:128]
            ident_t = msk_t[:, 128:256]
            mdiag_t = msk_t[:, 256:384]
            ones64 = pers.tile([65, 64], BF16, tag="ones64", name="ones64")
            # all memsets first so the Pool engine is done before attention
            nc.gpsimd.memset(ones64[64:65, :], 1.0)
            for p in range(NPAIR):
                for sb in range(NSB):
                    nc.gpsimd.memset(vp[p][sb][:, 64:65], 1.0)
                    nc.gpsimd.memset(vp[p][sb][:, 129:130], 1.0)

            # attention-phase SBUF pools created BEFORE the phase-B ld pool
            # so they get distinct address ranges — otherwise their first
            # tiles wait ~2.4us for the last rope op to release ld's range
            prb = root.enter_context(tc.tile_pool(name="prb", bufs=10))
            nrm = root.enter_context(tc.tile_pool(name="nrm", bufs=2))
            osb = root.enter_context(tc.tile_pool(name="osb", bufs=4))

            # PE warm-up: ~4us of dummy matmuls during the otherwise-dead
            # input-DMA window releases the HAM clock gate (PE runs at
            # 1.2GHz for its first ~3.4us of activity otherwise), so the
            # first real projections start at full clock
            with tc.tile_pool(name="wrm", bufs=1, space="PSUM") as wrm:
                wt = wrm.tile([64, 64], F32, tag="warm", name="warm")
                for _ in range(100):
                    nc.tensor.matmul(wt[:], ones64[64:65, 0:64],
                                     ones64[64:65, 0:64],
                                     start=True, stop=True)

            # ================= Phase B: projections + rope =================
            with ExitStack() as phb:
                ld = phb.enter_context(tc.tile_pool(name="ld", bufs=1))
                xts = [ld.tile([128, S], BF16, tag=f"xt{it}", name=f"xt{it}")
                       for it in range(NIT)]
                wqkv = [ld.tile([128, 3 * HPG * HD], BF16, tag=f"wqkv{it}",
                                name=f"wqkv{it}") for it in range(NIT)]
                wq_t = [w[:, 0:256] for w in wqkv]
                wk_t = [w[:, 256:512] for w in wqkv]
                wv_t = [w[:, 512:768] for w in wqkv]
                cs_t = ld.tile([128, 2 * S], BF16, tag="cs128", name="cs128")
                c_t = cs_t[:, 0:S]
                s_t = cs_t[:, S:2 * S]
                # x tiles paced with the packed weights they're consumed
                # with; the V weight slice ships first so the head's V
                # matmuls fire before the Q/K slice lands
                for it in range(NIT):
                    sl = slice(it * 128, (it + 1) * 128)
                    nc.sync.dma_start(out=xts[it][:], in_=xT_d[sl, :])
                    nc.sync.dma_start(out=wqkv[it][:, 512:768],
                                      in_=wqkv_d[sl, 512:768])
                    nc.sync.dma_start(out=wqkv[it][:, 0:512],
                                      in_=wqkv_d[sl, 0:512])
                nc.sync.dma_start(out=msk_t[:], in_=msk_d[:])
                nc.sync.dma_start(out=cs_t[:], in_=cs_d[:])
                nc.sync.dma_start(
                    out=wo2[:].rearrange("p (a d) -> p a d", a=NPAIR),
                    in_=woT_d[:].rearrange("(a p) d -> p a d", a=NPAIR,
                                           p=128))

                psV = phb.enter_context(
                    tc.tile_pool(name="psV", bufs=4, space="PSUM"))
                # psA slots serve both the Q/K accumulations and the rope
                # swap matmuls (alternating rotation)
                psA = phb.enter_context(
                    tc.tile_pool(name="psA", bufs=4, space="PSUM"))
                sbA = phb.enter_context(tc.tile_pool(name="sbA", bufs=6))

                def v_finish(sb, ps):
                    for p in range(NPAIR):
                        # psum cols [p*128, p*128+128) -> vp cols {0:64, 65:129}
                        src = ps[:, p * 128:(p + 1) * 128] \
                            .rearrange("p (b c) -> p b c", b=2, c=64)
                        dst = vp[p][sb][:, 0:130] \
                            .rearrange("p (b c) -> p b c", b=2, c=65)[:, :, 0:64]
                        nc.vector.tensor_copy(dst, src)

                def rope_copy(ps):
                    """Act copy of the projection psum (bf16)."""
                    a_sb = sbA.tile([128, 512], BF16, tag="a_sb", name="a_sb")
                    nc.scalar.activation(a_sb[:], ps[:], AF.Copy)
                    return a_sb

                def rope_finish(a_sb, dst, p, ch):
                    """rope: rot = A*C + swap(A)*S into dst[p][:, chunk].

                    Emitted one accumulation later than its rope_copy so the
                    PE swap matmul never waits on the Act copy."""
                    qs = slice(ch * 512, (ch + 1) * 512)
                    sw = psA.tile([128, 512], F32, tag="psA", name="psSW")
                    nc.tensor.matmul(sw[:], pm_t[:], a_sb[:],
                                     start=True, stop=True)
                    sw_sb = sbA.tile([128, 512], BF16, tag="sw_sb",
                                     name="sw_sb")
                    nc.vector.tensor_copy(sw_sb[:], sw[:])
                    t1 = sbA.tile([128, 512], BF16, tag="t1", name="t1")
                    nc.vector.tensor_mul(t1[:], a_sb[:], c_t[:, qs])
                    t2 = sbA.tile([128, 512], BF16, tag="t2", name="t2")
                    nc.vector.tensor_mul(t2[:], sw_sb[:], s_t[:, qs])
                    nc.vector.tensor_add(dst[p][:, qs], t1[:], t2[:])

                # head: while x/w tiles stream in, consume them it-major
                # across 8 V accumulations (2 packed per psum tile) + Q/K
                # (pair0, chunks 0-1) so the PE is never starved by the
                # input DMA
                # NOTE: do NOT pack two V accumulation groups into one psum
                # bank — on real HW start=True clears has_written at bank
                # granularity and corrupts the neighboring accumulation
                # (sim-invisible; cost us a debug round).
                head_v = [psV.tile([128, HPG * HD], F32, tag="psV",
                                   name=f"psVh{sb}") for sb in range(4)]
                head_qk = [psA.tile([128, 512], F32, tag="psA",
                                    name=f"psAh{i}") for i in range(4)]
                for it in range(NIT):
                    st = (it == 0)
                    sp = (it == NIT - 1)
                    for sb in range(4):
                        ssl = slice(sb * 128, (sb + 1) * 128)
                        nc.tensor.matmul(head_v[sb][:], xts[it][:, ssl],
                                         wv_t[it][:], start=st, stop=sp)
                    for ch in range(2):
                        qs = slice(ch * 512, (ch + 1) * 512)
                        nc.tensor.matmul(head_qk[ch][:], wq_t[it][:, 0:128],
                                         xts[it][:, qs], start=st, stop=sp)
                        nc.tensor.matmul(head_qk[2 + ch][:],
                                         wk_t[it][:, 0:128],
                                         xts[it][:, qs], start=st, stop=sp)
                for sb in range(4):
                    v_finish(sb, head_v[sb])
                head_units = [(head_qk[0], qt, 0, 0), (head_qk[2], kt, 0, 0),
                              (head_qk[1], qt, 0, 1), (head_qk[3], kt, 0, 1)]
                head_copies = [(rope_copy(ps), dst, p, ch)
                               for ps, dst, p, ch in head_units]

                # remaining V blocks; the head units' rope swaps slot in
                # between the V accumulations (PE stays dense)
                for sb in range(4, NSB):
                    ssl = slice(sb * 128, (sb + 1) * 128)
                    ps = psV.tile([128, HPG * HD], F32, tag="psV", name="psV")
                    for it in range(NIT):
                        nc.tensor.matmul(ps[:], xts[it][:, ssl], wv_t[it][:],
                                         start=(it == 0), stop=(it == NIT - 1))
                    v_finish(sb, ps)
                    if sb - 4 < len(head_copies):
                        rope_finish(*head_copies[sb - 4])

                # remaining Q/K projections, rope pipelined one unit behind
                pend = None
                for p in range(NPAIR):
                    pc = slice(p * 128, (p + 1) * 128)
                    for wt, dst in ((wq_t, qt), (wk_t, kt)):
                        for ch in range(S // 512):
                            if p == 0 and ch < 2:
                                continue  # done in the head
                            qs = slice(ch * 512, (ch + 1) * 512)
                            ps = psA.tile([128, 512], F32, tag="psA",
                                          name="psA")
                            for it in range(NIT):
                                nc.tensor.matmul(
                                    ps[:], wt[it][:, pc], xts[it][:, qs],
                                    start=(it == 0), stop=(it == NIT - 1))
                            a_sb = rope_copy(ps)
                            if pend is not None:
                                rope_finish(*pend)
                            pend = (a_sb, dst, p, ch)
                if pend is not None:
                    rope_finish(*pend)

            # ============ Phase C/D: attention + output projection ============
            with ExitStack() as phc:
                psS = phc.enter_context(
                    tc.tile_pool(name="psS", bufs=2, space="PSUM"))
                # ov gets both psO slots; wo psum borrows the psS slots
                # (idle mid-attention)
                psO = phc.enter_context(
                    tc.tile_pool(name="psO", bufs=2, space="PSUM"))

                def emit_scores(p, c, kb, sc):
                    """scores (+ causal diag mask) on PE for both heads."""
                    q0 = c * QCH
                    k0 = kb * 128
                    trim = max(q0, k0) if causal else q0
                    t_off = trim - q0
                    on_diag = causal and k0 >= q0
                    for h in range(2):
                        hsl = slice(h * 64, (h + 1) * 64)
                        nc.tensor.matmul(
                            sc[:, h * QCH + t_off:(h + 1) * QCH],
                            kt[p][hsl, k0:k0 + 128],
                            qt[p][hsl, trim:q0 + QCH],
                            start=True, stop=not on_diag)
                    if on_diag:
                        for h in range(2):
                            nc.tensor.matmul(
                                sc[:, h * QCH + t_off:h * QCH + t_off + 128],
                                ident_t[:], mdiag_t[:],
                                start=False, stop=True)

                def emit_exp(c, kb, sc, pt, split=False):
                    """split=True: one exp per head — lower latency to the
                    first PV at pipeline warm-up, slightly more overhead."""
                    q0 = c * QCH
                    t_off = (max(q0, kb * 128) - q0) if causal else 0
                    if split:
                        for h in range(2):
                            hs = slice(h * QCH + t_off, (h + 1) * QCH)
                            nc.scalar.activation(pt[:, hs], sc[:, hs],
                                                 AF.Exp, scale=SM_SCALE)
                        return
                    if t_off == 0:
                        # full k-block: the two head halves are contiguous,
                        # use a flat 2D AP (cheaper descriptor walk)
                        nc.scalar.activation(pt[:, 0:2 * QCH],
                                             sc[:, 0:2 * QCH],
                                             AF.Exp, scale=SM_SCALE)
                        return
                    sc3 = sc[:, 0:2 * QCH].rearrange(
                        "p (b c) -> p b c", b=2, c=QCH)[:, :, t_off:]
                    pt3 = pt[:, 0:2 * QCH].rearrange(
                        "p (b c) -> p b c", b=2, c=QCH)[:, :, t_off:]
                    nc.scalar.activation(pt3, sc3, AF.Exp, scale=SM_SCALE)

                def emit_pv(p, c, kb, kb_hi, pt, ov):
                    q0 = c * QCH
                    t_off = (max(q0, kb * 128) - q0) if causal else 0
                    for h in range(2):
                        nc.tensor.matmul(
                            ov[:, h * QCH + t_off:(h + 1) * QCH],
                            vp[p][kb][:, h * 65:h * 65 + 65],
                            pt[:, h * QCH + t_off:(h + 1) * QCH],
                            start=(kb == 0), stop=(kb == kb_hi - 1))

                ob_pend = {}

                def emit_wo_oc(sb, oc, on_act=False, tail=False):
                    """one wo psum tile: seq block sb, output half oc.
                    Normally the two halves share one [128,1024] staging
                    tile and a single output DMA (half the HWDGE descriptor
                    work); at the tail each half ships immediately and the
                    psum comes from the free scores slots for extra depth."""
                    ssl = slice(sb * 128, (sb + 1) * 128)
                    osl = slice(oc * 512, (oc + 1) * 512)
                    ps = psO.tile([128, 512], F32, tag="ov", name="psW")
                    for p in range(NPAIR):
                        nc.tensor.matmul(
                            ps[:], at[p][:, ssl], wo_t[p][:, osl],
                            start=(p == 0), stop=(p == NPAIR - 1))
                    if sb not in ob_pend:
                        ob_pend[sb] = osb.tile([128, 1024], BF16, tag="osb",
                                               name="osb")
                    ob = ob_pend[sb]
                    if on_act:
                        nc.scalar.activation(ob[:, osl], ps[:], AF.Copy)
                    else:
                        nc.vector.tensor_copy(ob[:, osl], ps[:])
                    if oc == 1:
                        nc.sync.dma_start(out=out_d[ssl, :],
                                          in_=ob_pend.pop(sb)[:])

                def emit_wo_sb(sb, on_act=False):
                    for oc in range(2):
                        emit_wo_oc(sb, oc, on_act=on_act)

                # ascending: early small chunks' norm chains hide under the
                # growing attention windows; only the last norm is exposed
                chunk_order = list(range(NCHUNK))
                pending_norm = None
                pair_jobs = [(ci, c, p) for ci, c in enumerate(chunk_order)
                             for p in range(NPAIR)]

                def emit_item(p, c, kb):
                    sc = psS.tile([128, 2 * QCH], F32, tag="sc", name="sc")
                    pt = prb.tile([128, 2 * QCH], BF16, tag="prb",
                                  name="prb")
                    emit_scores(p, c, kb, sc)
                    emit_exp(c, kb, sc, pt)
                    return (kb, pt)

                # `pro` carries the next pair's first two scores+exps,
                # pre-emitted before the previous pair's last PVs so the
                # ScalarE exps them while the PE drains the old pair — the
                # new pair's first PV then never waits on its exp
                pro = None
                for j, (ci, c, p) in enumerate(pair_jobs):
                    q0 = c * QCH
                    kb_hi = (c * KPC + KPC) if causal else NSB
                    # both heads' PV accum: h0 cols 0:512, h1 cols 512:1024
                    # data rows 0:64, denominator row 64
                    ov = psO.tile([65, 2 * QCH], F32, tag="ov", name="ov")
                    wo_queue = []
                    if p == 1 and ci > 0:
                        pc_ = chunk_order[ci - 1]
                        wo_queue = [(pc_ * KPC + i // 2, i % 2)
                                    for i in range(2 * KPC)]
                    if pro is None:
                        pro = [emit_item(p, c, kb) for kb in range(2)]
                    pvq = list(pro)  # scored+exp'd items awaiting PV
                    for idx, kb in enumerate(range(2, kb_hi)):
                        if idx == 1 and pending_norm is not None:
                            # previous pair's deferred normalization: its
                            # PE broadcast lands in a freed sc slot
                            pending_norm()
                            pending_norm = None
                        pvq.append(emit_item(p, c, kb))
                        if wo_queue and kb >= kb_hi - 2 * KPC:
                            emit_wo_oc(*wo_queue.pop(0))
                        pkb, ppt = pvq.pop(0)
                        emit_pv(p, c, pkb, kb_hi, ppt, ov)
                    # pre-emit the next pair's prologue, then drain this
                    # pair's remaining PVs
                    if j + 1 < len(pair_jobs):
                        _, nc_c, nc_p = pair_jobs[j + 1]
                        pro = [emit_item(nc_p, nc_c, 0)]
                        pkb, ppt = pvq.pop(0)
                        emit_pv(p, c, pkb, kb_hi, ppt, ov)
                        pro.append(emit_item(nc_p, nc_c, 1))
                    else:
                        pro = None
                    for pkb, ppt in pvq:
                        emit_pv(p, c, pkb, kb_hi, ppt, ov)
                    for sb_oc in wo_queue:
                        emit_wo_oc(*sb_oc)
                    # normalize: attnT = ov[0:64] / denom (row 64)
                    last_pair = (ci == NCHUNK - 1) and (p == NPAIR - 1)
                    if not last_pair:
                        # copy the denominator row out now (split across
                        # DVE and Act); the broadcast matmul, reciprocal
                        # and the normalizing multiplies are deferred
                        # into the next pair's attention loop
                        den = nrm.tile([65, 2 * QCH], BF16, tag="den",
                                       name="den")
                        nc.vector.tensor_copy(den[64:65, :],
                                              ov[64:65, :])

                        def make_norm(p=p, q0=q0, ov=ov, den=den):
                            def emit():
                                rps = psS.tile([128, 2 * QCH], F32,
                                               tag="sc", name="rps")
                                for hh in range(2):
                                    hs = slice(hh * QCH,
                                               (hh + 1) * QCH)
                                    nc.tensor.matmul(
                                        rps[0:64, hs], ones64[64:65, :],
                                        den[64:65, hs],
                                        start=True, stop=True)
                                rrec = nrm.tile([64, 2 * QCH], F32,
                                                tag="rrec", name="rrec")
                                nc.vector.reciprocal_approx_fast(
                                    rrec[:], rps[0:64, :])
                                nc.vector.tensor_mul(
                                    at[p][0:64, q0:q0 + QCH],
                                    ov[0:64, 0:QCH], rrec[:, 0:QCH])
                                atb = nrm.tile([64, QCH], BF16,
                                               tag="atb", name="atb")
                                nc.vector.tensor_mul(
                                    atb[:], ov[0:64, QCH:2 * QCH],
                                    rrec[:, QCH:2 * QCH])
                                nc.sync.dma_start(
                                    out=at[p][64:128, q0:q0 + QCH],
                                    in_=atb[:])
                            return emit

                        pending_norm = make_norm()
                    else:
                        # tail: broadcast via a PE outer product into a
                        # free scores slot — nothing left to hide the
                        # DMA bounce behind
                        # fully per-head split, h1 first: its
                        # partition-shift DMA and h0's chain overlap
                        rps = psS.tile([128, 2 * QCH], F32, tag="sc",
                                       name="rps")
                        den1 = nrm.tile([65, QCH], BF16, tag="den1",
                                        name="den1")
                        nc.vector.tensor_copy(den1[64:65, :],
                                              ov[64:65, QCH:2 * QCH])
                        nc.tensor.matmul(rps[0:64, QCH:2 * QCH],
                                         ones64[64:65, :],
                                         den1[64:65, :],
                                         start=True, stop=True)
                        rr1 = nrm.tile([64, QCH], F32, tag="rr1",
                                       name="rr1")
                        nc.vector.reciprocal_approx_fast(
                            rr1[:], rps[0:64, QCH:2 * QCH])
                        atb = nrm.tile([64, QCH], BF16, tag="atb",
                                       name="atb")
                        nc.vector.tensor_mul(
                            atb[:], ov[0:64, QCH:2 * QCH], rr1[:])
                        nc.sync.dma_start(
                            out=at[p][64:128, q0:q0 + QCH], in_=atb[:])
                        den0 = nrm.tile([65, QCH], BF16, tag="den0",
                                        name="den0")
                        nc.vector.tensor_copy(den0[64:65, :],
                                              ov[64:65, 0:QCH])
                        nc.tensor.matmul(rps[0:64, 0:QCH],
                                         ones64[64:65, :],
                                         den0[64:65, :],
                                         start=True, stop=True)
                        rr0 = nrm.tile([64, QCH], F32, tag="rr0",
                                       name="rr0")
                        nc.vector.reciprocal_approx_fast(
                            rr0[:], rps[0:64, 0:QCH])
                        nc.vector.tensor_mul(
                            at[p][0:64, q0:q0 + QCH], ov[0:64, 0:QCH],
                            rr0[:])
                        for sb in range(c * KPC, (c + 1) * KPC):
                            emit_wo_oc(sb, 0, on_act=True, tail=True)
                            emit_wo_oc(sb, 1, on_act=False, tail=True)

    nc.compile()
    return nc


def _host_prep(x, freqs_cos, freqs_sin, wq, wk, wv, wo):
    """Build the 8 per-core input maps (numpy, bf16)."""
    import ml_dtypes

    bf16 = ml_dtypes.bfloat16

    x = np.ascontiguousarray(x, dtype=np.float32)
    cosT = np.ascontiguousarray(freqs_cos.T, dtype=np.float32)  # [32, S]
    sinT = np.ascontiguousarray(freqs_sin.T, dtype=np.float32)

    c128 = np.tile(cosT, (4, 1))                                # [128, S]
    s128 = np.tile(np.concatenate([-sinT, sinT], 0), (2, 1))
    cs128 = np.ascontiguousarray(
        np.concatenate([c128, s128], axis=1)).astype(bf16)      # [128, 2S]
    # swap permutation: psum_sw = pmat.T @ A -> sw[m] = A[sigma(m)],
    # sigma swaps the 32-halves within each 64 block.
    pmat = np.zeros((128, 128), dtype=np.float32)
    for m in range(128):
        blk, off = divmod(m, 32)
        pmat[(blk ^ 1) * 32 + off, m] = 1.0
    ident = np.eye(128, dtype=np.float32)
    # causal diag mask, transposed: mdiagT[k, q] = 0 if k <= q else -1e9
    kk, qq = np.meshgrid(np.arange(128), np.arange(128), indexing="ij")
    mdiagT = np.where(kk <= qq, 0.0, NEG_INF).astype(np.float32)
    msk = np.ascontiguousarray(
        np.concatenate([pmat, ident, mdiagT], axis=1)).astype(bf16)

    # rotate-half row permutation within each head
    rh = np.concatenate([np.arange(0, HD, 2), np.arange(1, HD, 2)])

    xT = [np.ascontiguousarray(x[b].T).astype(bf16) for b in range(B)]

    in_maps = []
    for core in range(NCORES):
        b, g = divmod(core, GROUPS)
        heads = [g * HPG + j for j in range(HPG)]
        qrows, vrows = [], []
        for h in heads:
            base = h * HD
            qrows.extend((base + rh).tolist())
            vrows.extend(range(base, base + HD))
        qrows = np.array(qrows)
        vrows = np.array(vrows)
        wqT = wq[qrows, :].T                                     # [D, 256]
        wkT = wk[qrows, :].T
        wvT = wv[vrows, :].T
        wqkvT = np.ascontiguousarray(
            np.concatenate([wqT, wkT, wvT], axis=1)).astype(bf16)
        woT = np.ascontiguousarray(wo[:, vrows].T).astype(bf16)  # [256, D]
        in_maps.append({
            "xT": xT[b], "wqkvT": wqkvT, "woT": woT,
            "cs128": cs128, "msk": msk,
        })
    return in_maps


def _mask_kind(mask):
    m = np.asarray(mask).reshape(S, S)
    if not np.any(m):
        return "zeros"
    qq, kk = np.meshgrid(np.arange(S), np.arange(S), indexing="ij")
    causal = np.where(kk <= qq, 0.0, NEG_INF).astype(np.float32)  # [q, k]
    if np.array_equal(m, causal):
        return "causal"
    return "general"


def _reference_host(x, freqs_cos, freqs_sin, mask, wq, wk, wv, wo):
    """Correctness fallback for arbitrary masks (host numpy, float64)."""
    b, s, d = x.shape
    hd = d // H
    xq = (x @ wq.T).reshape(b, s, H, hd)
    xk = (x @ wk.T).reshape(b, s, H, hd)
    xv = (x @ wv.T).reshape(b, s, H, hd)

    def rope(t):
        tr = t.reshape(b, s, H, hd // 2, 2)
        t0, t1 = tr[..., 0], tr[..., 1]
        cos = freqs_cos[None, :, None, :]
        sin = freqs_sin[None, :, None, :]
        return np.stack([t0 * cos - t1 * sin, t0 * sin + t1 * cos],
                        -1).reshape(b, s, H, hd)

    xq, xk = rope(xq), rope(xk)
    sc = np.einsum("bqhd,bkhd->bhqk", xq, xk) / np.sqrt(hd) + mask
    sc = sc - sc.max(-1, keepdims=True)
    e = np.exp(sc)
    pr = e / e.sum(-1, keepdims=True)
    o = np.einsum("bhqk,bkhd->bqhd", pr, xv).reshape(b, s, d)
    return (o @ wo.T).astype(np.float32)


def kernel(x, freqs_cos, freqs_sin, mask, wq, wk, wv, wo):
    kind = _mask_kind(mask)
    if kind == "general":
        return _reference_host(np.asarray(x, np.float64),
                               np.asarray(freqs_cos, np.float64),
                               np.asarray(freqs_sin, np.float64),
                               np.asarray(mask, np.float64),
                               np.asarray(wq, np.float64),
                               np.asarray(wk, np.float64),
                               np.asarray(wv, np.float64),
                               np.asarray(wo, np.float64))

    if kind not in _PROG_CACHE:
        _PROG_CACHE[kind] = _build_program(kind)
    nc = _PROG_CACHE[kind]

    in_maps = _host_prep(np.asarray(x, np.float32),
                         np.asarray(freqs_cos, np.float32),
                         np.asarray(freqs_sin, np.float32),
                         np.asarray(wq, np.float32),
                         np.asarray(wk, np.float32),
                         np.asarray(wv, np.float32),
                         np.asarray(wo, np.float32))
    res = run_bass_kernel_spmd(nc, in_maps, list(range(NCORES)))
    out = np.zeros((B, S, D), dtype=np.float32)
    for core in range(NCORES):
        out[core // GROUPS] += np.asarray(res.results[core]["out"],
                                          dtype=np.float32)
    return out



# revision 5
# speedup vs baseline: 1.1240x; 1.1240x over previous
"""Trainium2 Bass kernel for nn_Attention_12515534700827.

Multi-head causal attention with RoPE: B=2, S=2048, D=1024, H=16, HD=64.
Sharding: 8 cores = 2 (batch) x 4 (head groups of 4 heads). Each core
computes its 4 heads' attention + its slice of the wo projection; the host
sums the 4 partial outputs per batch (the "all-reduce after wo").

v3 (fused single-stream): projections, attention, and the wo projection are
emitted as ONE interleaved instruction stream so the ScalarE exp stream (the
second-largest engine load) overlaps the projection/wo matmuls instead of
running in its own phase.

Key differences vs v2:
  - PV computed with probs as the STATIONARY operand and V' ([V|ones]) as
    the MOVING operand: out[q, vd|den] per (head, q-slice).  The moving free
    dim drops from ~512 to 65, halving PV cost; the softmax denominator
    arrives as psum column 64 per head so normalization becomes a
    per-partition scalar multiply (no PE broadcast, no reciprocal of a
    [64,1024] tile, no partition-shift DMA bounce).
  - The resulting attn tiles are [q, vd]; wo needs [vd, q].  Transposed via
    dma_start_transpose (XBAR 16x128 tiles, cheap on the DMA engines)
    straight into the persistent at[] tiles; the last pair uses PE
    transposes so the tail isn't gated on a DMA round trip.
  - Fused emission: after the head block (V sb0-3 + Q/K chunk0 for both
    pairs over x cols 0:1024), attention items start immediately; the
    remaining V blocks, Q/K chunks, rope chains and deferred wo tiles are
    "fillers" pulled between items to keep the PE dense while ScalarE
    streams the exps.
  - wo(c) is deferred ~2 chunks so it lands as filler in the late,
    otherwise Act-bound stretch.
  - Inputs land in few large DMAs (HWDGE descriptor time ~0.6us each).

PSUM budget (8 banks): scores 2x[128,1024] (4) + PV 2x[128,512] (2) +
misc single-shot rotation psX 2x[128,512] (2: rope swaps, V pairs, wo,
tail transposes).  PV packs 2 q-slices x 2 heads x 65 cols per bank with
one accumulation-group start/stop per bank (hardware clears has_written at
bank granularity).
"""

import sys

if "/opt/trn_rl_repo" not in sys.path:
    sys.path.insert(0, "/opt/trn_rl_repo")

import numpy as np

import concourse.mybir as mybir
import concourse.tile as tile
from concourse import bacc
from concourse.bass_utils import run_bass_kernel_spmd

F32 = mybir.dt.float32
BF16 = mybir.dt.bfloat16
AF = mybir.ActivationFunctionType

B, S, D, H, HD = 2, 2048, 1024, 16, 64
NCORES = 8
GROUPS = 4            # head groups (cores per batch)
HPG = H // GROUPS     # heads per core = 4
NPAIR = HPG // 2      # head pairs per core = 2
NEG_INF = -1e9
SM_SCALE = 1.0 / float(np.sqrt(HD))  # 0.125

NIT = D // 128        # 8 contraction tiles
NSB = S // 128        # 16 seq blocks
QCH = 512             # attention q-chunk
NCHUNK = S // QCH     # 4
KPC = QCH // 128      # k/q 128-blocks per chunk = 4

_PROG_CACHE = {}


def _build_program(mask_kind: str):
    """mask_kind: 'causal' (trimmed + diag mask) or 'zeros' (full)."""
    causal = mask_kind == "causal"
    nc = bacc.Bacc("TRN2", target_bir_lowering=False, debug=False,
                   num_devices=NCORES)

    xT_d = nc.dram_tensor("xT", [D, S], BF16, kind="ExternalInput").ap()
    # per 128-row block: [wq | wk | wv] column slices
    wqkv_d = nc.dram_tensor("wqkvT", [D, 3 * HPG * HD], BF16,
                            kind="ExternalInput").ap()
    woT_d = nc.dram_tensor("woT", [HPG * HD, D], BF16, kind="ExternalInput").ap()
    cs_d = nc.dram_tensor("cs128", [128, 2 * S], BF16, kind="ExternalInput").ap()
    # [pmat | ident | mdiagT]
    msk_d = nc.dram_tensor("msk", [128, 384], BF16, kind="ExternalInput").ap()
    out_d = nc.dram_tensor("out", [S, D], BF16, kind="ExternalOutput").ap()

    with tile.TileContext(nc) as tc:
        from contextlib import ExitStack

        with ExitStack() as root:
            pers = root.enter_context(tc.tile_pool(name="pers", bufs=1))

            # ---- persistent SBUF tiles ----
            qt = [pers.tile([128, S], BF16, tag=f"qt{p}", name=f"qt{p}")
                  for p in range(NPAIR)]
            kt = [pers.tile([128, S], BF16, tag=f"kt{p}", name=f"kt{p}")
                  for p in range(NPAIR)]
            # V' per (pair, s-block): [128,130] = V_A|ones|V_B|ones
            vp = [[pers.tile([128, 130], BF16, tag=f"vp{p}_{sb}",
                             name=f"vp{p}_{sb}")
                   for sb in range(NSB)] for p in range(NPAIR)]
            # attnT per pair: [vd(2 heads x 64), S]
            at = [pers.tile([128, S], BF16, tag=f"at{p}", name=f"at{p}")
                  for p in range(NPAIR)]
            wo2 = pers.tile([128, NPAIR * D], BF16, tag="wo2", name="wo2")
            wo_t = [wo2[:, p * D:(p + 1) * D] for p in range(NPAIR)]
            msk_t = pers.tile([128, 384], BF16, tag="msk", name="msk")
            pm_t = msk_t[:, 0:128]
            ident_t = msk_t[:, 128:256]
            mdiag_t = msk_t[:, 256:384]
            ones1 = pers.tile([1, 64], BF16, tag="ones1", name="ones1")

            # all memsets first so the Pool engine is done before attention
            nc.gpsimd.memset(ones1[:], 1.0)
            for p in range(NPAIR):
                for sb in range(NSB):
                    nc.gpsimd.memset(vp[p][sb][:, 64:65], 1.0)
                    nc.gpsimd.memset(vp[p][sb][:, 129:130], 1.0)

            ld = root.enter_context(tc.tile_pool(name="ld", bufs=1))
            sbA = root.enter_context(tc.tile_pool(name="sbA", bufs=2))
            prb = root.enter_context(tc.tile_pool(name="prb", bufs=5))
            asb = root.enter_context(tc.tile_pool(name="asb", bufs=3))
            nrm = root.enter_context(tc.tile_pool(name="nrm", bufs=4))
            osb = root.enter_context(tc.tile_pool(name="osb", bufs=4))
            psS = root.enter_context(
                tc.tile_pool(name="psS", bufs=2, space="PSUM"))
            psPV = root.enter_context(
                tc.tile_pool(name="psPV", bufs=2, space="PSUM"))
            psX = root.enter_context(
                tc.tile_pool(name="psX", bufs=2, space="PSUM"))

            # PE warm-up: dummy matmuls during the otherwise-dead input-DMA
            # window release the HAM clock gate so the first real
            # projections run at full clock
            wt = psX.tile([64, 64], F32, tag="x", name="warm")
            for _ in range(100):
                nc.tensor.matmul(wt[:], ones1[:], ones1[:],
                                 start=True, stop=True)

            # ---- input DMAs (few, large; wqkv split so it=0 lands early)
            xts = [ld.tile([128, S], BF16, tag=f"xt{it}", name=f"xt{it}")
                   for it in range(NIT)]
            wqkv = ld.tile([128, NIT * 3 * HPG * HD], BF16, tag="wqkv",
                           name="wqkv")
            wq_t = [wqkv[:, it * 768:it * 768 + 256] for it in range(NIT)]
            wk_t = [wqkv[:, it * 768 + 256:it * 768 + 512] for it in range(NIT)]
            wv_t = [wqkv[:, it * 768 + 512:it * 768 + 768] for it in range(NIT)]
            cs_t = ld.tile([128, 2 * S], BF16, tag="cs128", name="cs128")
            c_t = cs_t[:, 0:S]
            s_t = cs_t[:, S:2 * S]

            for half in range(2):
                its = slice(half * 4 * 128, (half + 1) * 4 * 128)
                nc.sync.dma_start(
                    out=wqkv[:, half * 3072:(half + 1) * 3072].rearrange(
                        "p (i c) -> p i c", i=4),
                    in_=wqkv_d[its, :].rearrange("(i p) c -> p i c", p=128))
            for it in range(NIT):
                sl = slice(it * 128, (it + 1) * 128)
                nc.sync.dma_start(out=xts[it][:, 0:1024],
                                  in_=xT_d[sl, 0:1024])
            nc.sync.dma_start(out=msk_t[:], in_=msk_d[:])
            # cos/sin for chunks 0-1, then 2-3 (rope chunk 0 needs it early)
            nc.sync.dma_start(
                out=cs_t[:].rearrange("p (h c) -> p h c", h=2)[:, :, 0:1024],
                in_=cs_d[:].rearrange("p (h c) -> p h c", h=2)[:, :, 0:1024])
            nc.sync.dma_start(
                out=cs_t[:].rearrange("p (h c) -> p h c", h=2)[:, :, 1024:2048],
                in_=cs_d[:].rearrange("p (h c) -> p h c", h=2)[:, :, 1024:2048])
            for it in range(NIT):
                sl = slice(it * 128, (it + 1) * 128)
                nc.sync.dma_start(out=xts[it][:, 1024:2048],
                                  in_=xT_d[sl, 1024:2048])
            nc.sync.dma_start(
                out=wo2[:].rearrange("p (a d) -> p a d", a=NPAIR),
                in_=woT_d[:].rearrange("(a p) d -> p a d", a=NPAIR, p=128))

            # ---------------- helpers ----------------
            def v_finish(sb, ps):
                """psum [128, 256] (pair-packed V) -> vp tiles, both pairs."""
                for p in range(NPAIR):
                    src = ps[:, p * 128:(p + 1) * 128] \
                        .rearrange("p (b c) -> p b c", b=2, c=64)
                    dst = vp[p][sb][:, 0:130] \
                        .rearrange("p (b c) -> p b c", b=2, c=65)[:, :, 0:64]
                    nc.vector.tensor_copy(dst, src)

            def rope_copy(ps):
                a_sb = sbA.tile([128, 512], BF16, tag="a_sb", name="a_sb")
                nc.scalar.activation(a_sb[:], ps[:], AF.Copy)
                return a_sb

            def rope_finish(a_sb, dst, p, ch):
                """rot = A*C + swap(A)*S into dst[p][:, chunk]."""
                qs = slice(ch * 512, (ch + 1) * 512)
                t1 = sbA.tile([128, 512], BF16, tag="t1", name="t1")
                nc.vector.tensor_mul(t1[:], a_sb[:], c_t[:, qs])
                sw = psX.tile([128, 512], F32, tag="x", name="psSW")
                nc.tensor.matmul(sw[:], pm_t[:], a_sb[:],
                                 start=True, stop=True)
                t2 = sbA.tile([128, 512], BF16, tag="t2", name="t2")
                nc.vector.tensor_mul(t2[:], sw[:], s_t[:, qs])
                nc.vector.tensor_add(dst[p][:, qs], t1[:], t2[:])

            # ---------------- head block ----------------
            # it-major over x cols 0:1024: V s-blocks 0-3 + Q/K chunk 0 for
            # both pairs, so both pairs' chunk-0 attention unlocks first.
            hv = [psPV.tile([128, 512], F32, tag="pv", name=f"hv{b}")
                  for b in range(2)]
            hq = [psS.tile([128, 2 * QCH], F32, tag="sc", name=f"hq{p}")
                  for p in range(NPAIR)]
            for it in range(NIT):
                st, sp = it == 0, it == NIT - 1
                for sb in range(4):
                    b, o = divmod(sb, 2)
                    nc.tensor.matmul(hv[b][:, o * 256:(o + 1) * 256],
                                     xts[it][:, sb * 128:(sb + 1) * 128],
                                     wv_t[it][:],
                                     start=(st and o == 0),
                                     stop=(sp and o == 1))
                for p in range(NPAIR):
                    pc = slice(p * 128, (p + 1) * 128)
                    nc.tensor.matmul(hq[p][:, 0:512], wq_t[it][:, pc],
                                     xts[it][:, 0:512], start=st, stop=sp)
                    nc.tensor.matmul(hq[p][:, 512:1024], wk_t[it][:, pc],
                                     xts[it][:, 0:512], start=st, stop=sp)
            for sb in range(4):
                b, o = divmod(sb, 2)
                v_finish(sb, hv[b][:, o * 256:(o + 1) * 256])
            # rope the 4 head units (chunk 0, both pairs), staggered
            hu = [(hq[0][:, 0:512], qt, 0), (hq[0][:, 512:1024], kt, 0),
                  (hq[1][:, 0:512], qt, 1), (hq[1][:, 512:1024], kt, 1)]
            hc = []
            for ps, dst, p in hu:
                hc.append((rope_copy(ps), dst, p))
            for a_sb, dst, p in hc:
                rope_finish(a_sb, dst, p, 0)

            # ---------------- filler machinery ----------------
            done_units = set()

            def g_v_unit(sb0):
                """V s-blocks (sb0, sb0+1): packed 2-per-bank accumulation."""
                t = psX.tile([128, 512], F32, tag="x", name=f"v{sb0}")
                for it in range(NIT):
                    st, sp = it == 0, it == NIT - 1
                    for o in range(2):
                        nc.tensor.matmul(
                            t[:, o * 256:(o + 1) * 256],
                            xts[it][:, (sb0 + o) * 128:(sb0 + o + 1) * 128],
                            wv_t[it][:],
                            start=(st and o == 0), stop=(sp and o == 1))
                        yield 107
                v_finish(sb0, t[:, 0:256])
                v_finish(sb0 + 1, t[:, 256:512])
                done_units.add(f"v{sb0}")
                yield 0

            rope_pend = [None]

            def g_qk_unit(p, w, ch):
                wt = wq_t if w == "q" else wk_t
                dst = qt if w == "q" else kt
                t = psX.tile([128, 512], F32, tag="x", name=f"qk{p}{w}{ch}")
                pc = slice(p * 128, (p + 1) * 128)
                for it in range(NIT):
                    nc.tensor.matmul(t[:], wt[it][:, pc],
                                     xts[it][:, ch * 512:(ch + 1) * 512],
                                     start=(it == 0), stop=(it == NIT - 1))
                    yield 213
                a_sb = rope_copy(t)
                if rope_pend[0] is not None:
                    pa, pdst, pp, pch, pname = rope_pend[0]
                    rope_finish(pa, pdst, pp, pch)
                    done_units.add(pname)
                    yield 213
                rope_pend[0] = (a_sb, dst, p, ch, f"{w}{p}{ch}")

            def flush_pend():
                if rope_pend[0] is not None:
                    pa, pdst, pp, pch, pname = rope_pend[0]
                    rope_finish(pa, pdst, pp, pch)
                    done_units.add(pname)
                    rope_pend[0] = None

            def g_rope_flush():
                if rope_pend[0] is not None:
                    flush_pend()
                    yield 213

            ob_pend = {}

            def emit_wo_oc(sb, oc, tail=False):
                ssl = slice(sb * 128, (sb + 1) * 128)
                osl = slice(oc * 512, (oc + 1) * 512)
                ps = psX.tile([128, 512], F32, tag="x", name="psW")
                for p in range(NPAIR):
                    nc.tensor.matmul(
                        ps[:], at[p][:, ssl], wo_t[p][:, osl],
                        start=(p == 0), stop=(p == NPAIR - 1))
                if sb not in ob_pend:
                    ob_pend[sb] = osb.tile([128, 1024], BF16, tag="osb",
                                           name="osb")
                ob = ob_pend[sb]
                if oc == 0:
                    nc.scalar.activation(ob[:, osl], ps[:], AF.Copy)
                else:
                    nc.vector.tensor_copy(ob[:, osl], ps[:])
                if tail:
                    nc.sync.dma_start(out=out_d[ssl, osl], in_=ob[:, osl])
                    if oc == 1:
                        ob_pend.pop(sb)
                elif oc == 1:
                    nc.sync.dma_start(out=out_d[ssl, :],
                                      in_=ob_pend.pop(sb)[:])

            def g_wo_chunk(c):
                for sb in range(c * KPC, (c + 1) * KPC):
                    for oc in range(2):
                        emit_wo_oc(sb, oc)
                        yield 426

            # filler order: chunk-1 q/k (x cols 512:1024, already loaded)
            # before the xB-dependent V blocks; V blocks paced so vp(sb) is
            # ready ~when chunk sb//4's PV needs it.
            fillers = [
                g_qk_unit(0, "q", 1), g_qk_unit(0, "k", 1),
                g_v_unit(4),
                g_qk_unit(1, "q", 1), g_qk_unit(1, "k", 1),
                g_v_unit(6), g_v_unit(8),
                g_qk_unit(0, "q", 2), g_qk_unit(0, "k", 2),
                g_v_unit(10),
                g_qk_unit(1, "q", 2), g_qk_unit(1, "k", 2),
                g_v_unit(12),
                g_qk_unit(0, "q", 3), g_qk_unit(0, "k", 3),
                g_v_unit(14),
                g_qk_unit(1, "q", 3), g_qk_unit(1, "k", 3),
                g_rope_flush(),
            ]
            fill_iq = [0]

            def pull(budget_ns):
                got = 0
                while fill_iq[0] < len(fillers):
                    g = fillers[fill_iq[0]]
                    try:
                        while got < budget_ns:
                            got += next(g)
                    except StopIteration:
                        fill_iq[0] += 1
                        continue
                    break
                return got

            def pull_until(units):
                while not units <= done_units:
                    # the last missing unit may be parked in the rope pend
                    if rope_pend[0] is not None and \
                            units <= (done_units | {rope_pend[0][4]}):
                        flush_pend()
                        return
                    if pull(600) == 0:
                        flush_pend()
                        assert units <= done_units, (
                            f"filler units {units - done_units} never emitted")
                        return

            # ---------------- attention ----------------
            def emit_scores(p, c, kb, sc):
                q0 = c * QCH
                k0 = kb * 128
                trim = max(q0, k0) if causal else q0
                t_off = trim - q0
                on_diag = causal and k0 >= q0
                for h in range(2):
                    hsl = slice(h * 64, (h + 1) * 64)
                    nc.tensor.matmul(
                        sc[:, h * QCH + t_off:(h + 1) * QCH],
                        kt[p][hsl, k0:k0 + 128],
                        qt[p][hsl, trim:q0 + QCH],
                        start=True, stop=not on_diag)
                if on_diag:
                    for h in range(2):
                        nc.tensor.matmul(
                            sc[:, h * QCH + t_off:h * QCH + t_off + 128],
                            ident_t[:], mdiag_t[:],
                            start=False, stop=True)

            def emit_exp(c, kb, sc, pt):
                q0 = c * QCH
                t_off = (max(q0, kb * 128) - q0) if causal else 0
                if t_off == 0:
                    nc.scalar.activation(pt[:, 0:2 * QCH], sc[:, 0:2 * QCH],
                                         AF.Exp, scale=SM_SCALE)
                    return
                sc3 = sc[:, 0:2 * QCH].rearrange(
                    "p (b c) -> p b c", b=2, c=QCH)[:, :, t_off:]
                pt3 = pt[:, 0:2 * QCH].rearrange(
                    "p (b c) -> p b c", b=2, c=QCH)[:, :, t_off:]
                nc.scalar.activation(pt3, sc3, AF.Exp, scale=SM_SCALE)

            def emit_pv(p, c, kb, kb_hi, pt, pvt):
                qs_lo = max(0, kb - 4 * c) if causal else 0
                for qs in range(qs_lo, 4):
                    bank, qsl = divmod(qs, 2)
                    last_kb = (4 * c + bank * 2 + 1) if causal else kb_hi - 1
                    for h in range(2):
                        col = qsl * 130 + h * 65
                        nc.tensor.matmul(
                            pvt[bank][:, col:col + 65],
                            pt[:, h * QCH + qs * 128:h * QCH + qs * 128 + 128],
                            vp[p][kb][:, h * 65:h * 65 + 65],
                            start=(kb == 0 and qsl == 0 and h == 0),
                            stop=(kb == last_kb and qsl == 1 and h == 1))

            def emit_norm(j, pvt, attn_sc):
                """normalize q-slice j of the pair-chunk into attn_sc."""
                bank, qsl = divmod(j, 2)
                rr = nrm.tile([128, 2], F32, tag="rr", name="rr")
                den = pvt[bank][:, qsl * 130:qsl * 130 + 130].rearrange(
                    "p (h c) -> p h c", h=2)[:, :, 64:65]
                nc.vector.reciprocal_approx_fast(
                    rr[:].rearrange("p (h c) -> p h c", c=1), den)
                for h in range(2):
                    nc.vector.tensor_scalar_mul(
                        attn_sc[:, j * 128 + h * 64:j * 128 + (h + 1) * 64],
                        pvt[bank][:, qsl * 130 + h * 65:qsl * 130 + h * 65 + 64],
                        rr[:, h:h + 1])

            pair_jobs = [(ci, c, p) for ci, c in enumerate(range(NCHUNK))
                         for p in range(NPAIR)]
            need_map = {}
            for ci, c, p in pair_jobs:
                req = set()
                for ch in range(1, c + 1):
                    req.add(f"q{p}{ch}")
                    req.add(f"k{p}{ch}")
                for sb0 in range(4, 4 * (c + 1), 2):
                    req.add(f"v{sb0}")
                need_map[(c, p)] = req

            for j, (ci, c, p) in enumerate(pair_jobs):
                kb_hi = (c * KPC + KPC) if causal else NSB
                last_pair = j == len(pair_jobs) - 1
                pull_until(need_map[(c, p)])

                pvt = [psPV.tile([128, 512], F32, tag="pv", name=f"pv{b}")
                       for b in range(2)]
                if last_pair:
                    attn_sc = asb.tile([128, 512], BF16, tag="af",
                                       name="attn_sf")
                else:
                    attn_sc = asb.tile([128, 512], BF16, tag="asb",
                                       name="attn_sc")

                pend = None     # (kb, pt) awaiting PV
                norm_q = []     # q-slices whose PV is emitted, norm pending

                def flush_norms(p=p, c=c, pvt=pvt, attn_sc=attn_sc,
                                last_pair=last_pair, norm_q=norm_q):
                    for jq in norm_q:
                        emit_norm(jq, pvt, attn_sc)
                        if last_pair:
                            # tail: PE transpose + evac + eager wo + ship
                            tp = psX.tile([128, 512], F32, tag="x",
                                          name="tp")
                            tpb = tp.bitcast(BF16)
                            nc.tensor.transpose(
                                tpb[:, 0:128],
                                attn_sc[:, jq * 128:(jq + 1) * 128],
                                ident_t[:])
                            qg = c * KPC + jq
                            nc.vector.tensor_copy(
                                at[p][:, qg * 128:(qg + 1) * 128],
                                tpb[:, 0:128])
                            emit_wo_oc(qg, 0, tail=True)
                            emit_wo_oc(qg, 1, tail=True)
                    del norm_q[:]

                budget = 450 if ci < NCHUNK - 1 else 750
                for kb in range(kb_hi):
                    sc = psS.tile([128, 2 * QCH], F32, tag="sc", name="sc")
                    pt = prb.tile([128, 2 * QCH], BF16, tag="prb", name="prb")
                    emit_scores(p, c, kb, sc)
                    emit_exp(c, kb, sc, pt)
                    pull(budget)
                    if pend is not None:
                        pkb, ppt = pend
                        emit_pv(p, c, pkb, kb_hi, ppt, pvt)
                        if causal and pkb >= 4 * c:
                            norm_q.append(pkb - 4 * c)
                        flush_norms()
                    pend = (kb, pt)
                # drain: last item's PV + its norm
                pull(300)
                pkb, ppt = pend
                emit_pv(p, c, pkb, kb_hi, ppt, pvt)
                if causal:
                    norm_q.append(pkb - 4 * c)
                else:
                    norm_q.extend(range(4))
                flush_norms()

                if not last_pair:
                    # blocked transpose of the whole pair-chunk into at[p]
                    nc.sync.dma_start_transpose(
                        at[p][:, c * QCH:(c + 1) * QCH].rearrange(
                            "v (b q) -> v b q", b=4),
                        attn_sc[:])
                    if p == NPAIR - 1:
                        fillers.append(g_wo_chunk(c))

            # leftover fillers (late wo chunks)
            pull(10**12)

    nc.compile()
    return nc


def _host_prep(x, freqs_cos, freqs_sin, wq, wk, wv, wo):
    """Build the 8 per-core input maps (numpy, bf16)."""
    import ml_dtypes

    bf16 = ml_dtypes.bfloat16

    x = np.ascontiguousarray(x, dtype=np.float32)
    cosT = np.ascontiguousarray(freqs_cos.T, dtype=np.float32)  # [32, S]
    sinT = np.ascontiguousarray(freqs_sin.T, dtype=np.float32)

    c128 = np.tile(cosT, (4, 1))                                # [128, S]
    s128 = np.tile(np.concatenate([-sinT, sinT], 0), (2, 1))
    cs128 = np.ascontiguousarray(
        np.concatenate([c128, s128], axis=1)).astype(bf16)      # [128, 2S]
    # swap permutation: psum_sw = pmat.T @ A -> sw[m] = A[sigma(m)],
    # sigma swaps the 32-halves within each 64 block.
    pmat = np.zeros((128, 128), dtype=np.float32)
    for m in range(128):
        blk, off = divmod(m, 32)
        pmat[(blk ^ 1) * 32 + off, m] = 1.0
    ident = np.eye(128, dtype=np.float32)
    # causal diag mask, transposed: mdiagT[k, q] = 0 if k <= q else -1e9
    kk, qq = np.meshgrid(np.arange(128), np.arange(128), indexing="ij")
    mdiagT = np.where(kk <= qq, 0.0, NEG_INF).astype(np.float32)
    msk = np.ascontiguousarray(
        np.concatenate([pmat, ident, mdiagT], axis=1)).astype(bf16)

    # rotate-half row permutation within each head
    rh = np.concatenate([np.arange(0, HD, 2), np.arange(1, HD, 2)])

    xT = [np.ascontiguousarray(x[b].T).astype(bf16) for b in range(B)]

    in_maps = []
    for core in range(NCORES):
        b, g = divmod(core, GROUPS)
        heads = [g * HPG + j for j in range(HPG)]
        qrows, vrows = [], []
        for h in heads:
            base = h * HD
            qrows.extend((base + rh).tolist())
            vrows.extend(range(base, base + HD))
        qrows = np.array(qrows)
        vrows = np.array(vrows)
        wqT = wq[qrows, :].T                                     # [D, 256]
        wkT = wk[qrows, :].T
        wvT = wv[vrows, :].T
        wqkvT = np.ascontiguousarray(
            np.concatenate([wqT, wkT, wvT], axis=1)).astype(bf16)
        woT = np.ascontiguousarray(wo[:, vrows].T).astype(bf16)  # [256, D]
        in_maps.append({
            "xT": xT[b], "wqkvT": wqkvT, "woT": woT,
            "cs128": cs128, "msk": msk,
        })
    return in_maps


def _mask_kind(mask):
    m = np.asarray(mask).reshape(S, S)
    if not np.any(m):
        return "zeros"
    qq, kk = np.meshgrid(np.arange(S), np.arange(S), indexing="ij")
    causal = np.where(kk <= qq, 0.0, NEG_INF).astype(np.float32)  # [q, k]
    if np.array_equal(m, causal):
        return "causal"
    return "general"


def _reference_host(x, freqs_cos, freqs_sin, mask, wq, wk, wv, wo):
    """Correctness fallback for arbitrary masks (host numpy, float64)."""
    b, s, d = x.shape
    hd = d // H
    xq = (x @ wq.T).reshape(b, s, H, hd)
    xk = (x @ wk.T).reshape(b, s, H, hd)
    xv = (x @ wv.T).reshape(b, s, H, hd)

    def rope(t):
        tr = t.reshape(b, s, H, hd // 2, 2)
        t0, t1 = tr[..., 0], tr[..., 1]
        cos = freqs_cos[None, :, None, :]
        sin = freqs_sin[None, :, None, :]
        return np.stack([t0 * cos - t1 * sin, t0 * sin + t1 * cos],
                        -1).reshape(b, s, H, hd)

    xq, xk = rope(xq), rope(xk)
    sc = np.einsum("bqhd,bkhd->bhqk", xq, xk) / np.sqrt(hd) + mask
    sc = sc - sc.max(-1, keepdims=True)
    e = np.exp(sc)
    pr = e / e.sum(-1, keepdims=True)
    o = np.einsum("bhqk,bkhd->bqhd", pr, xv).reshape(b, s, d)
    return (o @ wo.T).astype(np.float32)


def kernel(x, freqs_cos, freqs_sin, mask, wq, wk, wv, wo):
    kind = _mask_kind(mask)
    if kind == "general":
        return _reference_host(np.asarray(x, np.float64),
                               np.asarray(freqs_cos, np.float64),
                               np.asarray(freqs_sin, np.float64),
                               np.asarray(mask, np.float64),
                               np.asarray(wq, np.float64),
                               np.asarray(wk, np.float64),
                               np.asarray(wv, np.float64),
                               np.asarray(wo, np.float64))

    if kind not in _PROG_CACHE:
        _PROG_CACHE[kind] = _build_program(kind)
    nc = _PROG_CACHE[kind]

    in_maps = _host_prep(np.asarray(x, np.float32),
                         np.asarray(freqs_cos, np.float32),
                         np.asarray(freqs_sin, np.float32),
                         np.asarray(wq, np.float32),
                         np.asarray(wk, np.float32),
                         np.asarray(wv, np.float32),
                         np.asarray(wo, np.float32))
    res = run_bass_kernel_spmd(nc, in_maps, list(range(NCORES)))
    out = np.zeros((B, S, D), dtype=np.float32)
    for core in range(NCORES):
        out[core // GROUPS] += np.asarray(res.results[core]["out"],
                                          dtype=np.float32)
    return out


# revision 26
# speedup vs baseline: 1.1759x; 1.0462x over previous
"""Trainium2 Bass kernel for nn_Attention_12515534700827.

Multi-head causal attention with RoPE: B=2, S=2048, D=1024, H=16, HD=64.
Sharding: 8 cores = 2 (batch) x 4 (head groups of 4 heads). Each core
computes its 4 heads' attention + its slice of the wo projection; the host
sums the 4 partial outputs per batch (the "all-reduce after wo").

v3 (fused single-stream): projections, attention, and the wo projection are
emitted as ONE interleaved instruction stream so the ScalarE exp stream (the
second-largest engine load) overlaps the projection/wo matmuls instead of
running in its own phase.

Key differences vs v2:
  - PV computed with probs as the STATIONARY operand and V' ([V|ones]) as
    the MOVING operand: out[q, vd|den] per (head, q-slice).  The moving free
    dim drops from ~512 to 65, halving PV cost; the softmax denominator
    arrives as psum column 64 per head so normalization becomes a
    per-partition scalar multiply (no PE broadcast, no reciprocal of a
    [64,1024] tile, no partition-shift DMA bounce).
  - The resulting attn tiles are [q, vd]; wo needs [vd, q].  Transposed via
    dma_start_transpose (XBAR 16x128 tiles, cheap on the DMA engines)
    straight into the persistent at[] tiles; the last pair uses PE
    transposes so the tail isn't gated on a DMA round trip.
  - Fused emission: after the head block (V sb0-3 + Q/K chunk0 for both
    pairs over x cols 0:1024), attention items start immediately; the
    remaining V blocks, Q/K chunks, rope chains and deferred wo tiles are
    "fillers" pulled between items to keep the PE dense while ScalarE
    streams the exps.
  - wo(c) is deferred ~2 chunks so it lands as filler in the late,
    otherwise Act-bound stretch.
  - Inputs land in few large DMAs (HWDGE descriptor time ~0.6us each).

PSUM budget (8 banks): scores 2x[128,1024] (4) + PV 2x[128,512] (2) +
misc single-shot rotation psX 2x[128,512] (2: rope swaps, V pairs, wo,
tail transposes).  PV packs 2 q-slices x 2 heads x 65 cols per bank with
one accumulation-group start/stop per bank (hardware clears has_written at
bank granularity).
"""

import sys

if "/opt/trn_rl_repo" not in sys.path:
    sys.path.insert(0, "/opt/trn_rl_repo")

import numpy as np

import concourse.mybir as mybir
import concourse.tile as tile
from concourse import bacc
from concourse.bass_utils import run_bass_kernel_spmd

F32 = mybir.dt.float32
BF16 = mybir.dt.bfloat16
AF = mybir.ActivationFunctionType

B, S, D, H, HD = 2, 2048, 1024, 16, 64
NCORES = 8
GROUPS = 4            # head groups (cores per batch)
HPG = H // GROUPS     # heads per core = 4
NPAIR = HPG // 2      # head pairs per core = 2
NEG_INF = -1e9
SM_SCALE = 1.0 / float(np.sqrt(HD))  # 0.125

NIT = D // 128        # 8 contraction tiles
NSB = S // 128        # 16 seq blocks
QCH = 512             # attention q-chunk
NCHUNK = S // QCH     # 4
KPC = QCH // 128      # k/q 128-blocks per chunk = 4

_PROG_CACHE = {}


def _build_program(mask_kind: str):
    """mask_kind: 'causal' (trimmed + diag mask) or 'zeros' (full)."""
    causal = mask_kind == "causal"
    nc = bacc.Bacc("TRN2", target_bir_lowering=False, debug=False,
                   num_devices=NCORES)

    xT_d = nc.dram_tensor("xT", [D, S], BF16, kind="ExternalInput").ap()
    # per 128-row block: [wq | wk | wv] column slices
    wqkv_d = nc.dram_tensor("wqkvT", [D, 3 * HPG * HD], BF16,
                            kind="ExternalInput").ap()
    woT_d = nc.dram_tensor("woT", [HPG * HD, D], BF16, kind="ExternalInput").ap()
    cs_d = nc.dram_tensor("cs128", [128, 2 * S], BF16, kind="ExternalInput").ap()
    # [pmat | ident | mdiagT]
    msk_d = nc.dram_tensor("msk", [128, 384], BF16, kind="ExternalInput").ap()
    out_d = nc.dram_tensor("out", [S, D], BF16, kind="ExternalOutput").ap()

    with tile.TileContext(nc) as tc:
        from contextlib import ExitStack

        with ExitStack() as root:
            pers = root.enter_context(tc.tile_pool(name="pers", bufs=1))

            # ---- persistent SBUF tiles ----
            qt = [pers.tile([128, S], BF16, tag=f"qt{p}", name=f"qt{p}")
                  for p in range(NPAIR)]
            kt = [pers.tile([128, S], BF16, tag=f"kt{p}", name=f"kt{p}")
                  for p in range(NPAIR)]
            # V' per (pair, s-block): [128,130] = V_A|ones|V_B|ones
            vp = [[pers.tile([128, 130], BF16, tag=f"vp{p}_{sb}",
                             name=f"vp{p}_{sb}")
                   for sb in range(NSB)] for p in range(NPAIR)]
            # attnT per pair: [vd(2 heads x 64), S]
            at = [pers.tile([128, S], BF16, tag=f"at{p}", name=f"at{p}")
                  for p in range(NPAIR)]
            wo2 = pers.tile([128, NPAIR * D], BF16, tag="wo2", name="wo2")
            wo_t = [wo2[:, p * D:(p + 1) * D] for p in range(NPAIR)]
            msk_t = pers.tile([128, 384], BF16, tag="msk", name="msk")
            pm_t = msk_t[:, 0:128]
            ident_t = msk_t[:, 128:256]
            tri_t = msk_t[:, 256:384]   # 0/1 lower-k triangle (k <= q)
            ones1 = pers.tile([1, 64], BF16, tag="ones1", name="ones1")

            # all memsets first so the Pool engine is done before attention
            nc.gpsimd.memset(ones1[:], 1.0)
            for p in range(NPAIR):
                for sb in range(NSB):
                    nc.gpsimd.memset(vp[p][sb][:, 64:65], 1.0)
                    nc.gpsimd.memset(vp[p][sb][:, 129:130], 1.0)

            ld = root.enter_context(tc.tile_pool(name="ld", bufs=1))
            sbA = root.enter_context(tc.tile_pool(name="sbA", bufs=2))
            prb = root.enter_context(tc.tile_pool(name="prb", bufs=5))
            asb = root.enter_context(tc.tile_pool(name="asb", bufs=3))
            nrm = root.enter_context(tc.tile_pool(name="nrm", bufs=4))
            osb = root.enter_context(tc.tile_pool(name="osb", bufs=4))
            psS = root.enter_context(
                tc.tile_pool(name="psS", bufs=2, space="PSUM"))
            psPV = root.enter_context(
                tc.tile_pool(name="psPV", bufs=2, space="PSUM"))
            psX = root.enter_context(
                tc.tile_pool(name="psX", bufs=2, space="PSUM"))

            # PE warm-up: dummy matmuls during the otherwise-dead input-DMA
            # window release the HAM clock gate so the first real
            # projections run at full clock
            wt = psX.tile([64, 64], F32, tag="x", name="warm")
            for _ in range(180):
                nc.tensor.matmul(wt[:], ones1[:], ones1[:],
                                 start=True, stop=True)

            # ---- input DMAs (few, large; wqkv split so it=0 lands early)
            xts = [ld.tile([128, S], BF16, tag=f"xt{it}", name=f"xt{it}")
                   for it in range(NIT)]
            wqkv = ld.tile([128, NIT * 3 * HPG * HD], BF16, tag="wqkv",
                           name="wqkv")
            wq_t = [wqkv[:, it * 768:it * 768 + 256] for it in range(NIT)]
            wk_t = [wqkv[:, it * 768 + 256:it * 768 + 512] for it in range(NIT)]
            wv_t = [wqkv[:, it * 768 + 512:it * 768 + 768] for it in range(NIT)]
            cs_t = ld.tile([128, 2 * S], BF16, tag="cs128", name="cs128")
            c_t = cs_t[:, 0:S]
            s_t = cs_t[:, S:2 * S]

            for half in range(2):
                its = slice(half * 4 * 128, (half + 1) * 4 * 128)
                nc.sync.dma_start(
                    out=wqkv[:, half * 3072:(half + 1) * 3072].rearrange(
                        "p (i c) -> p i c", i=4),
                    in_=wqkv_d[its, :].rearrange("(i p) c -> p i c", p=128))
            for it in range(NIT):
                sl = slice(it * 128, (it + 1) * 128)
                nc.sync.dma_start(out=xts[it][:, 0:1024],
                                  in_=xT_d[sl, 0:1024])
            nc.sync.dma_start(out=msk_t[:], in_=msk_d[:])
            # cos/sin for chunks 0-1, then 2-3 (rope chunk 0 needs it early)
            nc.sync.dma_start(
                out=cs_t[:].rearrange("p (h c) -> p h c", h=2)[:, :, 0:1024],
                in_=cs_d[:].rearrange("p (h c) -> p h c", h=2)[:, :, 0:1024])
            nc.sync.dma_start(
                out=cs_t[:].rearrange("p (h c) -> p h c", h=2)[:, :, 1024:2048],
                in_=cs_d[:].rearrange("p (h c) -> p h c", h=2)[:, :, 1024:2048])
            for it in range(NIT):
                sl = slice(it * 128, (it + 1) * 128)
                nc.sync.dma_start(out=xts[it][:, 1024:2048],
                                  in_=xT_d[sl, 1024:2048])
            nc.sync.dma_start(
                out=wo2[:].rearrange("p (a d) -> p a d", a=NPAIR),
                in_=woT_d[:].rearrange("(a p) d -> p a d", a=NPAIR, p=128))

            # ---------------- helpers ----------------
            def v_finish(sb, ps):
                """psum [128, 256] (pair-packed V) -> vp tiles, both pairs."""
                for p in range(NPAIR):
                    src = ps[:, p * 128:(p + 1) * 128] \
                        .rearrange("p (b c) -> p b c", b=2, c=64)
                    dst = vp[p][sb][:, 0:130] \
                        .rearrange("p (b c) -> p b c", b=2, c=65)[:, :, 0:64]
                    nc.vector.tensor_copy(dst, src)

            def rope_copy(ps, on_act=True):
                a_sb = sbA.tile([128, 512], BF16, tag="a_sb", name="a_sb")
                if on_act:
                    nc.scalar.activation(a_sb[:], ps[:], AF.Copy)
                else:
                    nc.vector.tensor_copy(a_sb[:], ps[:])
                return a_sb

            def rope_finish(a_sb, dst, p, ch):
                """rot = A*C + swap(A)*S into dst[p][:, chunk]."""
                qs = slice(ch * 512, (ch + 1) * 512)
                t1 = sbA.tile([128, 512], BF16, tag="t1", name="t1")
                nc.vector.tensor_mul(t1[:], a_sb[:], c_t[:, qs])
                sw = psX.tile([128, 512], F32, tag="x", name="psSW")
                nc.tensor.matmul(sw[:], pm_t[:], a_sb[:],
                                 start=True, stop=True)
                t2 = sbA.tile([128, 512], BF16, tag="t2", name="t2")
                nc.vector.tensor_mul(t2[:], sw[:], s_t[:, qs])
                nc.vector.tensor_add(dst[p][:, qs], t1[:], t2[:])

            # ---------------- head block ----------------
            # it-major over x cols 0:1024: V s-blocks 0-3 + Q/K chunk 0 for
            # both pairs, so both pairs' chunk-0 attention unlocks first.
            hv = [psPV.tile([128, 512], F32, tag="pv", name=f"hv{b}")
                  for b in range(2)]
            hq = [psS.tile([128, 2 * QCH], F32, tag="sc", name=f"hq{p}")
                  for p in range(NPAIR)]
            for it in range(NIT):
                st, sp = it == 0, it == NIT - 1
                for sb in range(4):
                    b, o = divmod(sb, 2)
                    nc.tensor.matmul(hv[b][:, o * 256:(o + 1) * 256],
                                     xts[it][:, sb * 128:(sb + 1) * 128],
                                     wv_t[it][:],
                                     start=(st and o == 0),
                                     stop=(sp and o == 1))
                for p in range(NPAIR):
                    pc = slice(p * 128, (p + 1) * 128)
                    nc.tensor.matmul(hq[p][:, 0:512], wq_t[it][:, pc],
                                     xts[it][:, 0:512], start=st, stop=sp)
                    nc.tensor.matmul(hq[p][:, 512:1024], wk_t[it][:, pc],
                                     xts[it][:, 0:512], start=st, stop=sp)
            for sb in range(4):
                b, o = divmod(sb, 2)
                v_finish(sb, hv[b][:, o * 256:(o + 1) * 256])
            # rope pair 0's chunk-0 q/k now (unblocks the first items);
            # pair 1's chunk 0 becomes the first filler unit
            a_q0 = rope_copy(hq[0][:, 0:512])
            a_k0 = rope_copy(hq[0][:, 512:1024])
            rope_finish(a_q0, qt, 0, 0)
            rope_finish(a_k0, kt, 0, 0)

            # ---------------- filler machinery ----------------
            done_units = set()

            def g_v_unit(sb0):
                """V s-blocks (sb0, sb0+1): packed 2-per-bank accumulation."""
                t = psX.tile([128, 512], F32, tag="x", name=f"v{sb0}")
                for it in range(NIT):
                    st, sp = it == 0, it == NIT - 1
                    for o in range(2):
                        nc.tensor.matmul(
                            t[:, o * 256:(o + 1) * 256],
                            xts[it][:, (sb0 + o) * 128:(sb0 + o + 1) * 128],
                            wv_t[it][:],
                            start=(st and o == 0), stop=(sp and o == 1))
                        yield 107
                v_finish(sb0, t[:, 0:256])
                v_finish(sb0 + 1, t[:, 256:512])
                done_units.add(f"v{sb0}")
                yield 0

            rope_pend = [None]

            def g_p1_head_rope():
                """pair 1's chunk-0 rope (head produced the psum)."""
                a_q1 = rope_copy(hq[1][:, 0:512])
                a_k1 = rope_copy(hq[1][:, 512:1024])
                rope_finish(a_q1, qt, 1, 0)
                done_units.add("q10")
                yield 213
                rope_finish(a_k1, kt, 1, 0)
                done_units.add("k10")
                yield 213

            def g_qk_unit(p, w, ch):
                wt = wq_t if w == "q" else wk_t
                dst = qt if w == "q" else kt
                t = psX.tile([128, 512], F32, tag="x", name=f"qk{p}{w}{ch}")
                pc = slice(p * 128, (p + 1) * 128)
                for it in range(NIT):
                    nc.tensor.matmul(t[:], wt[it][:, pc],
                                     xts[it][:, ch * 512:(ch + 1) * 512],
                                     start=(it == 0), stop=(it == NIT - 1))
                    yield 213
                a_sb = rope_copy(t, on_act=(ch < 2))
                if rope_pend[0] is not None:
                    pa, pdst, pp, pch, pname = rope_pend[0]
                    rope_finish(pa, pdst, pp, pch)
                    done_units.add(pname)
                    yield 213
                rope_pend[0] = (a_sb, dst, p, ch, f"{w}{p}{ch}")

            def flush_pend():
                if rope_pend[0] is not None:
                    pa, pdst, pp, pch, pname = rope_pend[0]
                    rope_finish(pa, pdst, pp, pch)
                    done_units.add(pname)
                    rope_pend[0] = None

            def g_rope_flush():
                if rope_pend[0] is not None:
                    flush_pend()
                    yield 213

            ob_pend = {}

            def emit_wo_oc(sb, oc, tail=False):
                ssl = slice(sb * 128, (sb + 1) * 128)
                osl = slice(oc * 512, (oc + 1) * 512)
                ps = psX.tile([128, 512], F32, tag="x", name="psW")
                for p in range(NPAIR):
                    nc.tensor.matmul(
                        ps[:], at[p][:, ssl], wo_t[p][:, osl],
                        start=(p == 0), stop=(p == NPAIR - 1))
                if sb not in ob_pend:
                    ob_pend[sb] = osb.tile([128, 1024], BF16, tag="osb",
                                           name="osb")
                ob = ob_pend[sb]
                if tail and oc == 0:
                    # ScalarE is idle at the tail: evac halves in parallel
                    nc.scalar.activation(ob[:, osl], ps[:], AF.Copy)
                else:
                    nc.vector.tensor_copy(ob[:, osl], ps[:])
                if tail:
                    nc.sync.dma_start(out=out_d[ssl, osl], in_=ob[:, osl])
                    if oc == 1:
                        ob_pend.pop(sb)
                elif oc == 1:
                    nc.sync.dma_start(out=out_d[ssl, :],
                                      in_=ob_pend.pop(sb)[:])

            def g_wo_chunk(c):
                for sb in range(c * KPC, (c + 1) * KPC):
                    for oc in range(2):
                        emit_wo_oc(sb, oc)
                        yield 426

            # filler order: chunk-1 q/k (x cols 512:1024, already loaded)
            # before the xB-dependent V blocks; V blocks paced so vp(sb) is
            # ready ~when chunk sb//4's PV needs it.
            fillers = [
                g_p1_head_rope(),
                g_qk_unit(0, "q", 1), g_qk_unit(0, "k", 1),
                g_v_unit(4),
                g_qk_unit(1, "q", 1), g_qk_unit(1, "k", 1),
                g_v_unit(6), g_v_unit(8),
                g_qk_unit(0, "q", 2), g_qk_unit(0, "k", 2),
                g_v_unit(10),
                g_qk_unit(1, "q", 2), g_qk_unit(1, "k", 2),
                g_v_unit(12),
                g_qk_unit(0, "q", 3), g_qk_unit(0, "k", 3),
                g_v_unit(14),
                g_qk_unit(1, "q", 3), g_qk_unit(1, "k", 3),
                g_rope_flush(),
            ]
            fill_iq = [0]

            def pull(budget_ns):
                got = 0
                while fill_iq[0] < len(fillers):
                    g = fillers[fill_iq[0]]
                    try:
                        while got < budget_ns:
                            got += next(g)
                    except StopIteration:
                        fill_iq[0] += 1
                        continue
                    break
                return got

            def pull_until(units):
                while not units <= done_units:
                    # the last missing unit may be parked in the rope pend
                    if rope_pend[0] is not None and \
                            units <= (done_units | {rope_pend[0][4]}):
                        flush_pend()
                        return
                    if pull(600) == 0:
                        flush_pend()
                        assert units <= done_units, (
                            f"filler units {units - done_units} never emitted")
                        return

            # ---------------- attention ----------------
            def emit_scores(p, c, kb, sc):
                q0 = c * QCH
                k0 = kb * 128
                trim = max(q0, k0) if causal else q0
                t_off = trim - q0
                for h in range(2):
                    hsl = slice(h * 64, (h + 1) * 64)
                    nc.tensor.matmul(
                        sc[:, h * QCH + t_off:(h + 1) * QCH],
                        kt[p][hsl, k0:k0 + 128],
                        qt[p][hsl, trim:q0 + QCH],
                        start=True, stop=True)

            def emit_exp(c, kb, sc, pt):
                """exp (trimmed), then for diag items zero the upper-k
                triangle of the 128-block on the idle GPSIMD engine."""
                q0 = c * QCH
                t_off = (max(q0, kb * 128) - q0) if causal else 0
                if t_off == 0:
                    nc.scalar.activation(pt[:, 0:2 * QCH], sc[:, 0:2 * QCH],
                                         AF.Exp, scale=SM_SCALE)
                else:
                    sc3 = sc[:, 0:2 * QCH].rearrange(
                        "p (b c) -> p b c", b=2, c=QCH)[:, :, t_off:]
                    pt3 = pt[:, 0:2 * QCH].rearrange(
                        "p (b c) -> p b c", b=2, c=QCH)[:, :, t_off:]
                    nc.scalar.activation(pt3, sc3, AF.Exp, scale=SM_SCALE)
                if causal and kb * 128 >= q0:
                    dg = pt[:, 0:2 * QCH].rearrange(
                        "p (b c) -> p b c", b=2,
                        c=QCH)[:, :, t_off:t_off + 128]
                    tri3 = tri_t.rearrange("p (b c) -> p b c", b=1)
                    nc.gpsimd.tensor_mul(dg, dg,
                                         tri3.broadcast_to([128, 2, 128]))

            def emit_pv(p, c, kb, kb_hi, pt, pvt):
                qs_lo = max(0, kb - 4 * c) if causal else 0
                for qs in range(qs_lo, 4):
                    bank, qsl = divmod(qs, 2)
                    last_kb = (4 * c + bank * 2 + 1) if causal else kb_hi - 1
                    for h in range(2):
                        col = qsl * 130 + h * 65
                        nc.tensor.matmul(
                            pvt[bank][:, col:col + 65],
                            pt[:, h * QCH + qs * 128:h * QCH + qs * 128 + 128],
                            vp[p][kb][:, h * 65:h * 65 + 65],
                            start=(kb == 0 and qsl == 0 and h == 0),
                            stop=(kb == last_kb and qsl == 1 and h == 1))

            def emit_norm(j, pvt, attn_sc, h1_act=False):
                """normalize q-slice j of the pair-chunk into attn_sc."""
                bank, qsl = divmod(j, 2)
                rr = nrm.tile([128, 2], F32, tag="rr", name="rr")
                den = pvt[bank][:, qsl * 130:qsl * 130 + 130].rearrange(
                    "p (h c) -> p h c", h=2)[:, :, 64:65]
                nc.vector.reciprocal_approx_fast(
                    rr[:].rearrange("p (h c) -> p h c", c=1), den)
                for h in range(2):
                    dst = attn_sc[:, j * 128 + h * 64:j * 128 + (h + 1) * 64]
                    src = pvt[bank][:,
                                    qsl * 130 + h * 65:qsl * 130 + h * 65 + 64]
                    if h == 1 and h1_act:
                        # tail: h1 on the (by now idle) ScalarE so the two
                        # head halves normalize in parallel
                        nc.scalar.activation(dst, src, AF.Copy,
                                             scale=rr[:, 1:2])
                    else:
                        nc.vector.tensor_scalar_mul(dst, src, rr[:, h:h + 1])

            pair_jobs = [(ci, c, p) for ci, c in enumerate(range(NCHUNK))
                         for p in range(NPAIR)]
            dmatp_pend = [None]
            # group gate: only this chunk's q rope (+ pair 1's chunk-0 k);
            # k ropes and V blocks gate per-item below so their filler
            # units can slide into this group's item stream
            need_map = {}
            for ci, c, p in pair_jobs:
                req = set()
                if not (p == 0 and c == 0):
                    req.add(f"q{p}{c}")
                if p == 1 and c == 0:
                    req.add("k10")
                need_map[(c, p)] = req

            for j, (ci, c, p) in enumerate(pair_jobs):
                kb_hi = (c * KPC + KPC) if causal else NSB
                last_pair = j == len(pair_jobs) - 1
                pull_until(need_map[(c, p)])

                pvt = [psPV.tile([128, 512], F32, tag="pv", name=f"pv{b}")
                       for b in range(2)]
                if last_pair:
                    attn_sc = asb.tile([128, 512], BF16, tag="af",
                                       name="attn_sf")
                else:
                    attn_sc = asb.tile([128, 512], BF16, tag="asb",
                                       name="attn_sc")

                pend = None     # (kb, pt) awaiting PV
                norm_q = []     # q-slices whose PV is emitted, norm pending

                def flush_norms(p=p, c=c, pvt=pvt, attn_sc=attn_sc,
                                last_pair=last_pair, norm_q=norm_q):
                    for jq in norm_q:
                        emit_norm(jq, pvt, attn_sc, h1_act=last_pair)
                        if last_pair:
                            # tail: PE transpose + evac + eager wo + ship
                            tp = psX.tile([128, 512], F32, tag="x",
                                          name="tp")
                            tpb = tp.bitcast(BF16)
                            nc.tensor.transpose(
                                tpb[:, 0:128],
                                attn_sc[:, jq * 128:(jq + 1) * 128],
                                ident_t[:])
                            qg = c * KPC + jq
                            nc.vector.tensor_copy(
                                at[p][:, qg * 128:(qg + 1) * 128],
                                tpb[:, 0:128])
                            emit_wo_oc(qg, 0, tail=True)
                            emit_wo_oc(qg, 1, tail=True)
                    del norm_q[:]

                budget = (350, 350, 350, 460)[ci]
                for kb in range(kb_hi):
                    kc = kb // 4
                    if kc >= 1 or p == 1:
                        kname = f"k{p}{kc}"
                        if kname not in done_units:
                            pull_until({kname})
                    sc = psS.tile([128, 2 * QCH], F32, tag="sc", name="sc")
                    pt = prb.tile([128, 2 * QCH], BF16, tag="prb", name="prb")
                    emit_scores(p, c, kb, sc)
                    emit_exp(c, kb, sc, pt)
                    if kb == 2 and dmatp_pend[0] is not None:
                        # previous pair-chunk's attn transpose: deferred here
                        # so its sem wait is satisfied on arrival and doesn't
                        # head-of-line block the SP DMA queue
                        dmatp_pend[0]()
                        dmatp_pend[0] = None
                    pull(budget)
                    if pend is not None:
                        pkb, ppt = pend
                        if pkb >= 4:
                            vname = f"v{pkb & ~1}"
                            if vname not in done_units:
                                pull_until({vname})
                        emit_pv(p, c, pkb, kb_hi, ppt, pvt)
                        if causal and pkb >= 4 * c:
                            norm_q.append(pkb - 4 * c)
                        flush_norms()
                    pend = (kb, pt)
                # drain: last item's PV + its norm
                pull(300)
                pkb, ppt = pend
                if pkb >= 4:
                    vname = f"v{pkb & ~1}"
                    if vname not in done_units:
                        pull_until({vname})
                emit_pv(p, c, pkb, kb_hi, ppt, pvt)
                if causal:
                    norm_q.append(pkb - 4 * c)
                else:
                    norm_q.extend(range(4))
                flush_norms()

                if not last_pair:
                    # blocked transpose of the whole pair-chunk into at[p];
                    # emission deferred into the next pair-group
                    def mk_tp(p=p, c=c, attn_sc=attn_sc):
                        def emit():
                            nc.sync.dma_start_transpose(
                                at[p][:, c * QCH:(c + 1) * QCH].rearrange(
                                    "v (b q) -> v b q", b=4),
                                attn_sc[:])
                        return emit

                    dmatp_pend[0] = mk_tp()
                    if p == NPAIR - 1:
                        fillers.append(g_wo_chunk(c))

            # leftover fillers (late wo chunks)
            pull(10**12)

    nc.compile()
    return nc


def _host_prep(x, freqs_cos, freqs_sin, wq, wk, wv, wo):
    """Build the 8 per-core input maps (numpy, bf16)."""
    import ml_dtypes

    bf16 = ml_dtypes.bfloat16

    x = np.ascontiguousarray(x, dtype=np.float32)
    cosT = np.ascontiguousarray(freqs_cos.T, dtype=np.float32)  # [32, S]
    sinT = np.ascontiguousarray(freqs_sin.T, dtype=np.float32)

    c128 = np.tile(cosT, (4, 1))                                # [128, S]
    s128 = np.tile(np.concatenate([-sinT, sinT], 0), (2, 1))
    cs128 = np.ascontiguousarray(
        np.concatenate([c128, s128], axis=1)).astype(bf16)      # [128, 2S]
    # swap permutation: psum_sw = pmat.T @ A -> sw[m] = A[sigma(m)],
    # sigma swaps the 32-halves within each 64 block.
    pmat = np.zeros((128, 128), dtype=np.float32)
    for m in range(128):
        blk, off = divmod(m, 32)
        pmat[(blk ^ 1) * 32 + off, m] = 1.0
    ident = np.eye(128, dtype=np.float32)
    # causal diag 0/1 triangle: tri01[k, q] = 1 if k <= q else 0
    kk, qq = np.meshgrid(np.arange(128), np.arange(128), indexing="ij")
    tri01 = (kk <= qq).astype(np.float32)
    msk = np.ascontiguousarray(
        np.concatenate([pmat, ident, tri01], axis=1)).astype(bf16)

    # rotate-half row permutation within each head
    rh = np.concatenate([np.arange(0, HD, 2), np.arange(1, HD, 2)])

    xT = [np.ascontiguousarray(x[b].T).astype(bf16) for b in range(B)]

    in_maps = []
    for core in range(NCORES):
        b, g = divmod(core, GROUPS)
        heads = [g * HPG + j for j in range(HPG)]
        qrows, vrows = [], []
        for h in heads:
            base = h * HD
            qrows.extend((base + rh).tolist())
            vrows.extend(range(base, base + HD))
        qrows = np.array(qrows)
        vrows = np.array(vrows)
        wqT = wq[qrows, :].T                                     # [D, 256]
        wkT = wk[qrows, :].T
        wvT = wv[vrows, :].T
        wqkvT = np.ascontiguousarray(
            np.concatenate([wqT, wkT, wvT], axis=1)).astype(bf16)
        woT = np.ascontiguousarray(wo[:, vrows].T).astype(bf16)  # [256, D]
        in_maps.append({
            "xT": xT[b], "wqkvT": wqkvT, "woT": woT,
            "cs128": cs128, "msk": msk,
        })
    return in_maps


def _mask_kind(mask):
    m = np.asarray(mask).reshape(S, S)
    if not np.any(m):
        return "zeros"
    qq, kk = np.meshgrid(np.arange(S), np.arange(S), indexing="ij")
    causal = np.where(kk <= qq, 0.0, NEG_INF).astype(np.float32)  # [q, k]
    if np.array_equal(m, causal):
        return "causal"
    return "general"


def _reference_host(x, freqs_cos, freqs_sin, mask, wq, wk, wv, wo):
    """Correctness fallback for arbitrary masks (host numpy, float64)."""
    b, s, d = x.shape
    hd = d // H
    xq = (x @ wq.T).reshape(b, s, H, hd)
    xk = (x @ wk.T).reshape(b, s, H, hd)
    xv = (x @ wv.T).reshape(b, s, H, hd)

    def rope(t):
        tr = t.reshape(b, s, H, hd // 2, 2)
        t0, t1 = tr[..., 0], tr[..., 1]
        cos = freqs_cos[None, :, None, :]
        sin = freqs_sin[None, :, None, :]
        return np.stack([t0 * cos - t1 * sin, t0 * sin + t1 * cos],
                        -1).reshape(b, s, H, hd)

    xq, xk = rope(xq), rope(xk)
    sc = np.einsum("bqhd,bkhd->bhqk", xq, xk) / np.sqrt(hd) + mask
    sc = sc - sc.max(-1, keepdims=True)
    e = np.exp(sc)
    pr = e / e.sum(-1, keepdims=True)
    o = np.einsum("bhqk,bkhd->bqhd", pr, xv).reshape(b, s, d)
    return (o @ wo.T).astype(np.float32)


def kernel(x, freqs_cos, freqs_sin, mask, wq, wk, wv, wo):
    kind = _mask_kind(mask)
    if kind == "general":
        return _reference_host(np.asarray(x, np.float64),
                               np.asarray(freqs_cos, np.float64),
                               np.asarray(freqs_sin, np.float64),
                               np.asarray(mask, np.float64),
                               np.asarray(wq, np.float64),
                               np.asarray(wk, np.float64),
                               np.asarray(wv, np.float64),
                               np.asarray(wo, np.float64))

    if kind not in _PROG_CACHE:
        _PROG_CACHE[kind] = _build_program(kind)
    nc = _PROG_CACHE[kind]

    in_maps = _host_prep(np.asarray(x, np.float32),
                         np.asarray(freqs_cos, np.float32),
                         np.asarray(freqs_sin, np.float32),
                         np.asarray(wq, np.float32),
                         np.asarray(wk, np.float32),
                         np.asarray(wv, np.float32),
                         np.asarray(wo, np.float32))
    res = run_bass_kernel_spmd(nc, in_maps, list(range(NCORES)))
    out = np.zeros((B, S, D), dtype=np.float32)
    for core in range(NCORES):
        out[core // GROUPS] += np.asarray(res.results[core]["out"],
                                          dtype=np.float32)
    return out


# revision 47
# speedup vs baseline: 1.1769x; 1.0009x over previous
"""Trainium2 Bass kernel for nn_Attention_12515534700827.

Multi-head causal attention with RoPE: B=2, S=2048, D=1024, H=16, HD=64.
Sharding: 8 cores = 2 (batch) x 4 (head groups of 4 heads). Each core
computes its 4 heads' attention + its slice of the wo projection; the host
sums the 4 partial outputs per batch (the "all-reduce after wo").

v3 (fused single-stream): projections, attention, and the wo projection are
emitted as ONE interleaved instruction stream so the ScalarE exp stream (the
second-largest engine load) overlaps the projection/wo matmuls instead of
running in its own phase.

Key differences vs v2:
  - PV computed with probs as the STATIONARY operand and V' ([V|ones]) as
    the MOVING operand: out[q, vd|den] per (head, q-slice).  The moving free
    dim drops from ~512 to 65, halving PV cost; the softmax denominator
    arrives as psum column 64 per head so normalization becomes a
    per-partition scalar multiply (no PE broadcast, no reciprocal of a
    [64,1024] tile, no partition-shift DMA bounce).
  - The resulting attn tiles are [q, vd]; wo needs [vd, q].  Transposed via
    dma_start_transpose (XBAR 16x128 tiles, cheap on the DMA engines)
    straight into the persistent at[] tiles; the last pair uses PE
    transposes so the tail isn't gated on a DMA round trip.
  - Fused emission: after the head block (V sb0-3 + Q/K chunk0 for both
    pairs over x cols 0:1024), attention items start immediately; the
    remaining V blocks, Q/K chunks, rope chains and deferred wo tiles are
    "fillers" pulled between items to keep the PE dense while ScalarE
    streams the exps.
  - wo(c) is deferred ~2 chunks so it lands as filler in the late,
    otherwise Act-bound stretch.
  - Inputs land in few large DMAs (HWDGE descriptor time ~0.6us each).

PSUM budget (8 banks): scores 2x[128,1024] (4) + PV 2x[128,512] (2) +
misc single-shot rotation psX 2x[128,512] (2: rope swaps, V pairs, wo,
tail transposes).  PV packs 2 q-slices x 2 heads x 65 cols per bank with
one accumulation-group start/stop per bank (hardware clears has_written at
bank granularity).
"""

import sys

if "/opt/trn_rl_repo" not in sys.path:
    sys.path.insert(0, "/opt/trn_rl_repo")

import numpy as np

import concourse.mybir as mybir
import concourse.tile as tile
from concourse import bacc
from concourse.bass_utils import run_bass_kernel_spmd

F32 = mybir.dt.float32
BF16 = mybir.dt.bfloat16
AF = mybir.ActivationFunctionType

B, S, D, H, HD = 2, 2048, 1024, 16, 64
NCORES = 8
GROUPS = 4            # head groups (cores per batch)
HPG = H // GROUPS     # heads per core = 4
NPAIR = HPG // 2      # head pairs per core = 2
NEG_INF = -1e9
SM_SCALE = 1.0 / float(np.sqrt(HD))  # 0.125

NIT = D // 128        # 8 contraction tiles
NSB = S // 128        # 16 seq blocks
QCH = 512             # attention q-chunk
NCHUNK = S // QCH     # 4
KPC = QCH // 128      # k/q 128-blocks per chunk = 4

_PROG_CACHE = {}


def _build_program(mask_kind: str):
    """mask_kind: 'causal' (trimmed + diag mask) or 'zeros' (full)."""
    causal = mask_kind == "causal"
    nc = bacc.Bacc("TRN2", target_bir_lowering=False, debug=False,
                   num_devices=NCORES)

    xT_d = nc.dram_tensor("xT", [D, S], BF16, kind="ExternalInput").ap()
    # per 128-row block: [wq | wk | wv] column slices
    wqkv_d = nc.dram_tensor("wqkvT", [D, 3 * HPG * HD], BF16,
                            kind="ExternalInput").ap()
    woT_d = nc.dram_tensor("woT", [HPG * HD, D], BF16, kind="ExternalInput").ap()
    cs_d = nc.dram_tensor("cs128", [128, 2 * S], BF16, kind="ExternalInput").ap()
    # [pmat | ident | mdiagT]
    msk_d = nc.dram_tensor("msk", [128, 512], BF16, kind="ExternalInput").ap()
    out_d = nc.dram_tensor("out", [S, D], BF16, kind="ExternalOutput").ap()

    with tile.TileContext(nc) as tc:
        from contextlib import ExitStack

        with ExitStack() as root:
            pers = root.enter_context(tc.tile_pool(name="pers", bufs=1))

            # ---- persistent SBUF tiles ----
            qt = [pers.tile([128, S], BF16, tag=f"qt{p}", name=f"qt{p}")
                  for p in range(NPAIR)]
            kt = [pers.tile([128, S], BF16, tag=f"kt{p}", name=f"kt{p}")
                  for p in range(NPAIR)]
            # V' per (pair, s-block): [128,130] = V_A|ones|V_B|ones
            vp = [[pers.tile([128, 130], BF16, tag=f"vp{p}_{sb}",
                             name=f"vp{p}_{sb}")
                   for sb in range(NSB)] for p in range(NPAIR)]
            # attnT per pair: [vd(2 heads x 64), S]
            at = [pers.tile([128, S], BF16, tag=f"at{p}", name=f"at{p}")
                  for p in range(NPAIR)]
            wo2 = pers.tile([128, NPAIR * D], BF16, tag="wo2", name="wo2")
            wo_t = [wo2[:, p * D:(p + 1) * D] for p in range(NPAIR)]
            msk_t = pers.tile([128, 512], BF16, tag="msk", name="msk")
            pm_t = msk_t[:, 0:128]
            ident_t = msk_t[:, 128:256]
            tri_t = msk_t[:, 256:384]   # 0/1 lower-k triangle (k <= q)
            mdiag_t = msk_t[:, 384:512]  # additive -1e9 upper-k triangle
            ones1 = pers.tile([1, 64], BF16, tag="ones1", name="ones1")

            # all memsets first so the Pool engine is done before attention
            nc.gpsimd.memset(ones1[:], 1.0)
            for p in range(NPAIR):
                for sb in range(NSB):
                    nc.gpsimd.memset(vp[p][sb][:, 64:65], 1.0)
                    nc.gpsimd.memset(vp[p][sb][:, 129:130], 1.0)

            ld = root.enter_context(tc.tile_pool(name="ld", bufs=1))
            sbA = root.enter_context(tc.tile_pool(name="sbA", bufs=2))
            prb = root.enter_context(tc.tile_pool(name="prb", bufs=5))
            asb = root.enter_context(tc.tile_pool(name="asb", bufs=3))
            nrm = root.enter_context(tc.tile_pool(name="nrm", bufs=4))
            osb = root.enter_context(tc.tile_pool(name="osb", bufs=4))
            psS = root.enter_context(
                tc.tile_pool(name="psS", bufs=2, space="PSUM"))
            psPV = root.enter_context(
                tc.tile_pool(name="psPV", bufs=2, space="PSUM"))
            psX = root.enter_context(
                tc.tile_pool(name="psX", bufs=2, space="PSUM"))

            # PE warm-up: dummy matmuls during the otherwise-dead input-DMA
            # window release the HAM clock gate so the first real
            # projections run at full clock
            wt = psX.tile([64, 64], F32, tag="x", name="warm")
            for _ in range(180):
                nc.tensor.matmul(wt[:], ones1[:], ones1[:],
                                 start=True, stop=True)

            # ---- input DMAs (few, large; wqkv split so it=0 lands early)
            xts = [ld.tile([128, S], BF16, tag=f"xt{it}", name=f"xt{it}")
                   for it in range(NIT)]
            wqkv = ld.tile([128, NIT * 3 * HPG * HD], BF16, tag="wqkv",
                           name="wqkv")
            wq_t = [wqkv[:, it * 768:it * 768 + 256] for it in range(NIT)]
            wk_t = [wqkv[:, it * 768 + 256:it * 768 + 512] for it in range(NIT)]
            wv_t = [wqkv[:, it * 768 + 512:it * 768 + 768] for it in range(NIT)]
            cs_t = ld.tile([128, 2 * S], BF16, tag="cs128", name="cs128")
            c_t = cs_t[:, 0:S]
            s_t = cs_t[:, S:2 * S]

            for half in range(2):
                its = slice(half * 4 * 128, (half + 1) * 4 * 128)
                nc.sync.dma_start(
                    out=wqkv[:, half * 3072:(half + 1) * 3072].rearrange(
                        "p (i c) -> p i c", i=4),
                    in_=wqkv_d[its, :].rearrange("(i p) c -> p i c", p=128))
            for it in range(NIT):
                sl = slice(it * 128, (it + 1) * 128)
                nc.sync.dma_start(out=xts[it][:, 0:1024],
                                  in_=xT_d[sl, 0:1024])
            nc.sync.dma_start(out=msk_t[:], in_=msk_d[:])
            # cos/sin for chunks 0-1, then 2-3 (rope chunk 0 needs it early)
            nc.sync.dma_start(
                out=cs_t[:].rearrange("p (h c) -> p h c", h=2)[:, :, 0:1024],
                in_=cs_d[:].rearrange("p (h c) -> p h c", h=2)[:, :, 0:1024])
            nc.sync.dma_start(
                out=cs_t[:].rearrange("p (h c) -> p h c", h=2)[:, :, 1024:2048],
                in_=cs_d[:].rearrange("p (h c) -> p h c", h=2)[:, :, 1024:2048])
            for it in range(NIT):
                sl = slice(it * 128, (it + 1) * 128)
                nc.sync.dma_start(out=xts[it][:, 1024:2048],
                                  in_=xT_d[sl, 1024:2048])
            nc.sync.dma_start(
                out=wo2[:].rearrange("p (a d) -> p a d", a=NPAIR),
                in_=woT_d[:].rearrange("(a p) d -> p a d", a=NPAIR, p=128))

            # ---------------- helpers ----------------
            def v_finish(sb, ps):
                """psum [128, 256] (pair-packed V) -> vp tiles, both pairs."""
                for p in range(NPAIR):
                    src = ps[:, p * 128:(p + 1) * 128] \
                        .rearrange("p (b c) -> p b c", b=2, c=64)
                    dst = vp[p][sb][:, 0:130] \
                        .rearrange("p (b c) -> p b c", b=2, c=65)[:, :, 0:64]
                    nc.vector.tensor_copy(dst, src)

            def rope_copy(ps, on_act=True):
                a_sb = sbA.tile([128, 512], BF16, tag="a_sb", name="a_sb")
                if on_act:
                    nc.scalar.activation(a_sb[:], ps[:], AF.Copy)
                else:
                    nc.vector.tensor_copy(a_sb[:], ps[:])
                return a_sb

            def rope_finish(a_sb, dst, p, ch, sw_act=False):
                """rot = A*C + swap(A)*S into dst[p][:, chunk].

                sw_act: evacuate the swap psum on ScalarE so the DVE chain
                is 3 SBUF-only ops (shortest latency; used for the
                transition-critical head units)."""
                qs = slice(ch * 512, (ch + 1) * 512)
                t1 = sbA.tile([128, 512], BF16, tag="t1", name="t1")
                nc.vector.tensor_mul(t1[:], a_sb[:], c_t[:, qs])
                sw = psX.tile([128, 512], F32, tag="x", name="psSW")
                nc.tensor.matmul(sw[:], pm_t[:], a_sb[:],
                                 start=True, stop=True)
                t2 = sbA.tile([128, 512], BF16, tag="t2", name="t2")
                if sw_act:
                    sw_sb = sbA.tile([128, 512], BF16, tag="sw_sb",
                                     name="sw_sb")
                    nc.scalar.activation(sw_sb[:], sw[:], AF.Copy)
                    nc.vector.tensor_mul(t2[:], sw_sb[:], s_t[:, qs])
                else:
                    nc.vector.tensor_mul(t2[:], sw[:], s_t[:, qs])
                nc.vector.tensor_add(dst[p][:, qs], t1[:], t2[:])

            # ---------------- head block ----------------
            # it-major over x cols 0:1024: V s-blocks 0-3 + Q/K chunk 0 for
            # both pairs, so both pairs' chunk-0 attention unlocks first.
            hv = [psPV.tile([128, 512], F32, tag="pv", name=f"hv{b}")
                  for b in range(2)]
            hq = [psS.tile([128, 2 * QCH], F32, tag="sc", name=f"hq{p}")
                  for p in range(NPAIR)]
            for it in range(NIT):
                st, sp = it == 0, it == NIT - 1
                for sb in range(4):
                    b, o = divmod(sb, 2)
                    nc.tensor.matmul(hv[b][:, o * 256:(o + 1) * 256],
                                     xts[it][:, sb * 128:(sb + 1) * 128],
                                     wv_t[it][:],
                                     start=(st and o == 0),
                                     stop=(sp and o == 1))
                for p in range(NPAIR):
                    pc = slice(p * 128, (p + 1) * 128)
                    nc.tensor.matmul(hq[p][:, 0:512], wq_t[it][:, pc],
                                     xts[it][:, 0:512], start=st, stop=sp)
                    nc.tensor.matmul(hq[p][:, 512:1024], wk_t[it][:, pc],
                                     xts[it][:, 0:512], start=st, stop=sp)
            # rope pair 0's chunk-0 q/k now (unblocks the first items);
            # pair 1's chunk 0 becomes the first filler unit.  v_finish
            # after — the first PV only needs vp0 one item later.
            a_q0 = rope_copy(hq[0][:, 0:512])
            a_k0 = rope_copy(hq[0][:, 512:1024])
            rope_finish(a_q0, qt, 0, 0)
            rope_finish(a_k0, kt, 0, 0)
            for sb in range(4):
                b, o = divmod(sb, 2)
                v_finish(sb, hv[b][:, o * 256:(o + 1) * 256])

            # ---------------- filler machinery ----------------
            done_units = set()

            def g_v_unit(sb0):
                """V s-blocks (sb0, sb0+1): packed 2-per-bank accumulation."""
                t = psX.tile([128, 512], F32, tag="x", name=f"v{sb0}")
                for it in range(NIT):
                    st, sp = it == 0, it == NIT - 1
                    for o in range(2):
                        nc.tensor.matmul(
                            t[:, o * 256:(o + 1) * 256],
                            xts[it][:, (sb0 + o) * 128:(sb0 + o + 1) * 128],
                            wv_t[it][:],
                            start=(st and o == 0), stop=(sp and o == 1))
                        yield 107
                v_finish(sb0, t[:, 0:256])
                v_finish(sb0 + 1, t[:, 256:512])
                done_units.add(f"v{sb0}")
                yield 0

            rope_pend = [None]

            def g_p1_head_rope():
                """pair 1's chunk-0 rope (head produced the psum)."""
                a_q1 = rope_copy(hq[1][:, 0:512])
                a_k1 = rope_copy(hq[1][:, 512:1024])
                rope_finish(a_q1, qt, 1, 0)
                done_units.add("q10")
                yield 213
                rope_finish(a_k1, kt, 1, 0)
                done_units.add("k10")
                yield 213

            def g_qk_unit(p, w, ch):
                wt = wq_t if w == "q" else wk_t
                dst = qt if w == "q" else kt
                t = psX.tile([128, 512], F32, tag="x", name=f"qk{p}{w}{ch}")
                pc = slice(p * 128, (p + 1) * 128)
                for it in range(NIT):
                    nc.tensor.matmul(t[:], wt[it][:, pc],
                                     xts[it][:, ch * 512:(ch + 1) * 512],
                                     start=(it == 0), stop=(it == NIT - 1))
                    yield 213
                a_sb = rope_copy(t, on_act=(ch < 2))
                if rope_pend[0] is not None:
                    pa, pdst, pp, pch, pname = rope_pend[0]
                    rope_finish(pa, pdst, pp, pch)
                    done_units.add(pname)
                    yield 213
                rope_pend[0] = (a_sb, dst, p, ch, f"{w}{p}{ch}")

            def flush_pend():
                if rope_pend[0] is not None:
                    pa, pdst, pp, pch, pname = rope_pend[0]
                    rope_finish(pa, pdst, pp, pch)
                    done_units.add(pname)
                    rope_pend[0] = None

            def g_rope_flush():
                if rope_pend[0] is not None:
                    flush_pend()
                    yield 213

            ob_pend = {}

            def emit_wo_oc(sb, oc, tail=False):
                ssl = slice(sb * 128, (sb + 1) * 128)
                osl = slice(oc * 512, (oc + 1) * 512)
                ps = psX.tile([128, 512], F32, tag="x", name="psW")
                for p in range(NPAIR):
                    nc.tensor.matmul(
                        ps[:], at[p][:, ssl], wo_t[p][:, osl],
                        start=(p == 0), stop=(p == NPAIR - 1))
                if sb not in ob_pend:
                    ob_pend[sb] = osb.tile([128, 1024], BF16, tag="osb",
                                           name="osb")
                ob = ob_pend[sb]
                if tail and oc == 0:
                    # ScalarE is idle at the tail: evac halves in parallel
                    nc.scalar.activation(ob[:, osl], ps[:], AF.Copy)
                else:
                    nc.vector.tensor_copy(ob[:, osl], ps[:])
                if tail:
                    nc.sync.dma_start(out=out_d[ssl, osl], in_=ob[:, osl])
                    if oc == 1:
                        ob_pend.pop(sb)
                elif oc == 1:
                    nc.sync.dma_start(out=out_d[ssl, :],
                                      in_=ob_pend.pop(sb)[:])

            def g_wo_chunk(c):
                for sb in range(c * KPC, (c + 1) * KPC):
                    for oc in range(2):
                        emit_wo_oc(sb, oc)
                        yield 426

            # filler order: chunk-1 q/k (x cols 512:1024, already loaded)
            # before the xB-dependent V blocks; V blocks paced so vp(sb) is
            # ready ~when chunk sb//4's PV needs it.
            fillers = [
                g_p1_head_rope(),
                g_qk_unit(0, "q", 1), g_qk_unit(0, "k", 1),
                g_v_unit(4),
                g_qk_unit(1, "q", 1), g_qk_unit(1, "k", 1),
                g_v_unit(6), g_v_unit(8),
                g_qk_unit(0, "q", 2), g_qk_unit(0, "k", 2),
                g_v_unit(10),
                g_qk_unit(1, "q", 2), g_qk_unit(1, "k", 2),
                g_v_unit(12),
                g_qk_unit(0, "q", 3), g_qk_unit(0, "k", 3),
                g_v_unit(14),
                g_qk_unit(1, "q", 3), g_qk_unit(1, "k", 3),
                g_rope_flush(),
            ]
            fill_iq = [0]

            def pull(budget_ns):
                got = 0
                while fill_iq[0] < len(fillers):
                    g = fillers[fill_iq[0]]
                    try:
                        while got < budget_ns:
                            got += next(g)
                    except StopIteration:
                        fill_iq[0] += 1
                        continue
                    break
                return got

            def pull_until(units):
                while not units <= done_units:
                    # the last missing unit may be parked in the rope pend
                    if rope_pend[0] is not None and \
                            units <= (done_units | {rope_pend[0][4]}):
                        flush_pend()
                        return
                    if pull(600) == 0:
                        flush_pend()
                        assert units <= done_units, (
                            f"filler units {units - done_units} never emitted")
                        return

            # ---------------- attention ----------------
            def emit_scores(p, c, kb, sc, pe_mask=False):
                q0 = c * QCH
                k0 = kb * 128
                trim = max(q0, k0) if causal else q0
                t_off = trim - q0
                on_diag = causal and pe_mask and k0 >= q0
                for h in range(2):
                    hsl = slice(h * 64, (h + 1) * 64)
                    nc.tensor.matmul(
                        sc[:, h * QCH + t_off:(h + 1) * QCH],
                        kt[p][hsl, k0:k0 + 128],
                        qt[p][hsl, trim:q0 + QCH],
                        start=True, stop=not on_diag)
                if on_diag:
                    for h in range(2):
                        nc.tensor.matmul(
                            sc[:, h * QCH + t_off:h * QCH + t_off + 128],
                            ident_t[:], mdiag_t[:],
                            start=False, stop=True)

            def emit_exp(c, kb, sc, pt, pe_mask=False):
                """exp (trimmed); for diag items the upper-k triangle of the
                128-block is zeroed on the idle GPSIMD engine — except on
                the pe_mask path (tail), which folded -1e9 into the scores
                on the PE to keep the exp->PV latency minimal."""
                q0 = c * QCH
                t_off = (max(q0, kb * 128) - q0) if causal else 0
                if t_off == 0:
                    nc.scalar.activation(pt[:, 0:2 * QCH], sc[:, 0:2 * QCH],
                                         AF.Exp, scale=SM_SCALE)
                else:
                    sc3 = sc[:, 0:2 * QCH].rearrange(
                        "p (b c) -> p b c", b=2, c=QCH)[:, :, t_off:]
                    pt3 = pt[:, 0:2 * QCH].rearrange(
                        "p (b c) -> p b c", b=2, c=QCH)[:, :, t_off:]
                    nc.scalar.activation(pt3, sc3, AF.Exp, scale=SM_SCALE)
                if causal and not pe_mask and kb * 128 >= q0:
                    dg = pt[:, 0:2 * QCH].rearrange(
                        "p (b c) -> p b c", b=2,
                        c=QCH)[:, :, t_off:t_off + 128]
                    tri3 = tri_t.rearrange("p (b c) -> p b c", b=1)
                    nc.gpsimd.tensor_mul(dg, dg,
                                         tri3.broadcast_to([128, 2, 128]))

            def emit_pv(p, c, kb, kb_hi, pt, pvt):
                qs_lo = max(0, kb - 4 * c) if causal else 0
                for qs in range(qs_lo, 4):
                    bank, qsl = divmod(qs, 2)
                    last_kb = (4 * c + bank * 2 + 1) if causal else kb_hi - 1
                    for h in range(2):
                        col = qsl * 130 + h * 65
                        nc.tensor.matmul(
                            pvt[bank][:, col:col + 65],
                            pt[:, h * QCH + qs * 128:h * QCH + qs * 128 + 128],
                            vp[p][kb][:, h * 65:h * 65 + 65],
                            start=(kb == 0 and qsl == 0 and h == 0),
                            stop=(kb == last_kb and qsl == 1 and h == 1))

            def emit_norm(j, pvt, attn_sc, h1_act=False):
                """normalize q-slice j of the pair-chunk into attn_sc."""
                bank, qsl = divmod(j, 2)
                rr = nrm.tile([128, 2], F32, tag="rr", name="rr")
                den = pvt[bank][:, qsl * 130:qsl * 130 + 130].rearrange(
                    "p (h c) -> p h c", h=2)[:, :, 64:65]
                nc.vector.reciprocal_approx_fast(
                    rr[:].rearrange("p (h c) -> p h c", c=1), den)
                for h in range(2):
                    dst = attn_sc[:, j * 128 + h * 64:j * 128 + (h + 1) * 64]
                    src = pvt[bank][:,
                                    qsl * 130 + h * 65:qsl * 130 + h * 65 + 64]
                    if h == 1 and h1_act:
                        # tail: h1 on the (by now idle) ScalarE so the two
                        # head halves normalize in parallel
                        nc.scalar.activation(dst, src, AF.Copy,
                                             scale=rr[:, 1:2])
                    else:
                        nc.vector.tensor_scalar_mul(dst, src, rr[:, h:h + 1])

            # chunk processing order 1, 2, 3, 0: the final Act (exp) stretch
            # is the 4-item chunk 0, so the exp stream drains early and the
            # close is PE-dense.  The last two pair-groups run in "tail
            # mode": eager per-q-slice PE transposes + eager wo + immediate
            # ship instead of the DMA-transpose + deferred-wo pipeline.
            pair_jobs = [(0, 0), (0, 1), (1, 0), (1, 1), (2, 0), (2, 1),
                         (3, 0), (3, 1)]
            budgets = (350, 350, 350, 350, 350, 350, 460, 460)
            dmatp_pend = [None]
            # group gate: only this chunk's q rope; k ropes and V blocks
            # gate per-item below so their filler units can slide into this
            # group's item stream
            need_map = {}
            for c, p in pair_jobs:
                req = set()
                if not (p == 0 and c == 0):
                    req.add(f"q{p}{c}")
                need_map[(c, p)] = req

            for j, (c, p) in enumerate(pair_jobs):
                kb_hi = (c * KPC + KPC) if causal else NSB
                is_final = j == len(pair_jobs) - 1
                tail_mode = is_final
                pull_until(need_map[(c, p)])

                pvt = [psPV.tile([128, 512], F32, tag="pv", name=f"pv{b}")
                       for b in range(2)]
                if tail_mode:
                    attn_sc = asb.tile([128, 512], BF16, tag="af",
                                       name="attn_sf")
                else:
                    attn_sc = asb.tile([128, 512], BF16, tag="asb",
                                       name="attn_sc")

                pend_q = []     # (kb, pt) awaiting PV
                norm_q = []     # q-slices whose PV is emitted, norm pending

                def flush_norms(p=p, c=c, pvt=pvt, attn_sc=attn_sc,
                                tail_mode=tail_mode, is_final=is_final,
                                norm_q=norm_q):
                    for jq in norm_q:
                        emit_norm(jq, pvt, attn_sc, h1_act=is_final)
                        if tail_mode:
                            # PE transpose + evac + eager wo + ship
                            tp = psX.tile([128, 512], F32, tag="x",
                                          name="tp")
                            tpb = tp.bitcast(BF16)
                            nc.tensor.transpose(
                                tpb[:, 0:128],
                                attn_sc[:, jq * 128:(jq + 1) * 128],
                                ident_t[:])
                            qg = c * KPC + jq
                            nc.vector.tensor_copy(
                                at[p][:, qg * 128:(qg + 1) * 128],
                                tpb[:, 0:128])
                            emit_wo_oc(qg, 0, tail=True)
                            emit_wo_oc(qg, 1, tail=True)
                    del norm_q[:]

                def drain_one(p=p, c=c, kb_hi=kb_hi, pvt=pvt,
                              pend_q=pend_q, norm_q=norm_q,
                              flush_norms=flush_norms):
                    pkb, ppt = pend_q.pop(0)
                    if pkb >= 4:
                        vname = f"v{pkb & ~1}"
                        if vname not in done_units:
                            pull_until({vname})
                    emit_pv(p, c, pkb, kb_hi, ppt, pvt)
                    if causal and pkb >= 4 * c:
                        norm_q.append(pkb - 4 * c)
                    flush_norms()

                budget = budgets[j]
                for kb in range(kb_hi):
                    kc = kb // 4
                    if kc >= 1 or p == 1:
                        kname = f"k{p}{kc}"
                        if kname not in done_units:
                            pull_until({kname})
                    sc = psS.tile([128, 2 * QCH], F32, tag="sc", name="sc")
                    pt = prb.tile([128, 2 * QCH], BF16, tag="prb", name="prb")
                    emit_scores(p, c, kb, sc, pe_mask=is_final)
                    emit_exp(c, kb, sc, pt, pe_mask=is_final)
                    if kb == 2 and dmatp_pend[0] is not None:
                        # previous pair-chunk's attn transpose: deferred here
                        # so its sem wait is satisfied on arrival and doesn't
                        # head-of-line block the SP DMA queue
                        dmatp_pend[0]()
                        dmatp_pend[0] = None
                    pull(budget)
                    # drain pending PVs; a diag item's PV is held one extra
                    # item so the GPSIMD triangle-mask round trip is hidden
                    while pend_q:
                        diag0 = (causal and not is_final
                                 and pend_q[0][0] >= 4 * c)
                        if len(pend_q) == 1 and diag0:
                            break
                        drain_one()
                    pend_q.append((kb, pt))
                # drain remaining PVs + norms
                while pend_q:
                    pull(300)
                    drain_one()
                if not causal:
                    norm_q.extend(range(4))
                    flush_norms()

                if not tail_mode:
                    # blocked transpose of the whole pair-chunk into at[p];
                    # emission deferred into the next pair-group
                    def mk_tp(p=p, c=c, attn_sc=attn_sc):
                        def emit():
                            nc.sync.dma_start_transpose(
                                at[p][:, c * QCH:(c + 1) * QCH].rearrange(
                                    "v (b q) -> v b q", b=4),
                                attn_sc[:])
                        return emit

                    dmatp_pend[0] = mk_tp()
                    if p == NPAIR - 1:
                        fillers.append(g_wo_chunk(c))

            # leftover fillers (late wo chunks)
            pull(10**12)

    nc.compile()
    return nc


def _host_prep(x, freqs_cos, freqs_sin, wq, wk, wv, wo):
    """Build the 8 per-core input maps (numpy, bf16)."""
    import ml_dtypes

    bf16 = ml_dtypes.bfloat16

    x = np.ascontiguousarray(x, dtype=np.float32)
    cosT = np.ascontiguousarray(freqs_cos.T, dtype=np.float32)  # [32, S]
    sinT = np.ascontiguousarray(freqs_sin.T, dtype=np.float32)

    c128 = np.tile(cosT, (4, 1))                                # [128, S]
    s128 = np.tile(np.concatenate([-sinT, sinT], 0), (2, 1))
    cs128 = np.ascontiguousarray(
        np.concatenate([c128, s128], axis=1)).astype(bf16)      # [128, 2S]
    # swap permutation: psum_sw = pmat.T @ A -> sw[m] = A[sigma(m)],
    # sigma swaps the 32-halves within each 64 block.
    pmat = np.zeros((128, 128), dtype=np.float32)
    for m in range(128):
        blk, off = divmod(m, 32)
        pmat[(blk ^ 1) * 32 + off, m] = 1.0
    ident = np.eye(128, dtype=np.float32)
    # causal diag 0/1 triangle: tri01[k, q] = 1 if k <= q else 0
    kk, qq = np.meshgrid(np.arange(128), np.arange(128), indexing="ij")
    tri01 = (kk <= qq).astype(np.float32)
    mdiagT = np.where(kk <= qq, 0.0, NEG_INF).astype(np.float32)
    msk = np.ascontiguousarray(
        np.concatenate([pmat, ident, tri01, mdiagT], axis=1)).astype(bf16)

    # rotate-half row permutation within each head
    rh = np.concatenate([np.arange(0, HD, 2), np.arange(1, HD, 2)])

    xT = [np.ascontiguousarray(x[b].T).astype(bf16) for b in range(B)]

    in_maps = []
    for core in range(NCORES):
        b, g = divmod(core, GROUPS)
        heads = [g * HPG + j for j in range(HPG)]
        qrows, vrows = [], []
        for h in heads:
            base = h * HD
            qrows.extend((base + rh).tolist())
            vrows.extend(range(base, base + HD))
        qrows = np.array(qrows)
        vrows = np.array(vrows)
        wqT = wq[qrows, :].T                                     # [D, 256]
        wkT = wk[qrows, :].T
        wvT = wv[vrows, :].T
        wqkvT = np.ascontiguousarray(
            np.concatenate([wqT, wkT, wvT], axis=1)).astype(bf16)
        woT = np.ascontiguousarray(wo[:, vrows].T).astype(bf16)  # [256, D]
        in_maps.append({
            "xT": xT[b], "wqkvT": wqkvT, "woT": woT,
            "cs128": cs128, "msk": msk,
        })
    return in_maps


def _mask_kind(mask):
    m = np.asarray(mask).reshape(S, S)
    if not np.any(m):
        return "zeros"
    qq, kk = np.meshgrid(np.arange(S), np.arange(S), indexing="ij")
    causal = np.where(kk <= qq, 0.0, NEG_INF).astype(np.float32)  # [q, k]
    if np.array_equal(m, causal):
        return "causal"
    return "general"


def _reference_host(x, freqs_cos, freqs_sin, mask, wq, wk, wv, wo):
    """Correctness fallback for arbitrary masks (host numpy, float64)."""
    b, s, d = x.shape
    hd = d // H
    xq = (x @ wq.T).reshape(b, s, H, hd)
    xk = (x @ wk.T).reshape(b, s, H, hd)
    xv = (x @ wv.T).reshape(b, s, H, hd)

    def rope(t):
        tr = t.reshape(b, s, H, hd // 2, 2)
        t0, t1 = tr[..., 0], tr[..., 1]
        cos = freqs_cos[None, :, None, :]
        sin = freqs_sin[None, :, None, :]
        return np.stack([t0 * cos - t1 * sin, t0 * sin + t1 * cos],
                        -1).reshape(b, s, H, hd)

    xq, xk = rope(xq), rope(xk)
    sc = np.einsum("bqhd,bkhd->bhqk", xq, xk) / np.sqrt(hd) + mask
    sc = sc - sc.max(-1, keepdims=True)
    e = np.exp(sc)
    pr = e / e.sum(-1, keepdims=True)
    o = np.einsum("bhqk,bkhd->bqhd", pr, xv).reshape(b, s, d)
    return (o @ wo.T).astype(np.float32)


def kernel(x, freqs_cos, freqs_sin, mask, wq, wk, wv, wo):
    kind = _mask_kind(mask)
    if kind == "general":
        return _reference_host(np.asarray(x, np.float64),
                               np.asarray(freqs_cos, np.float64),
                               np.asarray(freqs_sin, np.float64),
                               np.asarray(mask, np.float64),
                               np.asarray(wq, np.float64),
                               np.asarray(wk, np.float64),
                               np.asarray(wv, np.float64),
                               np.asarray(wo, np.float64))

    if kind not in _PROG_CACHE:
        _PROG_CACHE[kind] = _build_program(kind)
    nc = _PROG_CACHE[kind]

    in_maps = _host_prep(np.asarray(x, np.float32),
                         np.asarray(freqs_cos, np.float32),
                         np.asarray(freqs_sin, np.float32),
                         np.asarray(wq, np.float32),
                         np.asarray(wk, np.float32),
                         np.asarray(wv, np.float32),
                         np.asarray(wo, np.float32))
    res = run_bass_kernel_spmd(nc, in_maps, list(range(NCORES)))
    out = np.zeros((B, S, D), dtype=np.float32)
    for core in range(NCORES):
        out[core // GROUPS] += np.asarray(res.results[core]["out"],
                                          dtype=np.float32)
    return out


# revision 52
# speedup vs baseline: 1.2275x; 1.0430x over previous
"""Trainium2 Bass kernel for nn_Attention_12515534700827.

Multi-head causal attention with RoPE: B=2, S=2048, D=1024, H=16, HD=64.
Sharding: 8 cores = 2 (batch) x 4 (head groups of 4 heads). Each core
computes its 4 heads' attention + its slice of the wo projection; the host
sums the 4 partial outputs per batch (the "all-reduce after wo").

v3 (fused single-stream): projections, attention, and the wo projection are
emitted as ONE interleaved instruction stream so the ScalarE exp stream (the
second-largest engine load) overlaps the projection/wo matmuls instead of
running in its own phase.

Key differences vs v2:
  - PV computed with probs as the STATIONARY operand and V' ([V|ones]) as
    the MOVING operand: out[q, vd|den] per (head, q-slice).  The moving free
    dim drops from ~512 to 65, halving PV cost; the softmax denominator
    arrives as psum column 64 per head so normalization becomes a
    per-partition scalar multiply (no PE broadcast, no reciprocal of a
    [64,1024] tile, no partition-shift DMA bounce).
  - The resulting attn tiles are [q, vd]; wo needs [vd, q].  Transposed via
    dma_start_transpose (XBAR 16x128 tiles, cheap on the DMA engines)
    straight into the persistent at[] tiles; the last pair uses PE
    transposes so the tail isn't gated on a DMA round trip.
  - Fused emission: after the head block (V sb0-3 + Q/K chunk0 for both
    pairs over x cols 0:1024), attention items start immediately; the
    remaining V blocks, Q/K chunks, rope chains and deferred wo tiles are
    "fillers" pulled between items to keep the PE dense while ScalarE
    streams the exps.
  - wo(c) is deferred ~2 chunks so it lands as filler in the late,
    otherwise Act-bound stretch.
  - Inputs land in few large DMAs (HWDGE descriptor time ~0.6us each).

PSUM budget (8 banks): scores 2x[128,1024] (4) + PV 2x[128,512] (2) +
misc single-shot rotation psX 2x[128,512] (2: rope swaps, V pairs, wo,
tail transposes).  PV packs 2 q-slices x 2 heads x 65 cols per bank with
one accumulation-group start/stop per bank (hardware clears has_written at
bank granularity).
"""

import sys

if "/opt/trn_rl_repo" not in sys.path:
    sys.path.insert(0, "/opt/trn_rl_repo")

import numpy as np

import concourse.mybir as mybir
import concourse.tile as tile
from concourse import bacc
from concourse.bass_utils import run_bass_kernel_spmd

F32 = mybir.dt.float32
BF16 = mybir.dt.bfloat16
AF = mybir.ActivationFunctionType

B, S, D, H, HD = 2, 2048, 1024, 16, 64
NCORES = 8
GROUPS = 4            # head groups (cores per batch)
HPG = H // GROUPS     # heads per core = 4
NPAIR = HPG // 2      # head pairs per core = 2
NEG_INF = -1e9
SM_SCALE = 1.0 / float(np.sqrt(HD))  # 0.125

NIT = D // 128        # 8 contraction tiles
NSB = S // 128        # 16 seq blocks
QCH = 512             # attention q-chunk
NCHUNK = S // QCH     # 4
KPC = QCH // 128      # k/q 128-blocks per chunk = 4

_PROG_CACHE = {}


def _build_program(mask_kind: str):
    """mask_kind: 'causal' (trimmed + diag mask) or 'zeros' (full)."""
    causal = mask_kind == "causal"
    nc = bacc.Bacc("TRN2", target_bir_lowering=False, debug=False,
                   num_devices=NCORES)

    xT_d = nc.dram_tensor("xT", [D, S], BF16, kind="ExternalInput").ap()
    # per 128-row block: [wq | wk | wv] column slices
    wqkv_d = nc.dram_tensor("wqkvT", [D, 3 * HPG * HD], BF16,
                            kind="ExternalInput").ap()
    woT_d = nc.dram_tensor("woT", [HPG * HD, D], BF16, kind="ExternalInput").ap()
    cs_d = nc.dram_tensor("cs128", [128, 2 * S], BF16, kind="ExternalInput").ap()
    # [pmat | ident | mdiagT]
    msk_d = nc.dram_tensor("msk", [128, 512], BF16, kind="ExternalInput").ap()
    out_d = nc.dram_tensor("out", [S, D], BF16, kind="ExternalOutput").ap()

    with tile.TileContext(nc) as tc:
        from contextlib import ExitStack

        with ExitStack() as root:
            pers = root.enter_context(tc.tile_pool(name="pers", bufs=1))

            # ---- persistent SBUF tiles ----
            qt = [pers.tile([128, S], BF16, tag=f"qt{p}", name=f"qt{p}")
                  for p in range(NPAIR)]
            kt = [pers.tile([128, S], BF16, tag=f"kt{p}", name=f"kt{p}")
                  for p in range(NPAIR)]
            # V' per (pair, s-block): [128,130] = V_A|ones|V_B|ones
            vp = [[pers.tile([128, 130], BF16, tag=f"vp{p}_{sb}",
                             name=f"vp{p}_{sb}")
                   for sb in range(NSB)] for p in range(NPAIR)]
            # attnT per pair: [vd(2 heads x 64), S]
            at = [pers.tile([128, S], BF16, tag=f"at{p}", name=f"at{p}")
                  for p in range(NPAIR)]
            wo2 = pers.tile([128, NPAIR * D], BF16, tag="wo2", name="wo2")
            wo_t = [wo2[:, p * D:(p + 1) * D] for p in range(NPAIR)]
            msk_t = pers.tile([128, 512], BF16, tag="msk", name="msk")
            pm_t = msk_t[:, 0:128]
            ident_t = msk_t[:, 128:256]
            tri_t = msk_t[:, 256:384]   # 0/1 lower-k triangle (k <= q)
            mdiag_t = msk_t[:, 384:512]  # additive -1e9 upper-k triangle
            ones1 = pers.tile([1, 64], BF16, tag="ones1", name="ones1")

            # all memsets first so the Pool engine is done before attention
            nc.gpsimd.memset(ones1[:], 1.0)
            for p in range(NPAIR):
                for sb in range(NSB):
                    nc.gpsimd.memset(vp[p][sb][:, 64:65], 1.0)
                    nc.gpsimd.memset(vp[p][sb][:, 129:130], 1.0)

            ld = root.enter_context(tc.tile_pool(name="ld", bufs=1))
            sbA = root.enter_context(tc.tile_pool(name="sbA", bufs=2))
            prb = root.enter_context(tc.tile_pool(name="prb", bufs=5))
            asb = root.enter_context(tc.tile_pool(name="asb", bufs=3))
            nrm = root.enter_context(tc.tile_pool(name="nrm", bufs=4))
            osb = root.enter_context(tc.tile_pool(name="osb", bufs=4))
            psS = root.enter_context(
                tc.tile_pool(name="psS", bufs=2, space="PSUM"))
            psPV = root.enter_context(
                tc.tile_pool(name="psPV", bufs=2, space="PSUM"))
            psX = root.enter_context(
                tc.tile_pool(name="psX", bufs=2, space="PSUM"))

            # PE warm-up: dummy matmuls during the otherwise-dead input-DMA
            # window release the HAM clock gate so the first real
            # projections run at full clock
            wt = psX.tile([64, 64], F32, tag="x", name="warm")
            import os as _os
            _wu = int(_os.environ.get("K_WARMUP", "180"))
            for _ in range(_wu):
                nc.tensor.matmul(wt[:], ones1[:], ones1[:],
                                 start=True, stop=True)

            # ---- input DMAs (few, large; wqkv split so it=0 lands early)
            xts = [ld.tile([128, S], BF16, tag=f"xt{it}", name=f"xt{it}")
                   for it in range(NIT)]
            wqkv = ld.tile([128, NIT * 3 * HPG * HD], BF16, tag="wqkv",
                           name="wqkv")
            wq_t = [wqkv[:, it * 768:it * 768 + 256] for it in range(NIT)]
            wk_t = [wqkv[:, it * 768 + 256:it * 768 + 512] for it in range(NIT)]
            wv_t = [wqkv[:, it * 768 + 512:it * 768 + 768] for it in range(NIT)]
            cs_t = ld.tile([128, 2 * S], BF16, tag="cs128", name="cs128")
            c_t = cs_t[:, 0:S]
            s_t = cs_t[:, S:2 * S]

            for half in range(2):
                its = slice(half * 4 * 128, (half + 1) * 4 * 128)
                nc.sync.dma_start(
                    out=wqkv[:, half * 3072:(half + 1) * 3072].rearrange(
                        "p (i c) -> p i c", i=4),
                    in_=wqkv_d[its, :].rearrange("(i p) c -> p i c", p=128))
            for it in range(NIT):
                sl = slice(it * 128, (it + 1) * 128)
                nc.sync.dma_start(out=xts[it][:, 0:1024],
                                  in_=xT_d[sl, 0:1024])
            nc.sync.dma_start(out=msk_t[:], in_=msk_d[:])
            # cos/sin for chunks 0-1, then 2-3 (rope chunk 0 needs it early)
            nc.sync.dma_start(
                out=cs_t[:].rearrange("p (h c) -> p h c", h=2)[:, :, 0:1024],
                in_=cs_d[:].rearrange("p (h c) -> p h c", h=2)[:, :, 0:1024])
            nc.sync.dma_start(
                out=cs_t[:].rearrange("p (h c) -> p h c", h=2)[:, :, 1024:2048],
                in_=cs_d[:].rearrange("p (h c) -> p h c", h=2)[:, :, 1024:2048])
            for it in range(NIT):
                sl = slice(it * 128, (it + 1) * 128)
                nc.sync.dma_start(out=xts[it][:, 1024:2048],
                                  in_=xT_d[sl, 1024:2048])
            nc.sync.dma_start(
                out=wo2[:].rearrange("p (a d) -> p a d", a=NPAIR),
                in_=woT_d[:].rearrange("(a p) d -> p a d", a=NPAIR, p=128))

            # ---------------- helpers ----------------
            def v_finish(sb, ps):
                """psum [128, 256] (pair-packed V) -> vp tiles, both pairs."""
                for p in range(NPAIR):
                    src = ps[:, p * 128:(p + 1) * 128] \
                        .rearrange("p (b c) -> p b c", b=2, c=64)
                    dst = vp[p][sb][:, 0:130] \
                        .rearrange("p (b c) -> p b c", b=2, c=65)[:, :, 0:64]
                    nc.vector.tensor_copy(dst, src)

            def rope_copy(ps, on_act=True):
                a_sb = sbA.tile([128, 512], BF16, tag="a_sb", name="a_sb")
                if on_act:
                    nc.scalar.activation(a_sb[:], ps[:], AF.Copy)
                else:
                    nc.vector.tensor_copy(a_sb[:], ps[:])
                return a_sb

            def rope_finish(a_sb, dst, p, ch, sw_act=False):
                """rot = A*C + swap(A)*S into dst[p][:, chunk].

                sw_act: evacuate the swap psum on ScalarE so the DVE chain
                is 3 SBUF-only ops (shortest latency; used for the
                transition-critical head units)."""
                qs = slice(ch * 512, (ch + 1) * 512)
                t1 = sbA.tile([128, 512], BF16, tag="t1", name="t1")
                nc.vector.tensor_mul(t1[:], a_sb[:], c_t[:, qs])
                sw = psX.tile([128, 512], F32, tag="x", name="psSW")
                nc.tensor.matmul(sw[:], pm_t[:], a_sb[:],
                                 start=True, stop=True)
                t2 = sbA.tile([128, 512], BF16, tag="t2", name="t2")
                if sw_act:
                    sw_sb = sbA.tile([128, 512], BF16, tag="sw_sb",
                                     name="sw_sb")
                    nc.scalar.activation(sw_sb[:], sw[:], AF.Copy)
                    nc.vector.tensor_mul(t2[:], sw_sb[:], s_t[:, qs])
                else:
                    nc.vector.tensor_mul(t2[:], sw[:], s_t[:, qs])
                nc.vector.tensor_add(dst[p][:, qs], t1[:], t2[:])

            # ---------------- head block ----------------
            # it-major over x cols 0:1024: V s-blocks 0-3 + Q/K chunk 0 for
            # both pairs, so both pairs' chunk-0 attention unlocks first.
            hv = [psPV.tile([128, 512], F32, tag="pv", name=f"hv{b}")
                  for b in range(2)]
            hq = [psS.tile([128, 2 * QCH], F32, tag="sc", name=f"hq{p}")
                  for p in range(NPAIR)]
            for it in range(NIT):
                st, sp = it == 0, it == NIT - 1
                for sb in range(4):
                    b, o = divmod(sb, 2)
                    nc.tensor.matmul(hv[b][:, o * 256:(o + 1) * 256],
                                     xts[it][:, sb * 128:(sb + 1) * 128],
                                     wv_t[it][:],
                                     start=(st and o == 0),
                                     stop=(sp and o == 1))
                for p in range(NPAIR):
                    pc = slice(p * 128, (p + 1) * 128)
                    nc.tensor.matmul(hq[p][:, 0:512], wq_t[it][:, pc],
                                     xts[it][:, 0:512], start=st, stop=sp)
                    nc.tensor.matmul(hq[p][:, 512:1024], wk_t[it][:, pc],
                                     xts[it][:, 0:512], start=st, stop=sp)
            # rope pair 0's chunk-0 q/k now (unblocks the first items);
            # pair 1's chunk 0 becomes the first filler unit.  v_finish
            # after — the first PV only needs vp0 one item later.
            a_q0 = rope_copy(hq[0][:, 0:512])
            a_k0 = rope_copy(hq[0][:, 512:1024])
            rope_finish(a_q0, qt, 0, 0)
            rope_finish(a_k0, kt, 0, 0)
            for sb in range(4):
                b, o = divmod(sb, 2)
                v_finish(sb, hv[b][:, o * 256:(o + 1) * 256])

            # ---------------- filler machinery ----------------
            done_units = set()

            def g_v_unit(sb0):
                """V s-blocks (sb0, sb0+1): packed 2-per-bank accumulation."""
                t = psX.tile([128, 512], F32, tag="x", name=f"v{sb0}")
                for it in range(NIT):
                    st, sp = it == 0, it == NIT - 1
                    for o in range(2):
                        nc.tensor.matmul(
                            t[:, o * 256:(o + 1) * 256],
                            xts[it][:, (sb0 + o) * 128:(sb0 + o + 1) * 128],
                            wv_t[it][:],
                            start=(st and o == 0), stop=(sp and o == 1))
                        yield 107
                v_finish(sb0, t[:, 0:256])
                v_finish(sb0 + 1, t[:, 256:512])
                done_units.add(f"v{sb0}")
                yield 0

            rope_pend = [None]

            def g_p1_head_rope():
                """pair 1's chunk-0 rope (head produced the psum)."""
                a_q1 = rope_copy(hq[1][:, 0:512])
                a_k1 = rope_copy(hq[1][:, 512:1024])
                rope_finish(a_q1, qt, 1, 0)
                done_units.add("q10")
                yield 213
                rope_finish(a_k1, kt, 1, 0)
                done_units.add("k10")
                yield 213

            def g_qk_unit(p, w, ch):
                wt = wq_t if w == "q" else wk_t
                dst = qt if w == "q" else kt
                t = psX.tile([128, 512], F32, tag="x", name=f"qk{p}{w}{ch}")
                pc = slice(p * 128, (p + 1) * 128)
                for it in range(NIT):
                    nc.tensor.matmul(t[:], wt[it][:, pc],
                                     xts[it][:, ch * 512:(ch + 1) * 512],
                                     start=(it == 0), stop=(it == NIT - 1))
                    yield 213
                a_sb = rope_copy(t, on_act=(ch < 2))
                if rope_pend[0] is not None:
                    pa, pdst, pp, pch, pname = rope_pend[0]
                    rope_finish(pa, pdst, pp, pch)
                    done_units.add(pname)
                    yield 213
                rope_pend[0] = (a_sb, dst, p, ch, f"{w}{p}{ch}")

            def flush_pend():
                if rope_pend[0] is not None:
                    pa, pdst, pp, pch, pname = rope_pend[0]
                    rope_finish(pa, pdst, pp, pch)
                    done_units.add(pname)
                    rope_pend[0] = None

            def g_rope_flush():
                if rope_pend[0] is not None:
                    flush_pend()
                    yield 213

            ob_pend = {}

            def emit_wo_oc(sb, oc, tail=False):
                ssl = slice(sb * 128, (sb + 1) * 128)
                osl = slice(oc * 512, (oc + 1) * 512)
                ps = psX.tile([128, 512], F32, tag="x", name="psW")
                for p in range(NPAIR):
                    nc.tensor.matmul(
                        ps[:], at[p][:, ssl], wo_t[p][:, osl],
                        start=(p == 0), stop=(p == NPAIR - 1))
                if sb not in ob_pend:
                    ob_pend[sb] = osb.tile([128, 1024], BF16, tag="osb",
                                           name="osb")
                ob = ob_pend[sb]
                if tail and oc == 0:
                    # ScalarE is idle at the tail: evac halves in parallel
                    nc.scalar.activation(ob[:, osl], ps[:], AF.Copy)
                else:
                    nc.vector.tensor_copy(ob[:, osl], ps[:])
                if oc == 1:
                    nc.sync.dma_start(out=out_d[ssl, :],
                                      in_=ob_pend.pop(sb)[:])

            def g_wo_chunk(c):
                for sb in range(c * KPC, (c + 1) * KPC):
                    for oc in range(2):
                        emit_wo_oc(sb, oc)
                        yield 426

            # filler order: chunk-1 q/k (x cols 512:1024, already loaded)
            # before the xB-dependent V blocks; V blocks paced so vp(sb) is
            # ready ~when chunk sb//4's PV needs it.
            fillers = [
                g_p1_head_rope(),
                g_qk_unit(0, "q", 1), g_qk_unit(0, "k", 1),
                g_v_unit(4),
                g_qk_unit(1, "q", 1), g_qk_unit(1, "k", 1),
                g_v_unit(6), g_v_unit(8),
                g_qk_unit(0, "q", 2), g_qk_unit(0, "k", 2),
                g_v_unit(10),
                g_qk_unit(1, "q", 2), g_qk_unit(1, "k", 2),
                g_v_unit(12),
                g_qk_unit(0, "q", 3), g_qk_unit(0, "k", 3),
                g_v_unit(14),
                g_qk_unit(1, "q", 3), g_qk_unit(1, "k", 3),
                g_rope_flush(),
            ]
            fill_iq = [0]

            def pull(budget_ns):
                got = 0
                while fill_iq[0] < len(fillers):
                    g = fillers[fill_iq[0]]
                    try:
                        while got < budget_ns:
                            got += next(g)
                    except StopIteration:
                        fill_iq[0] += 1
                        continue
                    break
                return got

            def pull_until(units):
                while not units <= done_units:
                    # the last missing unit may be parked in the rope pend
                    if rope_pend[0] is not None and \
                            units <= (done_units | {rope_pend[0][4]}):
                        flush_pend()
                        return
                    if pull(600) == 0:
                        flush_pend()
                        assert units <= done_units, (
                            f"filler units {units - done_units} never emitted")
                        return

            # ---------------- attention ----------------
            def emit_scores(p, c, kb, sc, pe_mask=False):
                q0 = c * QCH
                k0 = kb * 128
                trim = max(q0, k0) if causal else q0
                t_off = trim - q0
                on_diag = causal and pe_mask and k0 >= q0
                for h in range(2):
                    hsl = slice(h * 64, (h + 1) * 64)
                    nc.tensor.matmul(
                        sc[:, h * QCH + t_off:(h + 1) * QCH],
                        kt[p][hsl, k0:k0 + 128],
                        qt[p][hsl, trim:q0 + QCH],
                        start=True, stop=not on_diag)
                if on_diag:
                    for h in range(2):
                        nc.tensor.matmul(
                            sc[:, h * QCH + t_off:h * QCH + t_off + 128],
                            ident_t[:], mdiag_t[:],
                            start=False, stop=True)

            def emit_exp(c, kb, sc, pt, pe_mask=False):
                """exp (trimmed); for diag items the upper-k triangle of the
                128-block is zeroed on the idle GPSIMD engine — except on
                the pe_mask path (tail), which folded -1e9 into the scores
                on the PE to keep the exp->PV latency minimal."""
                q0 = c * QCH
                t_off = (max(q0, kb * 128) - q0) if causal else 0
                if t_off == 0:
                    nc.scalar.activation(pt[:, 0:2 * QCH], sc[:, 0:2 * QCH],
                                         AF.Exp, scale=SM_SCALE)
                else:
                    sc3 = sc[:, 0:2 * QCH].rearrange(
                        "p (b c) -> p b c", b=2, c=QCH)[:, :, t_off:]
                    pt3 = pt[:, 0:2 * QCH].rearrange(
                        "p (b c) -> p b c", b=2, c=QCH)[:, :, t_off:]
                    nc.scalar.activation(pt3, sc3, AF.Exp, scale=SM_SCALE)
                if causal and not pe_mask and kb * 128 >= q0:
                    dg = pt[:, 0:2 * QCH].rearrange(
                        "p (b c) -> p b c", b=2,
                        c=QCH)[:, :, t_off:t_off + 128]
                    tri3 = tri_t.rearrange("p (b c) -> p b c", b=1)
                    nc.gpsimd.tensor_mul(dg, dg,
                                         tri3.broadcast_to([128, 2, 128]))

            def emit_pv(p, c, kb, kb_hi, pt, pvt):
                qs_lo = max(0, kb - 4 * c) if causal else 0
                for qs in range(qs_lo, 4):
                    bank, qsl = divmod(qs, 2)
                    last_kb = (4 * c + bank * 2 + 1) if causal else kb_hi - 1
                    for h in range(2):
                        col = qsl * 130 + h * 65
                        nc.tensor.matmul(
                            pvt[bank][:, col:col + 65],
                            pt[:, h * QCH + qs * 128:h * QCH + qs * 128 + 128],
                            vp[p][kb][:, h * 65:h * 65 + 65],
                            start=(kb == 0 and qsl == 0 and h == 0),
                            stop=(kb == last_kb and qsl == 1 and h == 1))

            def emit_norm(j, pvt, attn_sc, h1_act=False):
                """normalize q-slice j of the pair-chunk into attn_sc."""
                bank, qsl = divmod(j, 2)
                rr = nrm.tile([128, 2], F32, tag="rr", name="rr")
                den = pvt[bank][:, qsl * 130:qsl * 130 + 130].rearrange(
                    "p (h c) -> p h c", h=2)[:, :, 64:65]
                nc.vector.reciprocal_approx_fast(
                    rr[:].rearrange("p (h c) -> p h c", c=1), den)
                for h in range(2):
                    dst = attn_sc[:, j * 128 + h * 64:j * 128 + (h + 1) * 64]
                    src = pvt[bank][:,
                                    qsl * 130 + h * 65:qsl * 130 + h * 65 + 64]
                    if h == 1 and h1_act:
                        # tail: h1 on the (by now idle) ScalarE so the two
                        # head halves normalize in parallel
                        nc.scalar.activation(dst, src, AF.Copy,
                                             scale=rr[:, 1:2])
                    else:
                        nc.vector.tensor_scalar_mul(dst, src, rr[:, h:h + 1])

            # chunk processing order 1, 2, 3, 0: the final Act (exp) stretch
            # is the 4-item chunk 0, so the exp stream drains early and the
            # close is PE-dense.  The last two pair-groups run in "tail
            # mode": eager per-q-slice PE transposes + eager wo + immediate
            # ship instead of the DMA-transpose + deferred-wo pipeline.
            pair_jobs = [(0, 0), (0, 1), (1, 0), (1, 1), (2, 0), (2, 1),
                         (3, 0), (3, 1)]
            import os as _os2
            budgets = tuple(int(v) for v in _os2.environ.get(
                "K_BUDGETS", "400,400,400,400,400,400,400,400").split(","))
            dmatp_pend = [None]
            # group gate: only this chunk's q rope; k ropes and V blocks
            # gate per-item below so their filler units can slide into this
            # group's item stream
            need_map = {}
            for c, p in pair_jobs:
                req = set()
                if not (p == 0 and c == 0):
                    req.add(f"q{p}{c}")
                need_map[(c, p)] = req

            for j, (c, p) in enumerate(pair_jobs):
                kb_hi = (c * KPC + KPC) if causal else NSB
                is_final = j == len(pair_jobs) - 1
                tail_mode = is_final
                pull_until(need_map[(c, p)])

                pvt = [psPV.tile([128, 512], F32, tag="pv", name=f"pv{b}")
                       for b in range(2)]
                if tail_mode:
                    attn_sc = asb.tile([128, 512], BF16, tag="af",
                                       name="attn_sf")
                else:
                    attn_sc = asb.tile([128, 512], BF16, tag="asb",
                                       name="attn_sc")

                pend_q = []     # (kb, pt) awaiting PV
                norm_q = []     # q-slices whose PV is emitted, norm pending

                def flush_norms(p=p, c=c, pvt=pvt, attn_sc=attn_sc,
                                tail_mode=tail_mode, is_final=is_final,
                                norm_q=norm_q):
                    for jq in norm_q:
                        emit_norm(jq, pvt, attn_sc, h1_act=is_final)
                        if tail_mode:
                            # PE transpose + evac + eager wo + ship
                            tp = psX.tile([128, 512], F32, tag="x",
                                          name="tp")
                            tpb = tp.bitcast(BF16)
                            nc.tensor.transpose(
                                tpb[:, 0:128],
                                attn_sc[:, jq * 128:(jq + 1) * 128],
                                ident_t[:])
                            qg = c * KPC + jq
                            nc.vector.tensor_copy(
                                at[p][:, qg * 128:(qg + 1) * 128],
                                tpb[:, 0:128])
                            emit_wo_oc(qg, 0, tail=True)
                            emit_wo_oc(qg, 1, tail=True)
                    del norm_q[:]

                def drain_one(p=p, c=c, kb_hi=kb_hi, pvt=pvt,
                              pend_q=pend_q, norm_q=norm_q,
                              flush_norms=flush_norms):
                    pkb, ppt = pend_q.pop(0)
                    if pkb >= 4:
                        vname = f"v{pkb & ~1}"
                        if vname not in done_units:
                            pull_until({vname})
                    emit_pv(p, c, pkb, kb_hi, ppt, pvt)
                    if causal and pkb >= 4 * c:
                        norm_q.append(pkb - 4 * c)
                    flush_norms()

                budget = budgets[j]
                if j == 0:
                    # the first scores wait on the head rope chain; emit a
                    # burst of (rope-independent) filler first so the
                    # in-order PE queue isn't parked behind that wait
                    pull(int(_os2.environ.get("K_PREPULL", "2800")))
                for kb in range(kb_hi):
                    kc = kb // 4
                    if kc >= 1 or p == 1:
                        kname = f"k{p}{kc}"
                        if kname not in done_units:
                            pull_until({kname})
                    sc = psS.tile([128, 2 * QCH], F32, tag="sc", name="sc")
                    pt = prb.tile([128, 2 * QCH], BF16, tag="prb", name="prb")
                    emit_scores(p, c, kb, sc, pe_mask=is_final)
                    emit_exp(c, kb, sc, pt, pe_mask=is_final)
                    if kb == 2 and dmatp_pend[0] is not None:
                        # previous pair-chunk's attn transpose: deferred here
                        # so its sem wait is satisfied on arrival and doesn't
                        # head-of-line block the SP DMA queue
                        dmatp_pend[0]()
                        dmatp_pend[0] = None
                    pull(budget)
                    # drain pending PVs; a diag item's PV is held one extra
                    # item so the GPSIMD triangle-mask round trip is hidden
                    while pend_q:
                        diag0 = (causal and not is_final
                                 and pend_q[0][0] >= 4 * c)
                        if len(pend_q) == 1 and diag0:
                            break
                        drain_one()
                    pend_q.append((kb, pt))
                # drain remaining PVs + norms
                while pend_q:
                    pull(int(_os2.environ.get("K_DRAINPULL", "600")))
                    drain_one()
                if not causal:
                    norm_q.extend(range(4))
                    flush_norms()

                if not tail_mode:
                    # blocked transpose of the whole pair-chunk into at[p];
                    # emission deferred into the next pair-group
                    def mk_tp(p=p, c=c, attn_sc=attn_sc):
                        def emit():
                            nc.sync.dma_start_transpose(
                                at[p][:, c * QCH:(c + 1) * QCH].rearrange(
                                    "v (b q) -> v b q", b=4),
                                attn_sc[:])
                        return emit

                    dmatp_pend[0] = mk_tp()
                    if p == NPAIR - 1:
                        fillers.append(g_wo_chunk(c))

            # leftover fillers (late wo chunks)
            pull(10**12)

    nc.compile()
    return nc


def _host_prep(x, freqs_cos, freqs_sin, wq, wk, wv, wo):
    """Build the 8 per-core input maps (numpy, bf16)."""
    import ml_dtypes

    bf16 = ml_dtypes.bfloat16

    x = np.ascontiguousarray(x, dtype=np.float32)
    cosT = np.ascontiguousarray(freqs_cos.T, dtype=np.float32)  # [32, S]
    sinT = np.ascontiguousarray(freqs_sin.T, dtype=np.float32)

    c128 = np.tile(cosT, (4, 1))                                # [128, S]
    s128 = np.tile(np.concatenate([-sinT, sinT], 0), (2, 1))
    cs128 = np.ascontiguousarray(
        np.concatenate([c128, s128], axis=1)).astype(bf16)      # [128, 2S]
    # swap permutation: psum_sw = pmat.T @ A -> sw[m] = A[sigma(m)],
    # sigma swaps the 32-halves within each 64 block.
    pmat = np.zeros((128, 128), dtype=np.float32)
    for m in range(128):
        blk, off = divmod(m, 32)
        pmat[(blk ^ 1) * 32 + off, m] = 1.0
    ident = np.eye(128, dtype=np.float32)
    # causal diag 0/1 triangle: tri01[k, q] = 1 if k <= q else 0
    kk, qq = np.meshgrid(np.arange(128), np.arange(128), indexing="ij")
    tri01 = (kk <= qq).astype(np.float32)
    mdiagT = np.where(kk <= qq, 0.0, NEG_INF).astype(np.float32)
    msk = np.ascontiguousarray(
        np.concatenate([pmat, ident, tri01, mdiagT], axis=1)).astype(bf16)

    # rotate-half row permutation within each head
    rh = np.concatenate([np.arange(0, HD, 2), np.arange(1, HD, 2)])

    xT = [np.ascontiguousarray(x[b].T).astype(bf16) for b in range(B)]

    in_maps = []
    for core in range(NCORES):
        b, g = divmod(core, GROUPS)
        heads = [g * HPG + j for j in range(HPG)]
        qrows, vrows = [], []
        for h in heads:
            base = h * HD
            qrows.extend((base + rh).tolist())
            vrows.extend(range(base, base + HD))
        qrows = np.array(qrows)
        vrows = np.array(vrows)
        wqT = wq[qrows, :].T                                     # [D, 256]
        wkT = wk[qrows, :].T
        wvT = wv[vrows, :].T
        wqkvT = np.ascontiguousarray(
            np.concatenate([wqT, wkT, wvT], axis=1)).astype(bf16)
        woT = np.ascontiguousarray(wo[:, vrows].T).astype(bf16)  # [256, D]
        in_maps.append({
            "xT": xT[b], "wqkvT": wqkvT, "woT": woT,
            "cs128": cs128, "msk": msk,
        })
    return in_maps


def _mask_kind(mask):
    m = np.asarray(mask).reshape(S, S)
    if not np.any(m):
        return "zeros"
    qq, kk = np.meshgrid(np.arange(S), np.arange(S), indexing="ij")
    causal = np.where(kk <= qq, 0.0, NEG_INF).astype(np.float32)  # [q, k]
    if np.array_equal(m, causal):
        return "causal"
    return "general"


def _reference_host(x, freqs_cos, freqs_sin, mask, wq, wk, wv, wo):
    """Correctness fallback for arbitrary masks (host numpy, float64)."""
    b, s, d = x.shape
    hd = d // H
    xq = (x @ wq.T).reshape(b, s, H, hd)
    xk = (x @ wk.T).reshape(b, s, H, hd)
    xv = (x @ wv.T).reshape(b, s, H, hd)

    def rope(t):
        tr = t.reshape(b, s, H, hd // 2, 2)
        t0, t1 = tr[..., 0], tr[..., 1]
        cos = freqs_cos[None, :, None, :]
        sin = freqs_sin[None, :, None, :]
        return np.stack([t0 * cos - t1 * sin, t0 * sin + t1 * cos],
                        -1).reshape(b, s, H, hd)

    xq, xk = rope(xq), rope(xk)
    sc = np.einsum("bqhd,bkhd->bhqk", xq, xk) / np.sqrt(hd) + mask
    sc = sc - sc.max(-1, keepdims=True)
    e = np.exp(sc)
    pr = e / e.sum(-1, keepdims=True)
    o = np.einsum("bhqk,bkhd->bqhd", pr, xv).reshape(b, s, d)
    return (o @ wo.T).astype(np.float32)


def kernel(x, freqs_cos, freqs_sin, mask, wq, wk, wv, wo):
    kind = _mask_kind(mask)
    if kind == "general":
        return _reference_host(np.asarray(x, np.float64),
                               np.asarray(freqs_cos, np.float64),
                               np.asarray(freqs_sin, np.float64),
                               np.asarray(mask, np.float64),
                               np.asarray(wq, np.float64),
                               np.asarray(wk, np.float64),
                               np.asarray(wv, np.float64),
                               np.asarray(wo, np.float64))

    if kind not in _PROG_CACHE:
        _PROG_CACHE[kind] = _build_program(kind)
    nc = _PROG_CACHE[kind]

    in_maps = _host_prep(np.asarray(x, np.float32),
                         np.asarray(freqs_cos, np.float32),
                         np.asarray(freqs_sin, np.float32),
                         np.asarray(wq, np.float32),
                         np.asarray(wk, np.float32),
                         np.asarray(wv, np.float32),
                         np.asarray(wo, np.float32))
    res = run_bass_kernel_spmd(nc, in_maps, list(range(NCORES)))
    out = np.zeros((B, S, D), dtype=np.float32)
    for core in range(NCORES):
        out[core // GROUPS] += np.asarray(res.results[core]["out"],
                                          dtype=np.float32)
    return out


# revision 55
# speedup vs baseline: 1.2355x; 1.0065x over previous
"""Trainium2 Bass kernel for nn_Attention_12515534700827.

Multi-head causal attention with RoPE: B=2, S=2048, D=1024, H=16, HD=64.
Sharding: 8 cores = 2 (batch) x 4 (head groups of 4 heads). Each core
computes its 4 heads' attention + its slice of the wo projection; the host
sums the 4 partial outputs per batch (the "all-reduce after wo").

v3 (fused single-stream): projections, attention, and the wo projection are
emitted as ONE interleaved instruction stream so the ScalarE exp stream (the
second-largest engine load) overlaps the projection/wo matmuls instead of
running in its own phase.

Key differences vs v2:
  - PV computed with probs as the STATIONARY operand and V' ([V|ones]) as
    the MOVING operand: out[q, vd|den] per (head, q-slice).  The moving free
    dim drops from ~512 to 65, halving PV cost; the softmax denominator
    arrives as psum column 64 per head so normalization becomes a
    per-partition scalar multiply (no PE broadcast, no reciprocal of a
    [64,1024] tile, no partition-shift DMA bounce).
  - The resulting attn tiles are [q, vd]; wo needs [vd, q].  Transposed via
    dma_start_transpose (XBAR 16x128 tiles, cheap on the DMA engines)
    straight into the persistent at[] tiles; the last pair uses PE
    transposes so the tail isn't gated on a DMA round trip.
  - Fused emission: after the head block (V sb0-3 + Q/K chunk0 for both
    pairs over x cols 0:1024), attention items start immediately; the
    remaining V blocks, Q/K chunks, rope chains and deferred wo tiles are
    "fillers" pulled between items to keep the PE dense while ScalarE
    streams the exps.
  - wo(c) is deferred ~2 chunks so it lands as filler in the late,
    otherwise Act-bound stretch.
  - Inputs land in few large DMAs (HWDGE descriptor time ~0.6us each).

PSUM budget (8 banks): scores 2x[128,1024] (4) + PV 2x[128,512] (2) +
misc single-shot rotation psX 2x[128,512] (2: rope swaps, V pairs, wo,
tail transposes).  PV packs 2 q-slices x 2 heads x 65 cols per bank with
one accumulation-group start/stop per bank (hardware clears has_written at
bank granularity).
"""

import sys

if "/opt/trn_rl_repo" not in sys.path:
    sys.path.insert(0, "/opt/trn_rl_repo")

import numpy as np

import concourse.mybir as mybir
import concourse.tile as tile
from concourse import bacc
from concourse.bass_utils import run_bass_kernel_spmd

F32 = mybir.dt.float32
BF16 = mybir.dt.bfloat16
AF = mybir.ActivationFunctionType

B, S, D, H, HD = 2, 2048, 1024, 16, 64
NCORES = 8
GROUPS = 4            # head groups (cores per batch)
HPG = H // GROUPS     # heads per core = 4
NPAIR = HPG // 2      # head pairs per core = 2
NEG_INF = -1e9
SM_SCALE = 1.0 / float(np.sqrt(HD))  # 0.125

NIT = D // 128        # 8 contraction tiles
NSB = S // 128        # 16 seq blocks
QCH = 512             # attention q-chunk
NCHUNK = S // QCH     # 4
KPC = QCH // 128      # k/q 128-blocks per chunk = 4

_PROG_CACHE = {}


def _build_program(mask_kind: str):
    """mask_kind: 'causal' (trimmed + diag mask) or 'zeros' (full)."""
    causal = mask_kind == "causal"
    nc = bacc.Bacc("TRN2", target_bir_lowering=False, debug=False,
                   num_devices=NCORES)

    xT_d = nc.dram_tensor("xT", [D, S], BF16, kind="ExternalInput").ap()
    # per 128-row block: [wq | wk | wv] column slices
    wqkv_d = nc.dram_tensor("wqkvT", [D, 3 * HPG * HD], BF16,
                            kind="ExternalInput").ap()
    woT_d = nc.dram_tensor("woT", [HPG * HD, D], BF16, kind="ExternalInput").ap()
    cs_d = nc.dram_tensor("cs128", [128, 2 * S], BF16, kind="ExternalInput").ap()
    # [pmat | ident | mdiagT]
    msk_d = nc.dram_tensor("msk", [128, 512], BF16, kind="ExternalInput").ap()
    out_d = nc.dram_tensor("out", [S, D], BF16, kind="ExternalOutput").ap()

    with tile.TileContext(nc) as tc:
        from contextlib import ExitStack

        with ExitStack() as root:
            pers = root.enter_context(tc.tile_pool(name="pers", bufs=1))

            # ---- persistent SBUF tiles ----
            qt = [pers.tile([128, S], BF16, tag=f"qt{p}", name=f"qt{p}")
                  for p in range(NPAIR)]
            kt = [pers.tile([128, S], BF16, tag=f"kt{p}", name=f"kt{p}")
                  for p in range(NPAIR)]
            # V' per (pair, s-block): [128,130] = V_A|ones|V_B|ones
            vp = [[pers.tile([128, 130], BF16, tag=f"vp{p}_{sb}",
                             name=f"vp{p}_{sb}")
                   for sb in range(NSB)] for p in range(NPAIR)]
            # attnT per pair: [vd(2 heads x 64), S]
            at = [pers.tile([128, S], BF16, tag=f"at{p}", name=f"at{p}")
                  for p in range(NPAIR)]
            wo2 = pers.tile([128, NPAIR * D], BF16, tag="wo2", name="wo2")
            wo_t = [wo2[:, p * D:(p + 1) * D] for p in range(NPAIR)]
            msk_t = pers.tile([128, 512], BF16, tag="msk", name="msk")
            pm_t = msk_t[:, 0:128]
            ident_t = msk_t[:, 128:256]
            tri_t = msk_t[:, 256:384]   # 0/1 lower-k triangle (k <= q)
            mdiag_t = msk_t[:, 384:512]  # additive -1e9 upper-k triangle
            ones1 = pers.tile([1, 64], BF16, tag="ones1", name="ones1")

            # all memsets first so the Pool engine is done before attention
            nc.gpsimd.memset(ones1[:], 1.0)
            for p in range(NPAIR):
                for sb in range(NSB):
                    nc.gpsimd.memset(vp[p][sb][:, 64:65], 1.0)
                    nc.gpsimd.memset(vp[p][sb][:, 129:130], 1.0)

            ld = root.enter_context(tc.tile_pool(name="ld", bufs=1))
            sbA = root.enter_context(tc.tile_pool(name="sbA", bufs=2))
            prb = root.enter_context(tc.tile_pool(name="prb", bufs=5))
            asb = root.enter_context(tc.tile_pool(name="asb", bufs=3))
            nrm = root.enter_context(tc.tile_pool(name="nrm", bufs=4))
            osb = root.enter_context(tc.tile_pool(name="osb", bufs=4))
            psS = root.enter_context(
                tc.tile_pool(name="psS", bufs=2, space="PSUM"))
            psPV = root.enter_context(
                tc.tile_pool(name="psPV", bufs=2, space="PSUM"))
            psX = root.enter_context(
                tc.tile_pool(name="psX", bufs=2, space="PSUM"))

            # PE warm-up: dummy matmuls during the otherwise-dead input-DMA
            # window release the HAM clock gate so the first real
            # projections run at full clock
            wt = psX.tile([64, 64], F32, tag="x", name="warm")
            import os as _os
            _wu = int(_os.environ.get("K_WARMUP", "180"))
            for _ in range(_wu):
                nc.tensor.matmul(wt[:], ones1[:], ones1[:],
                                 start=True, stop=True)

            # ---- input DMAs (few, large; wqkv split so it=0 lands early)
            xts = [ld.tile([128, S], BF16, tag=f"xt{it}", name=f"xt{it}")
                   for it in range(NIT)]
            wqkv = ld.tile([128, NIT * 3 * HPG * HD], BF16, tag="wqkv",
                           name="wqkv")
            wq_t = [wqkv[:, it * 768:it * 768 + 256] for it in range(NIT)]
            wk_t = [wqkv[:, it * 768 + 256:it * 768 + 512] for it in range(NIT)]
            wv_t = [wqkv[:, it * 768 + 512:it * 768 + 768] for it in range(NIT)]
            cs_t = ld.tile([128, 2 * S], BF16, tag="cs128", name="cs128")
            c_t = cs_t[:, 0:S]
            s_t = cs_t[:, S:2 * S]

            for half in range(2):
                its = slice(half * 4 * 128, (half + 1) * 4 * 128)
                nc.sync.dma_start(
                    out=wqkv[:, half * 3072:(half + 1) * 3072].rearrange(
                        "p (i c) -> p i c", i=4),
                    in_=wqkv_d[its, :].rearrange("(i p) c -> p i c", p=128))
            for it in range(NIT):
                sl = slice(it * 128, (it + 1) * 128)
                nc.sync.dma_start(out=xts[it][:, 0:1024],
                                  in_=xT_d[sl, 0:1024])
            nc.sync.dma_start(out=msk_t[:], in_=msk_d[:])
            # cos/sin chunks 0-1 now (the chunk-0 ropes need it right after
            # the head); chunks 2-3 after the xB stream — the ch2 ropes are
            # ~20us out and this keeps the early DMA window lean
            nc.sync.dma_start(
                out=cs_t[:].rearrange("p (h c) -> p h c", h=2)[:, :, 0:1024],
                in_=cs_d[:].rearrange("p (h c) -> p h c", h=2)[:, :, 0:1024])
            for it in range(NIT):
                sl = slice(it * 128, (it + 1) * 128)
                nc.sync.dma_start(out=xts[it][:, 1024:2048],
                                  in_=xT_d[sl, 1024:2048])
            nc.sync.dma_start(
                out=cs_t[:].rearrange("p (h c) -> p h c", h=2)[:, :, 1024:2048],
                in_=cs_d[:].rearrange("p (h c) -> p h c", h=2)[:, :, 1024:2048])
            nc.sync.dma_start(
                out=wo2[:].rearrange("p (a d) -> p a d", a=NPAIR),
                in_=woT_d[:].rearrange("(a p) d -> p a d", a=NPAIR, p=128))

            # ---------------- helpers ----------------
            def v_finish(sb, ps):
                """psum [128, 256] (pair-packed V) -> vp tiles, both pairs."""
                for p in range(NPAIR):
                    src = ps[:, p * 128:(p + 1) * 128] \
                        .rearrange("p (b c) -> p b c", b=2, c=64)
                    dst = vp[p][sb][:, 0:130] \
                        .rearrange("p (b c) -> p b c", b=2, c=65)[:, :, 0:64]
                    nc.vector.tensor_copy(dst, src)

            def rope_copy(ps, on_act=True):
                a_sb = sbA.tile([128, 512], BF16, tag="a_sb", name="a_sb")
                if on_act:
                    nc.scalar.activation(a_sb[:], ps[:], AF.Copy)
                else:
                    nc.vector.tensor_copy(a_sb[:], ps[:])
                return a_sb

            def rope_finish(a_sb, dst, p, ch, sw_act=False):
                """rot = A*C + swap(A)*S into dst[p][:, chunk].

                sw_act: evacuate the swap psum on ScalarE so the DVE chain
                is 3 SBUF-only ops (shortest latency; used for the
                transition-critical head units)."""
                qs = slice(ch * 512, (ch + 1) * 512)
                t1 = sbA.tile([128, 512], BF16, tag="t1", name="t1")
                nc.vector.tensor_mul(t1[:], a_sb[:], c_t[:, qs])
                sw = psX.tile([128, 512], F32, tag="x", name="psSW")
                nc.tensor.matmul(sw[:], pm_t[:], a_sb[:],
                                 start=True, stop=True)
                t2 = sbA.tile([128, 512], BF16, tag="t2", name="t2")
                if sw_act:
                    sw_sb = sbA.tile([128, 512], BF16, tag="sw_sb",
                                     name="sw_sb")
                    nc.scalar.activation(sw_sb[:], sw[:], AF.Copy)
                    nc.vector.tensor_mul(t2[:], sw_sb[:], s_t[:, qs])
                else:
                    nc.vector.tensor_mul(t2[:], sw[:], s_t[:, qs])
                nc.vector.tensor_add(dst[p][:, qs], t1[:], t2[:])

            # ---------------- head block ----------------
            # it-major over x cols 0:1024: V s-blocks 0-3 + Q/K chunk 0 for
            # both pairs, so both pairs' chunk-0 attention unlocks first.
            hv = [psPV.tile([128, 512], F32, tag="pv", name=f"hv{b}")
                  for b in range(2)]
            hq = [psS.tile([128, 2 * QCH], F32, tag="sc", name=f"hq{p}")
                  for p in range(NPAIR)]
            for it in range(NIT):
                st, sp = it == 0, it == NIT - 1
                for sb in range(4):
                    b, o = divmod(sb, 2)
                    nc.tensor.matmul(hv[b][:, o * 256:(o + 1) * 256],
                                     xts[it][:, sb * 128:(sb + 1) * 128],
                                     wv_t[it][:],
                                     start=(st and o == 0),
                                     stop=(sp and o == 1))
                for p in range(NPAIR):
                    pc = slice(p * 128, (p + 1) * 128)
                    nc.tensor.matmul(hq[p][:, 0:512], wq_t[it][:, pc],
                                     xts[it][:, 0:512], start=st, stop=sp)
                    nc.tensor.matmul(hq[p][:, 512:1024], wk_t[it][:, pc],
                                     xts[it][:, 0:512], start=st, stop=sp)
            # rope pair 0's chunk-0 q/k now (unblocks the first items);
            # pair 1's chunk 0 becomes the first filler unit.  v_finish
            # after — the first PV only needs vp0 one item later.
            a_q0 = rope_copy(hq[0][:, 0:512])
            a_k0 = rope_copy(hq[0][:, 512:1024])
            rope_finish(a_q0, qt, 0, 0)
            rope_finish(a_k0, kt, 0, 0)
            for sb in range(4):
                b, o = divmod(sb, 2)
                v_finish(sb, hv[b][:, o * 256:(o + 1) * 256])

            # ---------------- filler machinery ----------------
            done_units = set()

            def g_v_unit(sb0):
                """V s-blocks (sb0, sb0+1): packed 2-per-bank accumulation."""
                t = psX.tile([128, 512], F32, tag="x", name=f"v{sb0}")
                for it in range(NIT):
                    st, sp = it == 0, it == NIT - 1
                    for o in range(2):
                        nc.tensor.matmul(
                            t[:, o * 256:(o + 1) * 256],
                            xts[it][:, (sb0 + o) * 128:(sb0 + o + 1) * 128],
                            wv_t[it][:],
                            start=(st and o == 0), stop=(sp and o == 1))
                        yield 107
                v_finish(sb0, t[:, 0:256])
                v_finish(sb0 + 1, t[:, 256:512])
                done_units.add(f"v{sb0}")
                yield 0

            rope_pend = [None]

            def g_p1_head_rope():
                """pair 1's chunk-0 rope (head produced the psum)."""
                a_q1 = rope_copy(hq[1][:, 0:512])
                a_k1 = rope_copy(hq[1][:, 512:1024])
                rope_finish(a_q1, qt, 1, 0)
                done_units.add("q10")
                yield 213
                rope_finish(a_k1, kt, 1, 0)
                done_units.add("k10")
                yield 213

            def g_qk_unit(p, w, ch):
                wt = wq_t if w == "q" else wk_t
                dst = qt if w == "q" else kt
                t = psX.tile([128, 512], F32, tag="x", name=f"qk{p}{w}{ch}")
                pc = slice(p * 128, (p + 1) * 128)
                for it in range(NIT):
                    nc.tensor.matmul(t[:], wt[it][:, pc],
                                     xts[it][:, ch * 512:(ch + 1) * 512],
                                     start=(it == 0), stop=(it == NIT - 1))
                    yield 213
                a_sb = rope_copy(t, on_act=(ch < 2))
                if rope_pend[0] is not None:
                    pa, pdst, pp, pch, pname = rope_pend[0]
                    rope_finish(pa, pdst, pp, pch)
                    done_units.add(pname)
                    yield 213
                rope_pend[0] = (a_sb, dst, p, ch, f"{w}{p}{ch}")

            def flush_pend():
                if rope_pend[0] is not None:
                    pa, pdst, pp, pch, pname = rope_pend[0]
                    rope_finish(pa, pdst, pp, pch)
                    done_units.add(pname)
                    rope_pend[0] = None

            def g_rope_flush():
                if rope_pend[0] is not None:
                    flush_pend()
                    yield 213

            ob_pend = {}

            def emit_wo_oc(sb, oc, tail=False):
                ssl = slice(sb * 128, (sb + 1) * 128)
                osl = slice(oc * 512, (oc + 1) * 512)
                ps = psX.tile([128, 512], F32, tag="x", name="psW")
                for p in range(NPAIR):
                    nc.tensor.matmul(
                        ps[:], at[p][:, ssl], wo_t[p][:, osl],
                        start=(p == 0), stop=(p == NPAIR - 1))
                if sb not in ob_pend:
                    ob_pend[sb] = osb.tile([128, 1024], BF16, tag="osb",
                                           name="osb")
                ob = ob_pend[sb]
                if tail and oc == 0:
                    # ScalarE is idle at the tail: evac halves in parallel
                    nc.scalar.activation(ob[:, osl], ps[:], AF.Copy)
                else:
                    nc.vector.tensor_copy(ob[:, osl], ps[:])
                if oc == 1:
                    nc.sync.dma_start(out=out_d[ssl, :],
                                      in_=ob_pend.pop(sb)[:])

            def g_wo_chunk(c):
                for sb in range(c * KPC, (c + 1) * KPC):
                    for oc in range(2):
                        emit_wo_oc(sb, oc)
                        yield 426

            # filler order: chunk-1 q/k (x cols 512:1024, already loaded)
            # before the xB-dependent V blocks; V blocks paced so vp(sb) is
            # ready ~when chunk sb//4's PV needs it.
            fillers = [
                g_p1_head_rope(),
                g_qk_unit(0, "q", 1), g_qk_unit(0, "k", 1),
                g_v_unit(4),
                g_qk_unit(1, "q", 1), g_qk_unit(1, "k", 1),
                g_v_unit(6), g_v_unit(8),
                g_qk_unit(0, "q", 2), g_qk_unit(0, "k", 2),
                g_v_unit(10),
                g_qk_unit(1, "q", 2), g_qk_unit(1, "k", 2),
                g_v_unit(12),
                g_qk_unit(0, "q", 3), g_qk_unit(0, "k", 3),
                g_v_unit(14),
                g_qk_unit(1, "q", 3), g_qk_unit(1, "k", 3),
                g_rope_flush(),
            ]
            fill_iq = [0]

            def pull(budget_ns):
                got = 0
                while fill_iq[0] < len(fillers):
                    g = fillers[fill_iq[0]]
                    try:
                        while got < budget_ns:
                            got += next(g)
                    except StopIteration:
                        fill_iq[0] += 1
                        continue
                    break
                return got

            def pull_until(units):
                while not units <= done_units:
                    # the last missing unit may be parked in the rope pend
                    if rope_pend[0] is not None and \
                            units <= (done_units | {rope_pend[0][4]}):
                        flush_pend()
                        return
                    if pull(600) == 0:
                        flush_pend()
                        assert units <= done_units, (
                            f"filler units {units - done_units} never emitted")
                        return

            # ---------------- attention ----------------
            def emit_scores(p, c, kb, sc, pe_mask=False):
                q0 = c * QCH
                k0 = kb * 128
                trim = max(q0, k0) if causal else q0
                t_off = trim - q0
                on_diag = causal and pe_mask and k0 >= q0
                for h in range(2):
                    hsl = slice(h * 64, (h + 1) * 64)
                    nc.tensor.matmul(
                        sc[:, h * QCH + t_off:(h + 1) * QCH],
                        kt[p][hsl, k0:k0 + 128],
                        qt[p][hsl, trim:q0 + QCH],
                        start=True, stop=not on_diag)
                if on_diag:
                    for h in range(2):
                        nc.tensor.matmul(
                            sc[:, h * QCH + t_off:h * QCH + t_off + 128],
                            ident_t[:], mdiag_t[:],
                            start=False, stop=True)

            def emit_exp(c, kb, sc, pt, pe_mask=False):
                """exp (trimmed); for diag items the upper-k triangle of the
                128-block is zeroed on the idle GPSIMD engine — except on
                the pe_mask path (tail), which folded -1e9 into the scores
                on the PE to keep the exp->PV latency minimal."""
                q0 = c * QCH
                t_off = (max(q0, kb * 128) - q0) if causal else 0
                if t_off == 0:
                    nc.scalar.activation(pt[:, 0:2 * QCH], sc[:, 0:2 * QCH],
                                         AF.Exp, scale=SM_SCALE)
                else:
                    sc3 = sc[:, 0:2 * QCH].rearrange(
                        "p (b c) -> p b c", b=2, c=QCH)[:, :, t_off:]
                    pt3 = pt[:, 0:2 * QCH].rearrange(
                        "p (b c) -> p b c", b=2, c=QCH)[:, :, t_off:]
                    nc.scalar.activation(pt3, sc3, AF.Exp, scale=SM_SCALE)
                if causal and not pe_mask and kb * 128 >= q0:
                    dg = pt[:, 0:2 * QCH].rearrange(
                        "p (b c) -> p b c", b=2,
                        c=QCH)[:, :, t_off:t_off + 128]
                    tri3 = tri_t.rearrange("p (b c) -> p b c", b=1)
                    nc.gpsimd.tensor_mul(dg, dg,
                                         tri3.broadcast_to([128, 2, 128]))

            def emit_pv(p, c, kb, kb_hi, pt, pvt):
                qs_lo = max(0, kb - 4 * c) if causal else 0
                for qs in range(qs_lo, 4):
                    bank, qsl = divmod(qs, 2)
                    last_kb = (4 * c + bank * 2 + 1) if causal else kb_hi - 1
                    for h in range(2):
                        col = qsl * 130 + h * 65
                        nc.tensor.matmul(
                            pvt[bank][:, col:col + 65],
                            pt[:, h * QCH + qs * 128:h * QCH + qs * 128 + 128],
                            vp[p][kb][:, h * 65:h * 65 + 65],
                            start=(kb == 0 and qsl == 0 and h == 0),
                            stop=(kb == last_kb and qsl == 1 and h == 1))

            def emit_norm(j, pvt, attn_sc, h1_act=False):
                """normalize q-slice j of the pair-chunk into attn_sc."""
                bank, qsl = divmod(j, 2)
                rr = nrm.tile([128, 2], F32, tag="rr", name="rr")
                den = pvt[bank][:, qsl * 130:qsl * 130 + 130].rearrange(
                    "p (h c) -> p h c", h=2)[:, :, 64:65]
                nc.vector.reciprocal_approx_fast(
                    rr[:].rearrange("p (h c) -> p h c", c=1), den)
                for h in range(2):
                    dst = attn_sc[:, j * 128 + h * 64:j * 128 + (h + 1) * 64]
                    src = pvt[bank][:,
                                    qsl * 130 + h * 65:qsl * 130 + h * 65 + 64]
                    if h == 1 and h1_act:
                        # tail: h1 on the (by now idle) ScalarE so the two
                        # head halves normalize in parallel
                        nc.scalar.activation(dst, src, AF.Copy,
                                             scale=rr[:, 1:2])
                    else:
                        nc.vector.tensor_scalar_mul(dst, src, rr[:, h:h + 1])

            # chunk processing order 1, 2, 3, 0: the final Act (exp) stretch
            # is the 4-item chunk 0, so the exp stream drains early and the
            # close is PE-dense.  The last two pair-groups run in "tail
            # mode": eager per-q-slice PE transposes + eager wo + immediate
            # ship instead of the DMA-transpose + deferred-wo pipeline.
            pair_jobs = [(0, 0), (0, 1), (1, 0), (1, 1), (2, 0), (2, 1),
                         (3, 0), (3, 1)]
            import os as _os2
            budgets = tuple(int(v) for v in _os2.environ.get(
                "K_BUDGETS", "400,400,400,400,400,400,400,400").split(","))
            dmatp_pend = [None]
            # group gate: only this chunk's q rope; k ropes and V blocks
            # gate per-item below so their filler units can slide into this
            # group's item stream
            need_map = {}
            for c, p in pair_jobs:
                req = set()
                if not (p == 0 and c == 0):
                    req.add(f"q{p}{c}")
                need_map[(c, p)] = req

            for j, (c, p) in enumerate(pair_jobs):
                kb_hi = (c * KPC + KPC) if causal else NSB
                is_final = j == len(pair_jobs) - 1
                tail_mode = is_final
                pull_until(need_map[(c, p)])

                pvt = [psPV.tile([128, 512], F32, tag="pv", name=f"pv{b}")
                       for b in range(2)]
                if tail_mode:
                    attn_sc = asb.tile([128, 512], BF16, tag="af",
                                       name="attn_sf")
                else:
                    attn_sc = asb.tile([128, 512], BF16, tag="asb",
                                       name="attn_sc")

                pend_q = []     # (kb, pt) awaiting PV
                norm_q = []     # q-slices whose PV is emitted, norm pending

                def flush_norms(p=p, c=c, pvt=pvt, attn_sc=attn_sc,
                                tail_mode=tail_mode, is_final=is_final,
                                norm_q=norm_q):
                    for jq in norm_q:
                        emit_norm(jq, pvt, attn_sc, h1_act=is_final)
                        if tail_mode:
                            # PE transpose + evac + eager wo + ship
                            tp = psX.tile([128, 512], F32, tag="x",
                                          name="tp")
                            tpb = tp.bitcast(BF16)
                            nc.tensor.transpose(
                                tpb[:, 0:128],
                                attn_sc[:, jq * 128:(jq + 1) * 128],
                                ident_t[:])
                            qg = c * KPC + jq
                            nc.vector.tensor_copy(
                                at[p][:, qg * 128:(qg + 1) * 128],
                                tpb[:, 0:128])
                            emit_wo_oc(qg, 0, tail=True)
                            emit_wo_oc(qg, 1, tail=True)
                    del norm_q[:]

                def drain_one(p=p, c=c, kb_hi=kb_hi, pvt=pvt,
                              pend_q=pend_q, norm_q=norm_q,
                              flush_norms=flush_norms):
                    pkb, ppt = pend_q.pop(0)
                    if pkb >= 4:
                        vname = f"v{pkb & ~1}"
                        if vname not in done_units:
                            pull_until({vname})
                    emit_pv(p, c, pkb, kb_hi, ppt, pvt)
                    if causal and pkb >= 4 * c:
                        norm_q.append(pkb - 4 * c)
                    flush_norms()

                budget = budgets[j]
                if j == 0:
                    # the first scores wait on the head rope chain; emit a
                    # burst of (rope-independent) filler first so the
                    # in-order PE queue isn't parked behind that wait
                    pull(int(_os2.environ.get("K_PREPULL", "3200")))
                for kb in range(kb_hi):
                    kc = kb // 4
                    if kc >= 1 or p == 1:
                        kname = f"k{p}{kc}"
                        if kname not in done_units:
                            pull_until({kname})
                    sc = psS.tile([128, 2 * QCH], F32, tag="sc", name="sc")
                    pt = prb.tile([128, 2 * QCH], BF16, tag="prb", name="prb")
                    emit_scores(p, c, kb, sc, pe_mask=is_final)
                    emit_exp(c, kb, sc, pt, pe_mask=is_final)
                    if kb == 2 and dmatp_pend[0] is not None:
                        # previous pair-chunk's attn transpose: deferred here
                        # so its sem wait is satisfied on arrival and doesn't
                        # head-of-line block the SP DMA queue
                        dmatp_pend[0]()
                        dmatp_pend[0] = None
                    pull(budget)
                    # drain pending PVs; a diag item's PV is held one extra
                    # item so the GPSIMD triangle-mask round trip is hidden
                    pvdepth = int(_os2.environ.get("K_PVDEPTH", "1"))
                    while pend_q:
                        diag0 = (causal and not is_final
                                 and pend_q[0][0] >= 4 * c)
                        if len(pend_q) < pvdepth + (1 if diag0 else 0):
                            break
                        drain_one()
                    pend_q.append((kb, pt))
                # drain remaining PVs + norms
                while pend_q:
                    pull(int(_os2.environ.get("K_DRAINPULL", "500")))
                    drain_one()
                if not causal:
                    norm_q.extend(range(4))
                    flush_norms()

                if not tail_mode:
                    # blocked transpose of the whole pair-chunk into at[p];
                    # emission deferred into the next pair-group
                    def mk_tp(p=p, c=c, attn_sc=attn_sc):
                        def emit():
                            nc.sync.dma_start_transpose(
                                at[p][:, c * QCH:(c + 1) * QCH].rearrange(
                                    "v (b q) -> v b q", b=4),
                                attn_sc[:])
                        return emit

                    dmatp_pend[0] = mk_tp()
                    if p == NPAIR - 1:
                        fillers.append(g_wo_chunk(c))

            # leftover fillers (late wo chunks)
            pull(10**12)

    nc.compile()
    return nc


def _host_prep(x, freqs_cos, freqs_sin, wq, wk, wv, wo):
    """Build the 8 per-core input maps (numpy, bf16)."""
    import ml_dtypes

    bf16 = ml_dtypes.bfloat16

    x = np.ascontiguousarray(x, dtype=np.float32)
    cosT = np.ascontiguousarray(freqs_cos.T, dtype=np.float32)  # [32, S]
    sinT = np.ascontiguousarray(freqs_sin.T, dtype=np.float32)

    c128 = np.tile(cosT, (4, 1))                                # [128, S]
    s128 = np.tile(np.concatenate([-sinT, sinT], 0), (2, 1))
    cs128 = np.ascontiguousarray(
        np.concatenate([c128, s128], axis=1)).astype(bf16)      # [128, 2S]
    # swap permutation: psum_sw = pmat.T @ A -> sw[m] = A[sigma(m)],
    # sigma swaps the 32-halves within each 64 block.
    pmat = np.zeros((128, 128), dtype=np.float32)
    for m in range(128):
        blk, off = divmod(m, 32)
        pmat[(blk ^ 1) * 32 + off, m] = 1.0
    ident = np.eye(128, dtype=np.float32)
    # causal diag 0/1 triangle: tri01[k, q] = 1 if k <= q else 0
    kk, qq = np.meshgrid(np.arange(128), np.arange(128), indexing="ij")
    tri01 = (kk <= qq).astype(np.float32)
    mdiagT = np.where(kk <= qq, 0.0, NEG_INF).astype(np.float32)
    msk = np.ascontiguousarray(
        np.concatenate([pmat, ident, tri01, mdiagT], axis=1)).astype(bf16)

    # rotate-half row permutation within each head
    rh = np.concatenate([np.arange(0, HD, 2), np.arange(1, HD, 2)])

    xT = [np.ascontiguousarray(x[b].T).astype(bf16) for b in range(B)]

    in_maps = []
    for core in range(NCORES):
        b, g = divmod(core, GROUPS)
        heads = [g * HPG + j for j in range(HPG)]
        qrows, vrows = [], []
        for h in heads:
            base = h * HD
            qrows.extend((base + rh).tolist())
            vrows.extend(range(base, base + HD))
        qrows = np.array(qrows)
        vrows = np.array(vrows)
        wqT = wq[qrows, :].T                                     # [D, 256]
        wkT = wk[qrows, :].T
        wvT = wv[vrows, :].T
        wqkvT = np.ascontiguousarray(
            np.concatenate([wqT, wkT, wvT], axis=1)).astype(bf16)
        woT = np.ascontiguousarray(wo[:, vrows].T).astype(bf16)  # [256, D]
        in_maps.append({
            "xT": xT[b], "wqkvT": wqkvT, "woT": woT,
            "cs128": cs128, "msk": msk,
        })
    return in_maps


def _mask_kind(mask):
    m = np.asarray(mask).reshape(S, S)
    if not np.any(m):
        return "zeros"
    qq, kk = np.meshgrid(np.arange(S), np.arange(S), indexing="ij")
    causal = np.where(kk <= qq, 0.0, NEG_INF).astype(np.float32)  # [q, k]
    if np.array_equal(m, causal):
        return "causal"
    return "general"


def _reference_host(x, freqs_cos, freqs_sin, mask, wq, wk, wv, wo):
    """Correctness fallback for arbitrary masks (host numpy, float64)."""
    b, s, d = x.shape
    hd = d // H
    xq = (x @ wq.T).reshape(b, s, H, hd)
    xk = (x @ wk.T).reshape(b, s, H, hd)
    xv = (x @ wv.T).reshape(b, s, H, hd)

    def rope(t):
        tr = t.reshape(b, s, H, hd // 2, 2)
        t0, t1 = tr[..., 0], tr[..., 1]
        cos = freqs_cos[None, :, None, :]
        sin = freqs_sin[None, :, None, :]
        return np.stack([t0 * cos - t1 * sin, t0 * sin + t1 * cos],
                        -1).reshape(b, s, H, hd)

    xq, xk = rope(xq), rope(xk)
    sc = np.einsum("bqhd,bkhd->bhqk", xq, xk) / np.sqrt(hd) + mask
    sc = sc - sc.max(-1, keepdims=True)
    e = np.exp(sc)
    pr = e / e.sum(-1, keepdims=True)
    o = np.einsum("bhqk,bkhd->bqhd", pr, xv).reshape(b, s, d)
    return (o @ wo.T).astype(np.float32)


def kernel(x, freqs_cos, freqs_sin, mask, wq, wk, wv, wo):
    kind = _mask_kind(mask)
    if kind == "general":
        return _reference_host(np.asarray(x, np.float64),
                               np.asarray(freqs_cos, np.float64),
                               np.asarray(freqs_sin, np.float64),
                               np.asarray(mask, np.float64),
                               np.asarray(wq, np.float64),
                               np.asarray(wk, np.float64),
                               np.asarray(wv, np.float64),
                               np.asarray(wo, np.float64))

    if kind not in _PROG_CACHE:
        _PROG_CACHE[kind] = _build_program(kind)
    nc = _PROG_CACHE[kind]

    in_maps = _host_prep(np.asarray(x, np.float32),
                         np.asarray(freqs_cos, np.float32),
                         np.asarray(freqs_sin, np.float32),
                         np.asarray(wq, np.float32),
                         np.asarray(wk, np.float32),
                         np.asarray(wv, np.float32),
                         np.asarray(wo, np.float32))
    res = run_bass_kernel_spmd(nc, in_maps, list(range(NCORES)))
    out = np.zeros((B, S, D), dtype=np.float32)
    for core in range(NCORES):
        out[core // GROUPS] += np.asarray(res.results[core]["out"],
                                          dtype=np.float32)
    return out


# revision 61
# speedup vs baseline: 1.2376x; 1.0018x over previous
"""Trainium2 Bass kernel for nn_Attention_12515534700827.

Multi-head causal attention with RoPE: B=2, S=2048, D=1024, H=16, HD=64.
Sharding: 8 cores = 2 (batch) x 4 (head groups of 4 heads). Each core
computes its 4 heads' attention + its slice of the wo projection; the host
sums the 4 partial outputs per batch (the "all-reduce after wo").

v3 (fused single-stream): projections, attention, and the wo projection are
emitted as ONE interleaved instruction stream so the ScalarE exp stream (the
second-largest engine load) overlaps the projection/wo matmuls instead of
running in its own phase.

Key differences vs v2:
  - PV computed with probs as the STATIONARY operand and V' ([V|ones]) as
    the MOVING operand: out[q, vd|den] per (head, q-slice).  The moving free
    dim drops from ~512 to 65, halving PV cost; the softmax denominator
    arrives as psum column 64 per head so normalization becomes a
    per-partition scalar multiply (no PE broadcast, no reciprocal of a
    [64,1024] tile, no partition-shift DMA bounce).
  - The resulting attn tiles are [q, vd]; wo needs [vd, q].  Transposed via
    dma_start_transpose (XBAR 16x128 tiles, cheap on the DMA engines)
    straight into the persistent at[] tiles; the last pair uses PE
    transposes so the tail isn't gated on a DMA round trip.
  - Fused emission: after the head block (V sb0-3 + Q/K chunk0 for both
    pairs over x cols 0:1024), attention items start immediately; the
    remaining V blocks, Q/K chunks, rope chains and deferred wo tiles are
    "fillers" pulled between items to keep the PE dense while ScalarE
    streams the exps.
  - wo(c) is deferred ~2 chunks so it lands as filler in the late,
    otherwise Act-bound stretch.
  - Inputs land in few large DMAs (HWDGE descriptor time ~0.6us each).

PSUM budget (8 banks): scores 2x[128,1024] (4) + PV 2x[128,512] (2) +
misc single-shot rotation psX 2x[128,512] (2: rope swaps, V pairs, wo,
tail transposes).  PV packs 2 q-slices x 2 heads x 65 cols per bank with
one accumulation-group start/stop per bank (hardware clears has_written at
bank granularity).
"""

import sys

if "/opt/trn_rl_repo" not in sys.path:
    sys.path.insert(0, "/opt/trn_rl_repo")

import numpy as np

import concourse.mybir as mybir
import concourse.tile as tile
from concourse import bacc
from concourse.bass_utils import run_bass_kernel_spmd

F32 = mybir.dt.float32
BF16 = mybir.dt.bfloat16
AF = mybir.ActivationFunctionType

B, S, D, H, HD = 2, 2048, 1024, 16, 64
NCORES = 8
GROUPS = 4            # head groups (cores per batch)
HPG = H // GROUPS     # heads per core = 4
NPAIR = HPG // 2      # head pairs per core = 2
NEG_INF = -1e9
SM_SCALE = 1.0 / float(np.sqrt(HD))  # 0.125

NIT = D // 128        # 8 contraction tiles
NSB = S // 128        # 16 seq blocks
QCH = 512             # attention q-chunk
NCHUNK = S // QCH     # 4
KPC = QCH // 128      # k/q 128-blocks per chunk = 4

_PROG_CACHE = {}


def _build_program(mask_kind: str):
    """mask_kind: 'causal' (trimmed + diag mask) or 'zeros' (full)."""
    causal = mask_kind == "causal"
    nc = bacc.Bacc("TRN2", target_bir_lowering=False, debug=False,
                   num_devices=NCORES)

    xT_d = nc.dram_tensor("xT", [D, S], BF16, kind="ExternalInput").ap()
    # per 128-row block: [wq | wk | wv] column slices
    wqkv_d = nc.dram_tensor("wqkvT", [D, 3 * HPG * HD], BF16,
                            kind="ExternalInput").ap()
    woT_d = nc.dram_tensor("woT", [HPG * HD, D], BF16, kind="ExternalInput").ap()
    cs_d = nc.dram_tensor("cs128", [128, 2 * S], BF16, kind="ExternalInput").ap()
    # [pmat | ident | mdiagT]
    msk_d = nc.dram_tensor("msk", [128, 512], BF16, kind="ExternalInput").ap()
    out_d = nc.dram_tensor("out", [S, D], BF16, kind="ExternalOutput").ap()

    with tile.TileContext(nc) as tc:
        from contextlib import ExitStack

        with ExitStack() as root:
            pers = root.enter_context(tc.tile_pool(name="pers", bufs=1))

            # ---- persistent SBUF tiles ----
            qt = [pers.tile([128, S], BF16, tag=f"qt{p}", name=f"qt{p}")
                  for p in range(NPAIR)]
            kt = [pers.tile([128, S], BF16, tag=f"kt{p}", name=f"kt{p}")
                  for p in range(NPAIR)]
            # V' per (pair, s-block): [128,130] = V_A|ones|V_B|ones
            vp = [[pers.tile([128, 130], BF16, tag=f"vp{p}_{sb}",
                             name=f"vp{p}_{sb}")
                   for sb in range(NSB)] for p in range(NPAIR)]
            # attnT per pair: [vd(2 heads x 64), S]
            at = [pers.tile([128, S], BF16, tag=f"at{p}", name=f"at{p}")
                  for p in range(NPAIR)]
            wo2 = pers.tile([128, NPAIR * D], BF16, tag="wo2", name="wo2")
            wo_t = [wo2[:, p * D:(p + 1) * D] for p in range(NPAIR)]
            msk_t = pers.tile([128, 512], BF16, tag="msk", name="msk")
            pm_t = msk_t[:, 0:128]
            ident_t = msk_t[:, 128:256]
            tri_t = msk_t[:, 256:384]   # 0/1 lower-k triangle (k <= q)
            mdiag_t = msk_t[:, 384:512]  # additive -1e9 upper-k triangle
            ones1 = pers.tile([1, 64], BF16, tag="ones1", name="ones1")

            # all memsets first so the Pool engine is done before attention
            nc.gpsimd.memset(ones1[:], 1.0)
            for p in range(NPAIR):
                for sb in range(NSB):
                    nc.gpsimd.memset(vp[p][sb][:, 64:65], 1.0)
                    nc.gpsimd.memset(vp[p][sb][:, 129:130], 1.0)

            ld = root.enter_context(tc.tile_pool(name="ld", bufs=1))
            sbA = root.enter_context(tc.tile_pool(name="sbA", bufs=2))
            prb = root.enter_context(tc.tile_pool(name="prb", bufs=5))
            asb = root.enter_context(tc.tile_pool(name="asb", bufs=3))
            nrm = root.enter_context(tc.tile_pool(name="nrm", bufs=4))
            osb = root.enter_context(tc.tile_pool(name="osb", bufs=4))
            psS = root.enter_context(
                tc.tile_pool(name="psS", bufs=2, space="PSUM"))
            psPV = root.enter_context(
                tc.tile_pool(name="psPV", bufs=2, space="PSUM"))
            psX = root.enter_context(
                tc.tile_pool(name="psX", bufs=2, space="PSUM"))

            # PE warm-up: dummy matmuls during the otherwise-dead input-DMA
            # window release the HAM clock gate so the first real
            # projections run at full clock
            wt = psX.tile([64, 64], F32, tag="x", name="warm")
            import os as _os
            _wu = int(_os.environ.get("K_WARMUP", "180"))
            for _ in range(_wu):
                nc.tensor.matmul(wt[:], ones1[:], ones1[:],
                                 start=True, stop=True)

            # ---- input DMAs (few, large; wqkv split so it=0 lands early)
            xts = [ld.tile([128, S], BF16, tag=f"xt{it}", name=f"xt{it}")
                   for it in range(NIT)]
            wqkv = ld.tile([128, NIT * 3 * HPG * HD], BF16, tag="wqkv",
                           name="wqkv")
            wq_t = [wqkv[:, it * 768:it * 768 + 256] for it in range(NIT)]
            wk_t = [wqkv[:, it * 768 + 256:it * 768 + 512] for it in range(NIT)]
            wv_t = [wqkv[:, it * 768 + 512:it * 768 + 768] for it in range(NIT)]
            cs_t = ld.tile([128, 2 * S], BF16, tag="cs128", name="cs128")
            c_t = cs_t[:, 0:S]
            s_t = cs_t[:, S:2 * S]

            for half in range(2):
                its = slice(half * 4 * 128, (half + 1) * 4 * 128)
                nc.sync.dma_start(
                    out=wqkv[:, half * 3072:(half + 1) * 3072].rearrange(
                        "p (i c) -> p i c", i=4),
                    in_=wqkv_d[its, :].rearrange("(i p) c -> p i c", p=128))
            for it in range(NIT):
                sl = slice(it * 128, (it + 1) * 128)
                nc.sync.dma_start(out=xts[it][:, 0:1024],
                                  in_=xT_d[sl, 0:1024])
            nc.sync.dma_start(out=msk_t[:], in_=msk_d[:])
            # cos/sin chunks 0-1 now (the chunk-0 ropes need it right after
            # the head); chunks 2-3 after the xB stream — the ch2 ropes are
            # ~20us out and this keeps the early DMA window lean
            nc.sync.dma_start(
                out=cs_t[:].rearrange("p (h c) -> p h c", h=2)[:, :, 0:1024],
                in_=cs_d[:].rearrange("p (h c) -> p h c", h=2)[:, :, 0:1024])
            for it in range(NIT):
                sl = slice(it * 128, (it + 1) * 128)
                nc.sync.dma_start(out=xts[it][:, 1024:2048],
                                  in_=xT_d[sl, 1024:2048])
            nc.sync.dma_start(
                out=cs_t[:].rearrange("p (h c) -> p h c", h=2)[:, :, 1024:2048],
                in_=cs_d[:].rearrange("p (h c) -> p h c", h=2)[:, :, 1024:2048])
            nc.sync.dma_start(
                out=wo2[:].rearrange("p (a d) -> p a d", a=NPAIR),
                in_=woT_d[:].rearrange("(a p) d -> p a d", a=NPAIR, p=128))

            # ---------------- helpers ----------------
            def v_finish(sb, ps):
                """psum [128, 256] (pair-packed V) -> vp tiles, both pairs."""
                for p in range(NPAIR):
                    src = ps[:, p * 128:(p + 1) * 128] \
                        .rearrange("p (b c) -> p b c", b=2, c=64)
                    dst = vp[p][sb][:, 0:130] \
                        .rearrange("p (b c) -> p b c", b=2, c=65)[:, :, 0:64]
                    nc.vector.tensor_copy(dst, src)

            def rope_copy(ps, on_act=True):
                a_sb = sbA.tile([128, 512], BF16, tag="a_sb", name="a_sb")
                if on_act:
                    nc.scalar.activation(a_sb[:], ps[:], AF.Copy)
                else:
                    nc.vector.tensor_copy(a_sb[:], ps[:])
                return a_sb

            def rope_finish(a_sb, dst, p, ch, sw_act=False):
                """rot = A*C + swap(A)*S into dst[p][:, chunk].

                sw_act: evacuate the swap psum on ScalarE so the DVE chain
                is 3 SBUF-only ops (shortest latency; used for the
                transition-critical head units)."""
                qs = slice(ch * 512, (ch + 1) * 512)
                t1 = sbA.tile([128, 512], BF16, tag="t1", name="t1")
                nc.vector.tensor_mul(t1[:], a_sb[:], c_t[:, qs])
                sw = psX.tile([128, 512], F32, tag="x", name="psSW")
                nc.tensor.matmul(sw[:], pm_t[:], a_sb[:],
                                 start=True, stop=True)
                t2 = sbA.tile([128, 512], BF16, tag="t2", name="t2")
                if sw_act:
                    sw_sb = sbA.tile([128, 512], BF16, tag="sw_sb",
                                     name="sw_sb")
                    nc.scalar.activation(sw_sb[:], sw[:], AF.Copy)
                    nc.vector.tensor_mul(t2[:], sw_sb[:], s_t[:, qs])
                else:
                    nc.vector.tensor_mul(t2[:], sw[:], s_t[:, qs])
                nc.vector.tensor_add(dst[p][:, qs], t1[:], t2[:])

            # ---------------- head block ----------------
            # it-major over x cols 0:1024: V s-blocks 0-3 + Q/K chunk 0 for
            # both pairs, so both pairs' chunk-0 attention unlocks first.
            hv = [psPV.tile([128, 512], F32, tag="pv", name=f"hv{b}")
                  for b in range(2)]
            hq = [psS.tile([128, 2 * QCH], F32, tag="sc", name=f"hq{p}")
                  for p in range(NPAIR)]
            for it in range(NIT):
                st, sp = it == 0, it == NIT - 1
                for sb in range(4):
                    b, o = divmod(sb, 2)
                    nc.tensor.matmul(hv[b][:, o * 256:(o + 1) * 256],
                                     xts[it][:, sb * 128:(sb + 1) * 128],
                                     wv_t[it][:],
                                     start=(st and o == 0),
                                     stop=(sp and o == 1))
                for p in range(NPAIR):
                    pc = slice(p * 128, (p + 1) * 128)
                    nc.tensor.matmul(hq[p][:, 0:512], wq_t[it][:, pc],
                                     xts[it][:, 0:512], start=st, stop=sp)
                    nc.tensor.matmul(hq[p][:, 512:1024], wk_t[it][:, pc],
                                     xts[it][:, 0:512], start=st, stop=sp)
            # rope pair 0's chunk-0 q/k now (unblocks the first items);
            # pair 1's chunk 0 becomes the first filler unit.  v_finish
            # after — the first PV only needs vp0 one item later.
            a_q0 = rope_copy(hq[0][:, 0:512])
            a_k0 = rope_copy(hq[0][:, 512:1024])
            rope_finish(a_q0, qt, 0, 0)
            rope_finish(a_k0, kt, 0, 0)
            for sb in range(4):
                b, o = divmod(sb, 2)
                v_finish(sb, hv[b][:, o * 256:(o + 1) * 256])

            # ---------------- filler machinery ----------------
            done_units = set()

            def g_v_unit(sb0):
                """V s-blocks (sb0, sb0+1): packed 2-per-bank accumulation."""
                t = psX.tile([128, 512], F32, tag="x", name=f"v{sb0}")
                for it in range(NIT):
                    st, sp = it == 0, it == NIT - 1
                    for o in range(2):
                        nc.tensor.matmul(
                            t[:, o * 256:(o + 1) * 256],
                            xts[it][:, (sb0 + o) * 128:(sb0 + o + 1) * 128],
                            wv_t[it][:],
                            start=(st and o == 0), stop=(sp and o == 1))
                        yield 107
                v_finish(sb0, t[:, 0:256])
                v_finish(sb0 + 1, t[:, 256:512])
                done_units.add(f"v{sb0}")
                yield 0

            rope_pend = [None]

            def g_p1_head_rope():
                """pair 1's chunk-0 rope (head produced the psum)."""
                a_q1 = rope_copy(hq[1][:, 0:512])
                a_k1 = rope_copy(hq[1][:, 512:1024])
                rope_finish(a_q1, qt, 1, 0)
                done_units.add("q10")
                yield 213
                rope_finish(a_k1, kt, 1, 0)
                done_units.add("k10")
                yield 213

            def g_qk_unit(p, w, ch):
                wt = wq_t if w == "q" else wk_t
                dst = qt if w == "q" else kt
                t = psX.tile([128, 512], F32, tag="x", name=f"qk{p}{w}{ch}")
                pc = slice(p * 128, (p + 1) * 128)
                for it in range(NIT):
                    nc.tensor.matmul(t[:], wt[it][:, pc],
                                     xts[it][:, ch * 512:(ch + 1) * 512],
                                     start=(it == 0), stop=(it == NIT - 1))
                    yield 213
                a_sb = rope_copy(t, on_act=(ch < 2))
                if rope_pend[0] is not None:
                    pa, pdst, pp, pch, pname = rope_pend[0]
                    rope_finish(pa, pdst, pp, pch)
                    done_units.add(pname)
                    yield 213
                rope_pend[0] = (a_sb, dst, p, ch, f"{w}{p}{ch}")

            def flush_pend():
                if rope_pend[0] is not None:
                    pa, pdst, pp, pch, pname = rope_pend[0]
                    rope_finish(pa, pdst, pp, pch)
                    done_units.add(pname)
                    rope_pend[0] = None

            def g_rope_flush():
                if rope_pend[0] is not None:
                    flush_pend()
                    yield 213

            ob_pend = {}

            def emit_wo_oc(sb, oc, tail=False):
                ssl = slice(sb * 128, (sb + 1) * 128)
                osl = slice(oc * 512, (oc + 1) * 512)
                ps = psX.tile([128, 512], F32, tag="x", name="psW")
                for p in range(NPAIR):
                    nc.tensor.matmul(
                        ps[:], at[p][:, ssl], wo_t[p][:, osl],
                        start=(p == 0), stop=(p == NPAIR - 1))
                if sb not in ob_pend:
                    ob_pend[sb] = osb.tile([128, 1024], BF16, tag="osb",
                                           name="osb")
                ob = ob_pend[sb]
                if tail and oc == 1:
                    # ScalarE is idle at the tail: the later half evacs on
                    # it so the two halves run in parallel
                    nc.scalar.activation(ob[:, osl], ps[:], AF.Copy)
                else:
                    nc.vector.tensor_copy(ob[:, osl], ps[:])
                if oc == 1:
                    nc.sync.dma_start(out=out_d[ssl, :],
                                      in_=ob_pend.pop(sb)[:])

            def g_wo_chunk(c):
                for sb in range(c * KPC, (c + 1) * KPC):
                    for oc in range(2):
                        emit_wo_oc(sb, oc)
                        yield 426

            # filler order: chunk-1 q/k (x cols 512:1024, already loaded)
            # before the xB-dependent V blocks; V blocks paced so vp(sb) is
            # ready ~when chunk sb//4's PV needs it.
            fillers = [
                g_p1_head_rope(),
                g_qk_unit(0, "q", 1), g_qk_unit(0, "k", 1),
                g_v_unit(4),
                g_qk_unit(1, "q", 1), g_qk_unit(1, "k", 1),
                g_v_unit(6), g_v_unit(8),
                g_qk_unit(0, "q", 2), g_qk_unit(0, "k", 2),
                g_v_unit(10),
                g_qk_unit(1, "q", 2), g_qk_unit(1, "k", 2),
                g_v_unit(12),
                g_qk_unit(0, "q", 3), g_qk_unit(0, "k", 3),
                g_v_unit(14),
                g_qk_unit(1, "q", 3), g_qk_unit(1, "k", 3),
                g_rope_flush(),
            ]
            fill_iq = [0]

            def pull(budget_ns):
                got = 0
                while fill_iq[0] < len(fillers):
                    g = fillers[fill_iq[0]]
                    try:
                        while got < budget_ns:
                            got += next(g)
                    except StopIteration:
                        fill_iq[0] += 1
                        continue
                    break
                return got

            def pull_until(units):
                while not units <= done_units:
                    # the last missing unit may be parked in the rope pend
                    if rope_pend[0] is not None and \
                            units <= (done_units | {rope_pend[0][4]}):
                        flush_pend()
                        return
                    if pull(600) == 0:
                        flush_pend()
                        assert units <= done_units, (
                            f"filler units {units - done_units} never emitted")
                        return

            # ---------------- attention ----------------
            def emit_scores(p, c, kb, sc, pe_mask=False):
                q0 = c * QCH
                k0 = kb * 128
                trim = max(q0, k0) if causal else q0
                t_off = trim - q0
                on_diag = causal and pe_mask and k0 >= q0
                for h in range(2):
                    hsl = slice(h * 64, (h + 1) * 64)
                    nc.tensor.matmul(
                        sc[:, h * QCH + t_off:(h + 1) * QCH],
                        kt[p][hsl, k0:k0 + 128],
                        qt[p][hsl, trim:q0 + QCH],
                        start=True, stop=not on_diag)
                if on_diag:
                    for h in range(2):
                        nc.tensor.matmul(
                            sc[:, h * QCH + t_off:h * QCH + t_off + 128],
                            ident_t[:], mdiag_t[:],
                            start=False, stop=True)

            def emit_exp(c, kb, sc, pt, pe_mask=False):
                """exp (trimmed); for diag items the upper-k triangle of the
                128-block is zeroed on the idle GPSIMD engine — except on
                the pe_mask path (tail), which folded -1e9 into the scores
                on the PE to keep the exp->PV latency minimal."""
                q0 = c * QCH
                t_off = (max(q0, kb * 128) - q0) if causal else 0
                if t_off == 0:
                    nc.scalar.activation(pt[:, 0:2 * QCH], sc[:, 0:2 * QCH],
                                         AF.Exp, scale=SM_SCALE)
                else:
                    sc3 = sc[:, 0:2 * QCH].rearrange(
                        "p (b c) -> p b c", b=2, c=QCH)[:, :, t_off:]
                    pt3 = pt[:, 0:2 * QCH].rearrange(
                        "p (b c) -> p b c", b=2, c=QCH)[:, :, t_off:]
                    nc.scalar.activation(pt3, sc3, AF.Exp, scale=SM_SCALE)
                if causal and not pe_mask and kb * 128 >= q0:
                    dg = pt[:, 0:2 * QCH].rearrange(
                        "p (b c) -> p b c", b=2,
                        c=QCH)[:, :, t_off:t_off + 128]
                    tri3 = tri_t.rearrange("p (b c) -> p b c", b=1)
                    nc.gpsimd.tensor_mul(dg, dg,
                                         tri3.broadcast_to([128, 2, 128]))

            def emit_pv(p, c, kb, kb_hi, pt, pvt):
                qs_lo = max(0, kb - 4 * c) if causal else 0
                for qs in range(qs_lo, 4):
                    bank, qsl = divmod(qs, 2)
                    last_kb = (4 * c + bank * 2 + 1) if causal else kb_hi - 1
                    for h in range(2):
                        col = qsl * 130 + h * 65
                        nc.tensor.matmul(
                            pvt[bank][:, col:col + 65],
                            pt[:, h * QCH + qs * 128:h * QCH + qs * 128 + 128],
                            vp[p][kb][:, h * 65:h * 65 + 65],
                            start=(kb == 0 and qsl == 0 and h == 0),
                            stop=(kb == last_kb and qsl == 1 and h == 1))

            def emit_norm(j, pvt, attn_sc, h1_act=False):
                """normalize q-slice j of the pair-chunk into attn_sc."""
                bank, qsl = divmod(j, 2)
                rr = nrm.tile([128, 2], F32, tag="rr", name="rr")
                den = pvt[bank][:, qsl * 130:qsl * 130 + 130].rearrange(
                    "p (h c) -> p h c", h=2)[:, :, 64:65]
                nc.vector.reciprocal_approx_fast(
                    rr[:].rearrange("p (h c) -> p h c", c=1), den)
                for h in range(2):
                    dst = attn_sc[:, j * 128 + h * 64:j * 128 + (h + 1) * 64]
                    src = pvt[bank][:,
                                    qsl * 130 + h * 65:qsl * 130 + h * 65 + 64]
                    if h == 1 and h1_act:
                        # tail: h1 on the (by now idle) ScalarE so the two
                        # head halves normalize in parallel
                        nc.scalar.activation(dst, src, AF.Copy,
                                             scale=rr[:, 1:2])
                    else:
                        nc.vector.tensor_scalar_mul(dst, src, rr[:, h:h + 1])

            # chunk processing order 1, 2, 3, 0: the final Act (exp) stretch
            # is the 4-item chunk 0, so the exp stream drains early and the
            # close is PE-dense.  The last two pair-groups run in "tail
            # mode": eager per-q-slice PE transposes + eager wo + immediate
            # ship instead of the DMA-transpose + deferred-wo pipeline.
            pair_jobs = [(0, 0), (0, 1), (1, 0), (1, 1), (2, 0), (2, 1),
                         (3, 0), (3, 1)]
            import os as _os2
            budgets = tuple(int(v) for v in _os2.environ.get(
                "K_BUDGETS", "400,400,400,400,400,400,400,400").split(","))
            dmatp_pend = [None]
            # group gate: only this chunk's q rope; k ropes and V blocks
            # gate per-item below so their filler units can slide into this
            # group's item stream
            need_map = {}
            for c, p in pair_jobs:
                req = set()
                if not (p == 0 and c == 0):
                    req.add(f"q{p}{c}")
                need_map[(c, p)] = req

            for j, (c, p) in enumerate(pair_jobs):
                kb_hi = (c * KPC + KPC) if causal else NSB
                is_final = j == len(pair_jobs) - 1
                tail_mode = is_final
                pull_until(need_map[(c, p)])

                pvt = [psPV.tile([128, 512], F32, tag="pv", name=f"pv{b}")
                       for b in range(2)]
                if tail_mode:
                    attn_sc = asb.tile([128, 512], BF16, tag="af",
                                       name="attn_sf")
                else:
                    attn_sc = asb.tile([128, 512], BF16, tag="asb",
                                       name="attn_sc")

                pend_q = []     # (kb, pt) awaiting PV
                norm_q = []     # q-slices whose PV is emitted, norm pending

                wo_last = {}   # oc -> psum tile with the pair-0 half done

                def flush_norms(p=p, c=c, pvt=pvt, attn_sc=attn_sc,
                                tail_mode=tail_mode, is_final=is_final,
                                norm_q=norm_q, wo_last=wo_last):
                    for jq in norm_q:
                        emit_norm(jq, pvt, attn_sc, h1_act=is_final)
                        if not tail_mode:
                            continue
                        # PE transpose + evac + eager wo + ship
                        tp = psX.tile([128, 512], F32, tag="x", name="tp")
                        tpb = tp.bitcast(BF16)
                        nc.tensor.transpose(
                            tpb[:, 0:128],
                            attn_sc[:, jq * 128:(jq + 1) * 128],
                            ident_t[:])
                        qg = c * KPC + jq
                        nc.vector.tensor_copy(
                            at[p][:, qg * 128:(qg + 1) * 128],
                            tpb[:, 0:128])
                        if jq < 3:
                            emit_wo_oc(qg, 0, tail=True)
                            emit_wo_oc(qg, 1, tail=True)
                        if jq == 2:
                            # pre-run the pair-0 half of the LAST s-block's
                            # wo (at[0] is long ready), so the close after
                            # the last item only pays the pair-1 matmuls.
                            # oc0 borrows the freed qs01 PV bank; oc1 takes
                            # a psX slot (the qs23 PV bank is still live).
                            lsl = slice((qg + 1) * 128, (qg + 2) * 128)
                            for oc in range(1):
                                osl = slice(oc * 512, (oc + 1) * 512)
                                psl = psPV.tile([128, 512], F32,
                                                tag="pv", name="psWL")
                                nc.tensor.matmul(
                                    psl[:], at[0][:, lsl], wo_t[0][:, osl],
                                    start=True, stop=False)
                                wo_last[oc] = psl
                        elif jq == 3:
                            lsl = slice(qg * 128, (qg + 1) * 128)
                            ob = osb.tile([128, 1024], BF16, tag="osb",
                                          name="osb")
                            nc.tensor.matmul(
                                wo_last[0][:], at[1][:, lsl],
                                wo_t[1][:, 0:512], start=False, stop=True)
                            nc.vector.tensor_copy(ob[:, 0:512],
                                                  wo_last[0][:])
                            ps1 = psX.tile([128, 512], F32, tag="x",
                                           name="psW15")
                            for pp in range(NPAIR):
                                nc.tensor.matmul(
                                    ps1[:], at[pp][:, lsl],
                                    wo_t[pp][:, 512:1024],
                                    start=(pp == 0), stop=(pp == 1))
                            nc.scalar.activation(ob[:, 512:1024], ps1[:],
                                                 AF.Copy)
                            nc.sync.dma_start(out=out_d[lsl, :], in_=ob[:])
                    del norm_q[:]

                def drain_one(p=p, c=c, kb_hi=kb_hi, pvt=pvt,
                              pend_q=pend_q, norm_q=norm_q,
                              flush_norms=flush_norms):
                    pkb, ppt = pend_q.pop(0)
                    if pkb >= 4:
                        vname = f"v{pkb & ~1}"
                        if vname not in done_units:
                            pull_until({vname})
                    emit_pv(p, c, pkb, kb_hi, ppt, pvt)
                    if causal and pkb >= 4 * c:
                        norm_q.append(pkb - 4 * c)
                    flush_norms()

                budget = budgets[j]
                if j == 0:
                    # the first scores wait on the head rope chain; emit a
                    # burst of (rope-independent) filler first so the
                    # in-order PE queue isn't parked behind that wait
                    pull(int(_os2.environ.get("K_PREPULL", "3200")))
                for kb in range(kb_hi):
                    kc = kb // 4
                    if kc >= 1 or p == 1:
                        kname = f"k{p}{kc}"
                        if kname not in done_units:
                            pull_until({kname})
                    sc = psS.tile([128, 2 * QCH], F32, tag="sc", name="sc")
                    pt = prb.tile([128, 2 * QCH], BF16, tag="prb", name="prb")
                    emit_scores(p, c, kb, sc, pe_mask=is_final)
                    emit_exp(c, kb, sc, pt, pe_mask=is_final)
                    if kb == 2 and dmatp_pend[0] is not None:
                        # previous pair-chunk's attn transpose: deferred here
                        # so its sem wait is satisfied on arrival and doesn't
                        # head-of-line block the SP DMA queue
                        dmatp_pend[0]()
                        dmatp_pend[0] = None
                    pull(budget)
                    # drain pending PVs; a diag item's PV is held one extra
                    # item so the GPSIMD triangle-mask round trip is hidden
                    pvdepth = int(_os2.environ.get("K_PVDEPTH", "1"))
                    while pend_q:
                        diag0 = (causal and not is_final
                                 and pend_q[0][0] >= 4 * c)
                        if len(pend_q) < pvdepth + (1 if diag0 else 0):
                            break
                        drain_one()
                    pend_q.append((kb, pt))
                # drain remaining PVs + norms
                while pend_q:
                    pull(int(_os2.environ.get("K_DRAINPULL", "500")))
                    drain_one()
                if not causal:
                    norm_q.extend(range(4))
                    flush_norms()

                if not tail_mode:
                    # blocked transpose of the whole pair-chunk into at[p];
                    # emission deferred into the next pair-group
                    def mk_tp(p=p, c=c, attn_sc=attn_sc):
                        def emit():
                            nc.sync.dma_start_transpose(
                                at[p][:, c * QCH:(c + 1) * QCH].rearrange(
                                    "v (b q) -> v b q", b=4),
                                attn_sc[:])
                        return emit

                    dmatp_pend[0] = mk_tp()
                    if p == NPAIR - 1:
                        fillers.append(g_wo_chunk(c))

            # leftover fillers (late wo chunks)
            pull(10**12)

    nc.compile()
    return nc


def _host_prep(x, freqs_cos, freqs_sin, wq, wk, wv, wo):
    """Build the 8 per-core input maps (numpy, bf16)."""
    import ml_dtypes

    bf16 = ml_dtypes.bfloat16

    x = np.ascontiguousarray(x, dtype=np.float32)
    cosT = np.ascontiguousarray(freqs_cos.T, dtype=np.float32)  # [32, S]
    sinT = np.ascontiguousarray(freqs_sin.T, dtype=np.float32)

    c128 = np.tile(cosT, (4, 1))                                # [128, S]
    s128 = np.tile(np.concatenate([-sinT, sinT], 0), (2, 1))
    cs128 = np.ascontiguousarray(
        np.concatenate([c128, s128], axis=1)).astype(bf16)      # [128, 2S]
    # swap permutation: psum_sw = pmat.T @ A -> sw[m] = A[sigma(m)],
    # sigma swaps the 32-halves within each 64 block.
    pmat = np.zeros((128, 128), dtype=np.float32)
    for m in range(128):
        blk, off = divmod(m, 32)
        pmat[(blk ^ 1) * 32 + off, m] = 1.0
    ident = np.eye(128, dtype=np.float32)
    # causal diag 0/1 triangle: tri01[k, q] = 1 if k <= q else 0
    kk, qq = np.meshgrid(np.arange(128), np.arange(128), indexing="ij")
    tri01 = (kk <= qq).astype(np.float32)
    mdiagT = np.where(kk <= qq, 0.0, NEG_INF).astype(np.float32)
    msk = np.ascontiguousarray(
        np.concatenate([pmat, ident, tri01, mdiagT], axis=1)).astype(bf16)

    # rotate-half row permutation within each head
    rh = np.concatenate([np.arange(0, HD, 2), np.arange(1, HD, 2)])

    xT = [np.ascontiguousarray(x[b].T).astype(bf16) for b in range(B)]

    in_maps = []
    for core in range(NCORES):
        b, g = divmod(core, GROUPS)
        heads = [g * HPG + j for j in range(HPG)]
        qrows, vrows = [], []
        for h in heads:
            base = h * HD
            qrows.extend((base + rh).tolist())
            vrows.extend(range(base, base + HD))
        qrows = np.array(qrows)
        vrows = np.array(vrows)
        wqT = wq[qrows, :].T                                     # [D, 256]
        wkT = wk[qrows, :].T
        wvT = wv[vrows, :].T
        wqkvT = np.ascontiguousarray(
            np.concatenate([wqT, wkT, wvT], axis=1)).astype(bf16)
        woT = np.ascontiguousarray(wo[:, vrows].T).astype(bf16)  # [256, D]
        in_maps.append({
            "xT": xT[b], "wqkvT": wqkvT, "woT": woT,
            "cs128": cs128, "msk": msk,
        })
    return in_maps


def _mask_kind(mask):
    m = np.asarray(mask).reshape(S, S)
    if not np.any(m):
        return "zeros"
    qq, kk = np.meshgrid(np.arange(S), np.arange(S), indexing="ij")
    causal = np.where(kk <= qq, 0.0, NEG_INF).astype(np.float32)  # [q, k]
    if np.array_equal(m, causal):
        return "causal"
    return "general"


def _reference_host(x, freqs_cos, freqs_sin, mask, wq, wk, wv, wo):
    """Correctness fallback for arbitrary masks (host numpy, float64)."""
    b, s, d = x.shape
    hd = d // H
    xq = (x @ wq.T).reshape(b, s, H, hd)
    xk = (x @ wk.T).reshape(b, s, H, hd)
    xv = (x @ wv.T).reshape(b, s, H, hd)

    def rope(t):
        tr = t.reshape(b, s, H, hd // 2, 2)
        t0, t1 = tr[..., 0], tr[..., 1]
        cos = freqs_cos[None, :, None, :]
        sin = freqs_sin[None, :, None, :]
        return np.stack([t0 * cos - t1 * sin, t0 * sin + t1 * cos],
                        -1).reshape(b, s, H, hd)

    xq, xk = rope(xq), rope(xk)
    sc = np.einsum("bqhd,bkhd->bhqk", xq, xk) / np.sqrt(hd) + mask
    sc = sc - sc.max(-1, keepdims=True)
    e = np.exp(sc)
    pr = e / e.sum(-1, keepdims=True)
    o = np.einsum("bhqk,bkhd->bqhd", pr, xv).reshape(b, s, d)
    return (o @ wo.T).astype(np.float32)


def kernel(x, freqs_cos, freqs_sin, mask, wq, wk, wv, wo):
    kind = _mask_kind(mask)
    if kind == "general":
        return _reference_host(np.asarray(x, np.float64),
                               np.asarray(freqs_cos, np.float64),
                               np.asarray(freqs_sin, np.float64),
                               np.asarray(mask, np.float64),
                               np.asarray(wq, np.float64),
                               np.asarray(wk, np.float64),
                               np.asarray(wv, np.float64),
                               np.asarray(wo, np.float64))

    if kind not in _PROG_CACHE:
        _PROG_CACHE[kind] = _build_program(kind)
    nc = _PROG_CACHE[kind]

    in_maps = _host_prep(np.asarray(x, np.float32),
                         np.asarray(freqs_cos, np.float32),
                         np.asarray(freqs_sin, np.float32),
                         np.asarray(wq, np.float32),
                         np.asarray(wk, np.float32),
                         np.asarray(wv, np.float32),
                         np.asarray(wo, np.float32))
    res = run_bass_kernel_spmd(nc, in_maps, list(range(NCORES)))
    out = np.zeros((B, S, D), dtype=np.float32)
    for core in range(NCORES):
        out[core // GROUPS] += np.asarray(res.results[core]["out"],
                                          dtype=np.float32)
    return out
